# revision 10
# baseline (speedup 1.0000x reference)
"""Trainium2 Bass kernel for nn_CharTaggerBiLSTM, 8-core SPMD, 3 launches.

L1 char LSTM: data-parallel over batch (16 sentences/core). Transposed
   layout (features-on-partitions), f32r matmuls; emits the masked last
   hidden state per word -> DRAM.
L2 word LSTM: one direction per core (cores 0-3 forward, 4-7 backward),
   32 sentences/core so each weight stream serves twice the rows.
   Direction is data: backward cores receive the char outputs with the
   sentence axis reversed on host and their outputs are un-reversed.
   x-part/bias matmuls for step s+1 are issued during step s's
   elementwise work to keep PE fed.
L3 MLP + log_softmax: data-parallel (16 sentences/core), bf16 GEMMs.

Host does embedding gather, weight reshapes, the two reshard steps, and
reassembly.
"""

import sys
import functools
from contextlib import ExitStack

sys.path.insert(0, "/opt/trn_rl_repo")

import numpy as np
import ml_dtypes
from concourse import bacc, bass, mybir, tile, bass_utils

BF_NP = ml_dtypes.bfloat16
U8 = mybir.dt.uint8
F8T = mybir.dt.float8e4
DR = mybir.MatmulPerfMode.DoubleRow
E = 64


B, S, Lc = 128, 128, 20
AB, E = 100, 64
Hc, H, OUT = 256, 512, 50
NCORE = 8
BL = B // NCORE            # sentences per core in L1/L3
FP = mybir.dt.float32
FR = mybir.dt.float32r
BF = mybir.dt.bfloat16
G4 = 4 * Hc
WG = 4 * H

Sig = mybir.ActivationFunctionType.Sigmoid
TanhF = mybir.ActivationFunctionType.Tanh
ReluF = mybir.ActivationFunctionType.Relu
ExpF = mybir.ActivationFunctionType.Exp
LnF = mybir.ActivationFunctionType.Ln
IdentF = mybir.ActivationFunctionType.Identity


def build_l1(bl=BL, lmin=18):
    """Char LSTM, data-parallel; writes lastT [2,128,nl] bf16 to DRAM."""
    nl = bl * S
    nc = bacc.Bacc("TRN2", target_bir_lowering=False, debug=False,
                   num_devices=NCORE)
    d_eT = nc.dram_tensor("eT", [Lc, E, nl], BF, kind="ExternalInput")
    d_lenrep = nc.dram_tensor("lenrep", [128, nl], BF, kind="ExternalInput")
    d_cWx2 = nc.dram_tensor("cWx2", [128, 4, 128], BF, kind="ExternalInput")
    d_cWhT = nc.dram_tensor("cWhT", [2, 128, G4], BF, kind="ExternalInput")
    d_cbias = nc.dram_tensor("cbias", [128, G4 // 128], FP,
                             kind="ExternalInput")
    d_last = nc.dram_tensor("lastT", [2, 128, nl], BF, kind="ExternalOutput")

    CH = 1024
    NCH = nl // CH
    GF = [Sig, Sig, TanhF, Sig]          # gate funcs for gi = i, f, g, o

    with tile.TileContext(nc) as tc:
        with ExitStack() as c1:
            cw = c1.enter_context(tc.tile_pool(name="cweights", bufs=1))
            cst = c1.enter_context(tc.tile_pool(name="cstate", bufs=1))
            ein = c1.enter_context(tc.tile_pool(name="ein", bufs=2))
            ctmp = c1.enter_context(tc.tile_pool(name="ctmp", bufs=3))
            cps = c1.enter_context(tc.tile_pool(name="cpsum", bufs=4,
                                                space="PSUM"))
            cWx2 = cw.tile([128, 4, 128], BF, tag="cWx2", name="cWx2")
            cWh = cw.tile([128, 2, G4], BF, tag="cWh", name="cWh")
            cb = cw.tile([128, G4 // 128], FP, tag="cb", name="cb")
            lenr = cw.tile([128, nl], BF, tag="lenr", name="lenr")
            nc.sync.dma_start(cWx2[:], d_cWx2.ap()[:])
            nc.sync.dma_start(cWh[:], d_cWhT.ap().rearrange("k p g -> p k g"))
            nc.sync.dma_start(cb[:], d_cbias.ap()[:])
            nc.sync.dma_start(lenr[:], d_lenrep.ap()[:])

            last = cst.tile([128, 2, nl], BF, tag="last", name="last")
            hh = [cst.tile([128, 2, nl], BF, tag=f"h{p}", name=f"h{p}")
                  for p in range(2)]
            cc = cst.tile([128, 2, nl], BF, tag="cc", name="cc")
            nc.vector.memset(cc[:], 0.0)
            nc.vector.memset(last[:], 0.0)

            for t in range(Lc):
                et2 = ein.tile([128, nl], BF, tag="et2", name="et2")
                nc.sync.dma_start(et2[0:E, :], d_eT.ap()[t])
                nc.sync.dma_start(et2[E:128, :], d_eT.ap()[t])
                masked = t >= lmin - 1
                hprev = hh[t % 2]
                hcur = hh[(t + 1) % 2]
                for ci in range(NCH):
                    cs = slice(ci * CH, (ci + 1) * CH)
                    if masked:
                        mk = ctmp.tile([128, CH], U8, tag="mk", name="mk")
                        nc.gpsimd.tensor_scalar(mk[:], lenr[:, cs], float(t),
                                                None,
                                                op0=mybir.AluOpType.is_gt)
                    for j in range(2):
                        ps = [cps.tile([128, CH], FP, tag="ps", name="ps")
                              for _ in range(4)]
                        # x-part: two K=64 row strips per PE pass, plus
                        # bias-free accumulation of the two h chunks;
                        # matmul outputs are split into 512-col halves
                        # (one PSUM bank each)
                        for hw_ in range(2):
                            o5 = slice(hw_ * 512, (hw_ + 1) * 512)
                            c5 = slice(ci * CH + hw_ * 512,
                                       ci * CH + (hw_ + 1) * 512)
                            for pi in range(2):
                                sl = 2 * j + pi
                                nc.tensor.matmul(ps[2 * pi][:, o5],
                                                 cWx2[0:E, sl, :],
                                                 et2[0:E, c5],
                                                 start=True, stop=(t == 0))
                                nc.tensor.matmul(ps[2 * pi + 1][:, o5],
                                                 cWx2[E:128, sl, :],
                                                 et2[E:128, c5],
                                                 start=True, stop=(t == 0))
                            if t > 0:
                                for gi in range(4):
                                    m = 2 * gi + j
                                    for k in range(2):
                                        nc.tensor.matmul(
                                            ps[gi][:, o5],
                                            cWh[:, k, m * 128:(m + 1) * 128],
                                            hprev[:, k, c5],
                                            start=False, stop=(k == 1))
                        gsb = [ctmp.tile([128, CH], BF, tag=f"g{gi}",
                                         name=f"g{gi}") for gi in range(4)]
                        for gi in range(4):
                            m = 2 * gi + j
                            nc.scalar.activation(gsb[gi][:], ps[gi][:],
                                                 GF[gi], bias=cb[:, m:m + 1])
                        ig = ctmp.tile([128, CH], BF, tag="ig", name="ig")
                        nc.vector.tensor_mul(ig[:], gsb[0][:], gsb[2][:])
                        fc = ctmp.tile([128, CH], BF, tag="fc", name="fc")
                        nc.vector.tensor_mul(fc[:], gsb[1][:], cc[:, j, cs])
                        nc.vector.tensor_add(cc[:, j, cs], fc[:], ig[:])
                        tct = ctmp.tile([128, CH], BF, tag="tct", name="tct")
                        nc.scalar.activation(tct[:], cc[:, j, cs], TanhF)
                        nc.vector.tensor_mul(hcur[:, j, cs], gsb[3][:],
                                             tct[:])
                        if masked:
                            nc.vector.copy_predicated(last[:, j, cs], mk[:],
                                                      hcur[:, j, cs])
            for j in range(2):
                nc.sync.dma_start(d_last.ap()[j], last[:, j, :])
    nc.compile()
    return nc


def build_l2_v4(bl2=32):
    """Word LSTM v4: gates-on-partitions, sentences-moving.

    Per step: psum tile [128, 16, 32] f32 (1 bank) holds all 2048 gates
    (16 blocks of 128 gate-dims on partitions) x 32 sentences on free.
    Seeded by bias (K=1 bf16) + x-part (fp8 DR vs last char state), then
    h-part (fp8 DR vs h^T ring) accumulates. One sigmoid act covers all
    gates (tanh(g) folded as 2*sig(2x)-1 with the 2x pre-scaled into the
    g rows host-side); cell tail on DVE; h^T ring written directly by
    the o*tanh(c) mul - no transposes, no inject.
    Gate type order: f, i, o, g (blocks 0:4, 4:8, 8:12, 12:16).
    """
    nl = bl2 * S
    nc = bacc.Bacc("TRN2", target_bir_lowering=False, debug=False,
                   num_devices=NCORE)
    d_lastT = nc.dram_tensor("lastT8", [128, 2, S, bl2], F8T,
                             kind="ExternalInput")
    d_wh8 = nc.dram_tensor("wh8v4", [128, 2, 16, 2, 128], F8T,
                           kind="ExternalInput")
    d_wx8 = nc.dram_tensor("wx8v4", [128, 16, 2, 128], F8T,
                           kind="ExternalInput")
    d_wb = nc.dram_tensor("wb16", [1, 16, 128], BF, kind="ExternalInput")
    d_ones = nc.dram_tensor("ones32", [1, bl2], BF, kind="ExternalInput")
    d_hs = nc.dram_tensor("hsTh", [4, 128, nl], BF, kind="ExternalOutput")

    with tile.TileContext(nc) as tc:
        with ExitStack() as c2:
            ww = c2.enter_context(tc.tile_pool(name="wweights", bufs=1))
            wst = c2.enter_context(tc.tile_pool(name="wstate", bufs=1))
            rgp = c2.enter_context(tc.tile_pool(name="wring", bufs=2))
            cpl = c2.enter_context(tc.tile_pool(name="wcell", bufs=2))
            sgp = c2.enter_context(tc.tile_pool(name="wsg", bufs=2))
            wtmp = c2.enter_context(tc.tile_pool(name="wtmp", bufs=2))
            wps = c2.enter_context(tc.tile_pool(name="wpsum", bufs=3,
                                                space="PSUM"))
            wh = ww.tile([128, 2, 16, 2, 128], F8T, tag="wh", name="wh")
            wx = ww.tile([128, 16, 2, 128], F8T, tag="wx", name="wx")
            wb = ww.tile([1, 16, 128], BF, tag="wb", name="wb")
            ones = ww.tile([1, bl2], BF, tag="ones", name="ones")
            lastT = ww.tile([128, 2, S, bl2], BF, tag="lastT", name="lastT")
            nc.sync.dma_start(wh[:], d_wh8.ap()[:])
            nc.sync.dma_start(wx[:], d_wx8.ap()[:])
            nc.sync.dma_start(wb[:], d_wb.ap()[:])
            nc.sync.dma_start(ones[:], d_ones.ap()[:])
            for sc in range(4):
                ss = slice(sc * (S // 4), (sc + 1) * (S // 4))
                nc.sync.dma_start(lastT[:, :, ss, :], d_lastT.ap()[:, :, ss, :])
            hsT = wst.tile([128, 4, S, bl2], BF, tag="hsT", name="hsT")

            rinit = [rgp.tile([128, 4, bl2], F8T, tag="ring", name=f"ri{i}")
                     for i in range(2)]
            for t_ in rinit:
                nc.vector.memset(t_[:], 0.0)
            ring_prev = rinit[1]
            cinit = [cpl.tile([128, 4, bl2], BF, tag="cc", name=f"ci{i}")
                     for i in range(2)]
            nc.vector.memset(cinit[1][:], 0.0)
            c_prev = cinit[1]

            for s in range(S):
                ps = wps.tile([128, 16, bl2], FP, tag="ps", name="ps")
                for gb in range(16):
                    nc.tensor.matmul(ps[:, gb, :], wb[:, gb, :], ones[:],
                                     start=True, stop=False,
                                     skip_group_check=True)
                for gb in range(16):
                    nc.tensor.matmul(ps[:, gb, :], wx[:, gb, :, :],
                                     lastT[:, :, s, :],
                                     start=False, stop=(s == 0),
                                     perf_mode=DR, skip_group_check=True)
                if s > 0:
                    for gb in range(16):
                        for q in range(2):
                            nc.tensor.matmul(
                                ps[:, gb, :], wh[:, q, gb, :, :],
                                ring_prev[:, 2 * q:2 * q + 2, :],
                                start=False, stop=(q == 1),
                                perf_mode=DR, skip_group_check=True)
                sg = sgp.tile([128, 16, bl2], BF, tag="sg", name="sg")
                nc.scalar.activation(sg[:], ps[:], Sig, scale=1.0 / 16.0)
                fco = wtmp.tile([128, 4, bl2], BF, tag="fco", name="fco")
                nc.vector.tensor_mul(fco[:], sg[:, 0:4, :], c_prev[:])
                tg = wtmp.tile([128, 4, bl2], BF, tag="tg", name="tg")
                nc.vector.scalar_tensor_tensor(
                    tg[:], sg[:, 12:16, :], 0.5, sg[:, 4:8, :],
                    op0=mybir.AluOpType.subtract, op1=mybir.AluOpType.mult)
                c_new = cpl.tile([128, 4, bl2], BF, tag="cc", name="cc")
                nc.vector.scalar_tensor_tensor(
                    c_new[:], tg[:], 2.0, fco[:],
                    op0=mybir.AluOpType.mult, op1=mybir.AluOpType.add)
                tct = wtmp.tile([128, 4, bl2], BF, tag="tct", name="tct")
                nc.scalar.activation(tct[:], c_new[:], TanhF)
                ring_new = rgp.tile([128, 4, bl2], F8T, tag="ring",
                                    name="ring")
                nc.vector.tensor_mul(ring_new[:], sg[:, 8:12, :], tct[:])
                nc.gpsimd.tensor_mul(hsT[:, :, s, :], sg[:, 8:12, :], tct[:])
                ring_prev = ring_new
                c_prev = c_new
            nc.sync.dma_start(
                d_hs.ap().rearrange("k p (s b) -> p k s b", b=bl2), hsT[:])
    nc.compile()
    return nc


def build_l2_v5(bl2=32, ngrp=2):
    """Word LSTM v5: like v4 but sentences split into ngrp interleaved
    groups with independent recurrence chains, so each group's
    (smaller) elementwise ops overlap the other group's matmuls."""
    nl = bl2 * S
    gw = bl2 // ngrp                 # sentences per group
    nc = bacc.Bacc("TRN2", target_bir_lowering=False, debug=False,
                   num_devices=NCORE)
    d_lastT = nc.dram_tensor("lastT16", [128, 2, S, bl2], BF,
                             kind="ExternalInput")
    d_wh8 = nc.dram_tensor("wh8v4", [128, 2, 16, 2, 128], F8T,
                           kind="ExternalInput")
    d_wx8 = nc.dram_tensor("wx16v4", [128, 16, 2, 128], BF,
                           kind="ExternalInput")
    d_wb = nc.dram_tensor("wb16", [1, 16, 128], BF, kind="ExternalInput")
    d_ones = nc.dram_tensor("ones32", [1, bl2], BF, kind="ExternalInput")
    d_hs = nc.dram_tensor("hsTh", [4, 128, nl], BF, kind="ExternalOutput")

    with tile.TileContext(nc) as tc:
        with ExitStack() as c2:
            ww = c2.enter_context(tc.tile_pool(name="wweights", bufs=1))
            wst = c2.enter_context(tc.tile_pool(name="wstate", bufs=1))
            rgp = c2.enter_context(tc.tile_pool(name="wring", bufs=2 * ngrp))
            cpl = c2.enter_context(tc.tile_pool(name="wcell", bufs=2 * ngrp))
            sgp = c2.enter_context(tc.tile_pool(name="wsg", bufs=2 * ngrp))
            wtmp = c2.enter_context(tc.tile_pool(name="wtmp", bufs=2 * ngrp))
            wps = c2.enter_context(tc.tile_pool(name="wpsum", bufs=3,
                                                space="PSUM"))
            wh = ww.tile([128, 2, 16, 2, 128], F8T, tag="wh", name="wh")
            wx = ww.tile([128, 16, 2, 128], BF, tag="wx", name="wx")
            wb = ww.tile([1, 16, 128], BF, tag="wb", name="wb")
            ones = ww.tile([1, bl2], BF, tag="ones", name="ones")
            lastT = ww.tile([128, 2, S, bl2], BF, tag="lastT", name="lastT")
            nc.sync.dma_start(wh[:], d_wh8.ap()[:])
            nc.sync.dma_start(wx[:], d_wx8.ap()[:])
            nc.sync.dma_start(wb[:], d_wb.ap()[:])
            nc.sync.dma_start(ones[:], d_ones.ap()[:])
            for sc in range(4):
                ss = slice(sc * (S // 4), (sc + 1) * (S // 4))
                nc.sync.dma_start(lastT[:, :, ss, :], d_lastT.ap()[:, :, ss, :])
            hsT = wst.tile([128, 4, S, bl2], BF, tag="hsT", name="hsT")

            c_prev, ring_prev = [], []
            for g in range(ngrp):
                ct = cpl.tile([128, 4, gw], BF, tag=f"cc{g}", name=f"ci{g}")
                nc.vector.memset(ct[:], 0.0)
                c_prev.append(ct)
                ring_prev.append(None)

            def step_mms(g, s):
                # psum tile is a full 2KB bank: start=True zeroes the whole
                # bank, so exactly one matmul (first bias) carries start.
                gs = slice(g * gw, (g + 1) * gw)
                ps = wps.tile([128, 16, 32], FP, tag=f"ps{g}", name=f"ps{g}")
                for gb in range(16):
                    nc.tensor.matmul(ps[:, gb, 0:gw], wb[:, gb, :],
                                     ones[:, gs],
                                     start=(gb == 0), stop=False,
                                     skip_group_check=True)
                for gb in range(16):
                    for r in range(2):
                        nc.tensor.matmul(ps[:, gb, 0:gw], wx[:, gb, r, :],
                                         lastT[:, r, s, gs],
                                         start=False,
                                         stop=(s == 0 and gb == 15
                                               and r == 1),
                                         skip_group_check=True)
                if s > 0:
                    for gb in range(16):
                        for q in range(2):
                            nc.tensor.matmul(
                                ps[:, gb, 0:gw], wh[:, q, gb, :, :],
                                ring_prev[g][:, 2 * q:2 * q + 2, :],
                                start=False,
                                stop=(gb == 15 and q == 1),
                                perf_mode=DR, skip_group_check=True)
                return ps

            def step_tail(g, s, ps):
                gs = slice(g * gw, (g + 1) * gw)
                sg = sgp.tile([128, 16, gw], BF, tag=f"sg{g}", name=f"sg{g}")
                nc.scalar.activation(sg[:], ps[:, :, 0:gw], Sig,
                                     scale=1.0 / 16.0)
                fco = wtmp.tile([128, 4, gw], BF, tag=f"fco{g}",
                                name=f"fco{g}")
                nc.vector.tensor_mul(fco[:], sg[:, 0:4, :], c_prev[g][:])
                tg = wtmp.tile([128, 4, gw], BF, tag=f"tg{g}", name=f"tg{g}")
                nc.vector.scalar_tensor_tensor(
                    tg[:], sg[:, 12:16, :], 0.5, sg[:, 4:8, :],
                    op0=mybir.AluOpType.subtract, op1=mybir.AluOpType.mult)
                c_new = cpl.tile([128, 4, gw], BF, tag=f"cc{g}",
                                 name=f"cc{g}")
                nc.vector.scalar_tensor_tensor(
                    c_new[:], tg[:], 2.0, fco[:],
                    op0=mybir.AluOpType.mult, op1=mybir.AluOpType.add)
                tct = wtmp.tile([128, 4, gw], BF, tag=f"tct{g}",
                                name=f"tct{g}")
                nc.scalar.activation(tct[:], c_new[:], TanhF)
                ring_new = rgp.tile([128, 4, gw], F8T, tag=f"ring{g}",
                                    name=f"ring{g}")
                nc.vector.tensor_mul(ring_new[:], sg[:, 8:12, :], tct[:])
                nc.gpsimd.tensor_mul(hsT[:, :, s, gs], sg[:, 8:12, :],
                                     tct[:])
                ring_prev[g] = ring_new
                c_prev[g] = c_new

            pend = {}
            for s in range(S):
                for g in range(ngrp):
                    pend[g] = step_mms(g, s)
                    og = (g + 1) % ngrp
                    if (og, 'tail') in pend:
                        gg, ss, pp = pend.pop((og, 'tail'))
                        step_tail(gg, ss, pp)
                    pend[(g, 'tail')] = (g, s, pend[g])
            for g in range(ngrp):
                if (g, 'tail') in pend:
                    gg, ss, pp = pend.pop((g, 'tail'))
                    step_tail(gg, ss, pp)
            nc.sync.dma_start(
                d_hs.ap().rearrange("k p (s b) -> p k s b", b=bl2), hsT[:])
    nc.compile()
    return nc


def build_l2(bl2=32, fp8=True):
    """Word LSTM v3: fp8 DoubleRow via half-pad windows, step-major."""
    nl = bl2 * S
    nc = bacc.Bacc("TRN2", target_bir_lowering=False, debug=False,
                   num_devices=NCORE)
    d_last = nc.dram_tensor("lastT2", [2, 128, nl], BF, kind="ExternalInput")
    d_wIT = nc.dram_tensor("wIT", [2, 128, WG], BF, kind="ExternalInput")
    d_wb = nc.dram_tensor("wb", [1, WG], BF, kind="ExternalInput")
    d_ones = nc.dram_tensor("onesr", [1, 128], BF, kind="ExternalInput")
    d_eyeb = nc.dram_tensor("eyeb", [128, 32], BF, kind="ExternalInput")
    d_scl = nc.dram_tensor("scl64", [64, 1], FP, kind="ExternalInput")
    d_wh = nc.dram_tensor("wh8", [2, 128, 2, WG], F8T, kind="ExternalInput")
    d_hs = nc.dram_tensor("hsTh", [4, 128, nl], BF, kind="ExternalOutput")
    NT = nl // 128
    IdF = mybir.ActivationFunctionType.Identity

    with tile.TileContext(nc) as tc:
        with ExitStack() as c2:
            ww = c2.enter_context(tc.tile_pool(name="wweights", bufs=1))
            wst = c2.enter_context(tc.tile_pool(name="wstate", bufs=1))
            wtmp = c2.enter_context(tc.tile_pool(name="wtmp", bufs=3))
            eyeb = ww.tile([128, 32], BF, tag="eyeb", name="eyeb")
            nc.sync.dma_start(eyeb[:], d_eyeb.ap()[:])
            ones = ww.tile([1, 128], BF, tag="ones", name="ones")
            nc.sync.dma_start(ones[:], d_ones.ap()[:])
            wbt = ww.tile([1, WG], BF, tag="wbt", name="wbt")
            nc.sync.dma_start(wbt[:], d_wb.ap()[:])
            scl = ww.tile([64, 1], FP, tag="scl", name="scl")
            nc.sync.dma_start(scl[:], d_scl.ap()[:])
            wh = ww.tile([128, 2, 2, WG], F8T, tag="wh", name="wh")
            nc.sync.dma_start(wh[:],
                              d_wh.ap().rearrange("q p i g -> p q i g"))
            xt = wst.tile([128, NT, WG], BF, tag="xt", name="xt")
            hsT = wst.tile([128, 4, S, bl2], BF, tag="hsT", name="hsT")

            lw = c2.enter_context(tc.tile_pool(name="lw", bufs=1))
            psA = c2.enter_context(tc.tile_pool(name="psA", bufs=2,
                                                space="PSUM"))
            lpool = c2.enter_context(tc.tile_pool(name="lpool", bufs=3))
            wIT = lw.tile([128, 2, WG], BF, tag="wIT", name="wIT")
            nc.sync.dma_start(wIT[:],
                              d_wIT.ap().rearrange("k p g -> p k g"))

            def emit_a(tt):
                ts = slice(tt * 128, (tt + 1) * 128)
                lt = lpool.tile([128, 2, 128], BF, tag="lt", name="lt")
                for j2 in range(2):
                    nc.sync.dma_start(lt[:, j2, :], d_last.ap()[j2][:, ts])
                for nch in range(4):
                    sl = slice(nch * 512, (nch + 1) * 512)
                    px = psA.tile([128, 512], FP, tag="px", name="px")
                    nc.tensor.matmul(px[:], ones[:, 0:128], wbt[:, sl],
                                     start=True, stop=False)
                    for j2 in range(2):
                        nc.tensor.matmul(px[:], lt[:, j2, :], wIT[:, j2, sl],
                                         start=False, stop=(j2 == 1))
                    if nch < 2:
                        nc.vector.tensor_scalar(xt[:, tt, sl], px[:], 16.0,
                                                None,
                                                op0=mybir.AluOpType.mult)
                    else:
                        nc.scalar.activation(xt[:, tt, sl], px[:], IdF,
                                             scale=16.0)

            APRE = 3
            for tt in range(APRE):
                emit_a(tt)

            wps = c2.enter_context(tc.tile_pool(name="wpsum", bufs=2,
                                                space="PSUM"))
            wpt = c2.enter_context(tc.tile_pool(name="wpt", bufs=2,
                                                space="PSUM"))
            rgp = c2.enter_context(tc.tile_pool(name="wring", bufs=1))
            # fp8 ring: h^T lives at cols 32-63 of a zero-padded window
            # tile; shifted 64-wide windows stack two gates per DR output
            rlist = []
            for ri in range(3):
                rt = rgp.tile([128, 4, 96], F8T, tag=f"r{ri}",
                              name=f"r{ri}")
                nc.vector.memset(rt[:], 0.0)
                rlist.append(rt)
            c32 = wst.tile([32, 512], BF, tag="c32", name="c32")
            nc.vector.memset(c32[:], 0.0)

            banks = {}

            def emit_inject(s):
                tt, so = divmod(s, 4)
                rs = slice(32 * so, 32 * so + 32)
                pA = wps.tile([64, 512], FP, tag="pA", name="pA")
                pB = wps.tile([64, 512], FP, tag="pB", name="pB")
                banks[s] = (pA, pB)
                for ti, pt_ in ((0, pA), (1, pB)):
                    for half in range(2):
                        g4 = (2 * ti + half) * 512
                        nc.tensor.matmul(pt_[32 * half:32 * half + 32, :],
                                         eyeb[rs, :],
                                         xt[rs, tt, g4:g4 + 512],
                                         start=True, stop=(s == 0),
                                         tile_position=(32 * so, 32 * half),
                                         skip_group_check=True)

            emit_inject(0)
            for s in range(S):
                tt, so = divmod(s, 4)
                if so == 0 and tt + APRE < NT:
                    emit_a(tt + APRE)
                pA, pB = banks.pop(s)
                ring = rlist[s % 3]
                nring = rlist[(s + 1) % 3]
                if s > 0:
                    for ti, pt_ in ((0, pA), (1, pB)):
                        for q in range(2):
                            for half in range(2):
                                g4 = (2 * ti + half) * 512
                                win = slice(32, 96) if half == 0 else \
                                    slice(0, 64)
                                nc.tensor.matmul(
                                    pt_[:], ring[:, 2 * q:2 * q + 2, win],
                                    wh[:, q, :, g4:g4 + 512],
                                    perf_mode=DR, start=False,
                                    stop=(q == 1 and half == 1),
                                    skip_group_check=True)
                if s + 1 < S:
                    emit_inject(s + 1)
                # acts: pA = (f|i) sigmoid; pB = (o|g') sigmoid with the
                # g strip at 2x scale (tanh(x) = 2*sigmoid(2x)-1)
                fi = wtmp.tile([64, 512], BF, tag="fi", name="fi")
                og = wtmp.tile([64, 512], BF, tag="og", name="og")
                nc.scalar.activation(fi[:], pA[:], Sig, scale=1.0 / 16.0)
                nc.scalar.activation(og[:], pB[:], Sig, scale=scl[:])
                g0 = wtmp.tile([64, 512], BF, tag="g0", name="g0")
                nc.vector.tensor_scalar(g0[32:64, :], og[32:64, :], 2.0,
                                        -1.0, op0=mybir.AluOpType.mult,
                                        op1=mybir.AluOpType.add)
                fc = wtmp.tile([32, 512], BF, tag="fc", name="fc")
                nc.vector.tensor_mul(fc[:], fi[0:32, :], c32[:])
                ig = wtmp.tile([32, 512], BF, tag="ig", name="ig")
                nc.vector.tensor_mul(ig[:], fi[32:64, :], g0[32:64, :])
                nc.vector.tensor_add(c32[:], fc[:], ig[:])
                # transposed tail, all inputs at base partition 0
                co = wpt.tile([128, 2, 4, bl2], BF, tag="co", name="co")
                # o-transposes first: they depend only on the act and run
                # during the DVE cell chain instead of queueing behind the
                # c-transposes (which wait on the add) in the PE FIFO
                for kk in range(4):
                    nc.tensor.transpose(co[:, 1, kk, :],
                                        og[0:32, kk * 128:(kk + 1) * 128],
                                        eyeb[0:32, 0:bl2])
                for kk in range(4):
                    nc.tensor.transpose(co[:, 0, kk, :],
                                        c32[:, kk * 128:(kk + 1) * 128],
                                        eyeb[0:32, 0:bl2])
                tct = wtmp.tile([128, 4, bl2], BF, tag="tct", name="tct")
                nc.scalar.activation(tct[:], co[:, 0, :, :], TanhF)
                nc.vector.tensor_mul(nring[:, :, 32:64], tct[:],
                                     co[:, 1, :, :])
                nc.vector.tensor_mul(hsT[:, :, s, :], tct[:],
                                     co[:, 1, :, :])
            nc.sync.dma_start(
                d_hs.ap().rearrange("k p (s b) -> p k s b", b=bl2), hsT[:])
    nc.compile()
    return nc


def build_l3(bl=BL):
    """MLP + log_softmax, data-parallel."""
    nl = bl * S
    nc = bacc.Bacc("TRN2", target_bir_lowering=False, debug=False,
                   num_devices=NCORE)
    d_hs = nc.dram_tensor("hsT8", [8, 128, nl], BF, kind="ExternalInput")
    d_W1T = nc.dram_tensor("W1T", [8, 128, 256], BF, kind="ExternalInput")
    d_b1 = nc.dram_tensor("b1m", [128, 2], FP, kind="ExternalInput")
    d_W2T = nc.dram_tensor("W2T", [2, 128, 256], BF, kind="ExternalInput")
    d_b2 = nc.dram_tensor("b2m", [128, 2], FP, kind="ExternalInput")
    d_W3T = nc.dram_tensor("W3T", [2, 128, OUT], BF, kind="ExternalInput")
    d_b3 = nc.dram_tensor("b3m", [OUT, 1], FP, kind="ExternalInput")
    d_eye = nc.dram_tensor("eye", [128, 128], FP, kind="ExternalInput")
    d_y = nc.dram_tensor("y", [nl, OUT], FP, kind="ExternalOutput")

    CH = min(512, nl)
    NCH = (nl + CH - 1) // CH

    with tile.TileContext(nc) as tc:
        with ExitStack() as c3:
            mw = c3.enter_context(tc.tile_pool(name="mweights", bufs=1))
            mact = c3.enter_context(tc.tile_pool(name="mact", bufs=1))
            mtmp = c3.enter_context(tc.tile_pool(name="mtmp", bufs=4))
            mps = c3.enter_context(tc.tile_pool(name="mpsum", bufs=2,
                                                space="PSUM"))
            sps = c3.enter_context(tc.tile_pool(name="spsum", bufs=2,
                                                space="PSUM"))
            eye_sb = mw.tile([128, 128], FP, tag="eye", name="eye")
            nc.sync.dma_start(eye_sb[:], d_eye.ap()[:])
            W1 = mw.tile([128, 8, 256], BF, tag="W1", name="W1")
            W2 = mw.tile([128, 2, 256], BF, tag="W2", name="W2")
            W3 = mw.tile([128, 2, OUT], BF, tag="W3", name="W3")
            b1 = mw.tile([128, 2], FP, tag="b1", name="b1")
            b2 = mw.tile([128, 2], FP, tag="b2", name="b2")
            b3 = mw.tile([OUT, 1], FP, tag="b3", name="b3")
            nc.sync.dma_start(W1[:], d_W1T.ap().rearrange("k p g -> p k g"))
            nc.sync.dma_start(W2[:], d_W2T.ap().rearrange("k p g -> p k g"))
            nc.sync.dma_start(W3[:], d_W3T.ap().rearrange("k p g -> p k g"))
            nc.sync.dma_start(b1[:], d_b1.ap()[:])
            nc.sync.dma_start(b2[:], d_b2.ap()[:])
            nc.sync.dma_start(b3[:], d_b3.ap()[:])
            hsT = [mw.tile([128, nl], BF, tag=f"hsT{k}", name=f"hsT{k}")
                   for k in range(8)]
            for ci in range((nl + 511) // 512):
                cs = slice(ci * 512, min(nl, (ci + 1) * 512))
                for k in range(8):
                    nc.sync.dma_start(hsT[k][:, cs], d_hs.ap()[k][:, cs])
            h1 = [mact.tile([128, nl], BF, tag=f"h1{m}", name=f"h1{m}")
                  for m in range(2)]
            h2 = [mact.tile([128, nl], BF, tag=f"h2{m}", name=f"h2{m}")
                  for m in range(2)]
            for ci in range(NCH):
                cs = slice(ci * CH, (ci + 1) * CH)
                for m in range(2):
                    p = mps.tile([128, CH], FP, tag="mp1", name="mp1")
                    for k in range(8):
                        nc.tensor.matmul(
                            p[:], W1[:, k, m * 128:(m + 1) * 128],
                            hsT[k][:, cs], start=(k == 0), stop=(k == 7))
                    nc.scalar.activation(h1[m][:, cs], p[:], ReluF,
                                         bias=b1[:, m:m + 1])
            for ci in range(NCH):
                cs = slice(ci * CH, (ci + 1) * CH)
                for m in range(2):
                    p = mps.tile([128, CH], FP, tag="mp2", name="mp2")
                    for k in range(2):
                        nc.tensor.matmul(
                            p[:], W2[:, k, m * 128:(m + 1) * 128],
                            h1[k][:, cs], start=(k == 0), stop=(k == 1))
                    nc.scalar.activation(h2[m][:, cs], p[:], ReluF,
                                         bias=b2[:, m:m + 1])
            # two passes batched by ACT function: all Exp, then all Ln,
            # so the Exp/Ln activation tables load once each instead of
            # per position-tile
            npt = max(1, nl // 128)
            lgs = [mact.tile([128, OUT], FP, tag=f"lgs{pi}", name=f"lgs{pi}")
                   for pi in range(npt)]
            nmxs = [mact.tile([128, 1], FP, tag=f"nmx{pi}", name=f"nmx{pi}")
                    for pi in range(npt)]
            sms = [mact.tile([128, 1], FP, tag=f"sm{pi}", name=f"sm{pi}")
                   for pi in range(npt)]
            for pi in range(npt):
                pcount = min(128, nl - pi * 128)
                psl = slice(pi * 128, pi * 128 + pcount)
                lg = mps.tile([OUT, pcount], FP, tag="mp3", name="mp3")
                for k in range(2):
                    nc.tensor.matmul(lg[:], W3[:, k, :], h2[k][:, psl],
                                     start=(k == 0), stop=(k == 1))
                lgb = mtmp.tile([OUT, pcount], FP, tag="lgb", name="lgb")
                nc.scalar.activation(lgb[:], lg[:], IdentF, bias=b3[:, 0:1])
                lgr = sps.tile([pcount, OUT], FP, tag="lgr", name="lgr")
                nc.tensor.transpose(lgr[:], lgb[:], eye_sb[0:OUT, 0:OUT])
                nc.vector.tensor_reduce(nmxs[pi][0:pcount, :], lgr[:],
                                        axis=mybir.AxisListType.X,
                                        op=mybir.AluOpType.max, negate=True)
                ex = mtmp.tile([pcount, OUT], FP, tag="ex", name="ex")
                nc.scalar.activation(ex[:], lgr[:], ExpF,
                                     bias=nmxs[pi][0:pcount, :],
                                     accum_out=sms[pi][0:pcount, :])
                nc.vector.tensor_copy(lgs[pi][0:pcount, :], lgr[:])
            for pi in range(npt):
                pcount = min(128, nl - pi * 128)
                psl = slice(pi * 128, pi * 128 + pcount)
                lsm = mtmp.tile([pcount, 1], FP, tag="lsm", name="lsm")
                nc.scalar.activation(lsm[:], sms[pi][0:pcount, :], LnF)
                shift = mtmp.tile([pcount, 1], FP, tag="shift", name="shift")
                nc.vector.tensor_sub(shift[:], nmxs[pi][0:pcount, :], lsm[:])
                yt = mtmp.tile([pcount, OUT], FP, tag="yt", name="yt")
                nc.vector.tensor_scalar(yt[:], lgs[pi][0:pcount, :],
                                        shift[:], None,
                                        op0=mybir.AluOpType.add)
                nc.sync.dma_start(d_y.ap()[psl, :], yt[:])
    nc.compile()
    return nc


def _prep_shared(inputs):
    f32 = np.float32
    cWxT = np.asarray(inputs["cW_ih"], f32).T
    cWx2 = np.zeros((128, 4, 128), f32)
    for j in range(2):
        for pi in range(2):
            cWx2[0:E, 2 * j + pi] = cWxT[:, (j + 4 * pi) * 128:
                                         (j + 4 * pi) * 128 + 128]
            cWx2[E:128, 2 * j + pi] = cWxT[:, (2 + j + 4 * pi) * 128:
                                           (2 + j + 4 * pi) * 128 + 128]
    cWhT = np.ascontiguousarray(
        np.asarray(inputs["cW_hh"], f32).T).reshape(2, 128, G4)
    cbias = (np.asarray(inputs["cb_ih"], f32)
             + np.asarray(inputs["cb_hh"], f32))
    cbias_m = np.ascontiguousarray(cbias.reshape(G4 // 128, 128).T)
    wW, wb = [], []
    for pre in ("f", "b"):
        wih = np.asarray(inputs[pre + "W_ih"], f32)
        whh = np.asarray(inputs[pre + "W_hh"], f32)
        wW.append(np.ascontiguousarray(
            np.concatenate([wih.T, whh.T], 0)).reshape(6, 128, WG))
        wb.append((np.asarray(inputs[pre + "b_ih"], f32)
                   + np.asarray(inputs[pre + "b_hh"], f32)).reshape(1, WG))
    W1T = np.ascontiguousarray(
        np.asarray(inputs["W1"], f32).T.astype(BF_NP)).reshape(8, 128, 256)
    b1m = np.ascontiguousarray(np.asarray(inputs["b1"], f32).reshape(2, 128).T)
    W2T = np.ascontiguousarray(
        np.asarray(inputs["W2"], f32).T.astype(BF_NP)).reshape(2, 128, 256)
    b2m = np.ascontiguousarray(np.asarray(inputs["b2"], f32).reshape(2, 128).T)
    W3T = np.ascontiguousarray(
        np.asarray(inputs["W3"], f32).T.astype(BF_NP)).reshape(2, 128, OUT)
    b3m = np.ascontiguousarray(np.asarray(inputs["b3"], f32).reshape(OUT, 1))
    eye = np.eye(128, dtype=f32)
    onesr = np.ones((1, 128), f32)
    return dict(cWx2=cWx2.astype(BF_NP), cWhT=cWhT.astype(BF_NP),
                cbias=cbias_m, wW=wW, wb=wb, W1T=W1T,
                b1m=b1m, W2T=W2T, b2m=b2m, W3T=W3T, b3m=b3m, eye=eye,
                onesr=onesr)


def _l1_maps(inputs, sh, bl, ncores):
    x = np.asarray(inputs["x"])
    emb = np.asarray(inputs["emb"], np.float32).astype(BF_NP)
    nl = bl * S
    maps = []
    for c in range(ncores):
        xc = x[c * bl:(c + 1) * bl].reshape(nl, Lc)
        lengths = (xc != 0).sum(axis=1).astype(np.float32)
        lenrep = np.ascontiguousarray(
            np.broadcast_to(lengths[None, :].astype(BF_NP), (128, nl)))
        eT = np.ascontiguousarray(emb[xc].transpose(1, 2, 0))
        maps.append(dict(eT=eT, lenrep=lenrep, cWx2=sh["cWx2"],
                         cWhT=sh["cWhT"], cbias=sh["cbias"]))
    return maps


@functools.lru_cache(maxsize=4)
def _modules(bl, lmin=18):
    return build_l1(bl, lmin), build_l2_v5(32, 2), build_l3(bl)


def _gate_blocks(w, order):
    h4 = w.shape[0] // 4
    return np.concatenate([w[g * h4:(g + 1) * h4] for g in order], axis=0)


ORD = (1, 0, 3, 2)   # PyTorch (i,f,g,o) -> (f,i,o,g)
F8_NP = ml_dtypes.float8_e4m3


def _prep_l2_v4(inputs):
    f32 = np.float32
    wh8, wx8, wb16 = [], [], []
    sc = np.full((4 * H, 1), 16.0, f32)
    sc[3 * H:] *= 2.0          # tanh(g) = 2*sig(2x)-1: fold the 2x here
    for pre in ("f", "b"):
        wih = _gate_blocks(np.asarray(inputs[pre + "W_ih"], f32), ORD)
        whh = _gate_blocks(np.asarray(inputs[pre + "W_hh"], f32), ORD)
        b = (_gate_blocks(np.asarray(inputs[pre + "b_ih"], f32), ORD)
             + _gate_blocks(np.asarray(inputs[pre + "b_hh"], f32), ORD))
        whhT = (whh * sc).T                      # [512, 2048]
        wihT = (wih * sc).T                      # [256, 2048]
        a = whhT.reshape(2, 2, 128, 16, 128)     # [q, r, p, gb, m]
        wh8.append(np.ascontiguousarray(
            a.transpose(2, 0, 3, 1, 4)).astype(F8_NP))
        a = wihT.reshape(2, 128, 16, 128)        # [r, p, gb, m]
        wx8.append(np.ascontiguousarray(
            a.transpose(1, 2, 0, 3)).astype(BF_NP))
        wb16.append(np.ascontiguousarray(
            (b * sc[:, 0]).reshape(1, 16, 128)).astype(BF_NP))
    return dict(wh8=wh8, wx8=wx8, wb16=wb16,
                ones32=np.ones((1, 32), BF_NP))


def _l2_maps_v4(last_full, sh2, ncores):
    maps = []
    half = ncores // 2
    for c in range(ncores):
        d = 0 if c < half else 1
        g = c % half
        lt = last_full[:, :, g * 32 * S:(g + 1) * 32 * S].reshape(
            2, 128, 32, S)
        if d == 1:
            lt = lt[:, :, :, ::-1]
        lt = lt.transpose(1, 0, 3, 2)            # [128, 2, S, 32]
        maps.append(dict(lastT16=np.ascontiguousarray(lt).astype(BF_NP),
                         wh8v4=sh2["wh8"][d], wx16v4=sh2["wx8"][d],
                         wb16=sh2["wb16"][d], ones32=sh2["ones32"]))
    return maps


def _prep_l2(inputs):
    f32 = np.float32
    wIT, wb, wh8 = [], [], []
    for pre in ("f", "b"):
        wih = _gate_blocks(np.asarray(inputs[pre + "W_ih"], f32), ORD)
        whh = _gate_blocks(np.asarray(inputs[pre + "W_hh"], f32), ORD)
        wIT.append(np.ascontiguousarray(wih.T.astype(BF_NP)).reshape(
            2, 128, 4 * H))
        wb.append((_gate_blocks(np.asarray(inputs[pre + "b_ih"], f32), ORD)
                   + _gate_blocks(np.asarray(inputs[pre + "b_hh"], f32),
                                  ORD)).reshape(1, 4 * H).astype(BF_NP))
        whhT = whh.T * 16.0
        wh8.append(np.ascontiguousarray(
            whhT.reshape(2, 2, 128, 4 * H).transpose(0, 2, 1, 3)
            .astype(F8_NP)))
    eyeb = np.zeros((128, 32), f32)
    for p in range(128):
        eyeb[p, p % 32] = 1.0
    scl64 = np.full((64, 1), 1.0 / 16.0, f32)
    scl64[32:64] = 2.0 / 16.0
    return dict(wIT=wIT, wb=wb, wh8=wh8, eyeb=eyeb.astype(BF_NP),
                scl64=scl64, onesr=np.ones((1, 128), f32).astype(BF_NP))


def _l2_maps_v3(last_full, sh2, ncores):
    maps = []
    half = ncores // 2
    for c in range(ncores):
        d = 0 if c < half else 1
        g = c % half
        lt = last_full[:, :, g * 32 * S:(g + 1) * 32 * S].reshape(
            2, 128, 32, S)
        if d == 1:
            lt = lt[:, :, :, ::-1]
        lt = np.ascontiguousarray(
            lt.transpose(0, 1, 3, 2).reshape(2, 128, 32 * S)).astype(BF_NP)
        maps.append(dict(lastT2=lt, wIT=sh2["wIT"][d], wb=sh2["wb"][d],
                         onesr=sh2["onesr"], eyeb=sh2["eyeb"],
                         scl64=sh2["scl64"], wh8=sh2["wh8"][d]))
    return maps


def _l3_maps_v3(hs_f, hs_b, sh, bl, ncores):
    nl = bl * S
    maps = []
    for c in range(ncores):
        g, hf = c // 2, c % 2
        sl = slice(hf * nl, (hf + 1) * nl)
        hs8 = np.concatenate([hs_f[g][:, :, sl], hs_b[g][:, :, sl]], axis=0)
        maps.append(dict(hsT8=np.ascontiguousarray(hs8), W1T=sh["W1T"],
                         b1m=sh["b1m"], W2T=sh["W2T"], b2m=sh["b2m"],
                         W3T=sh["W3T"], b3m=sh["b3m"], eye=sh["eye"]))
    return maps


def _pipeline(inputs, bl, ncores, run_l1, run_l2, run_l3):
    sh = _prep_shared(inputs)
    sh2 = _prep_l2_v4(inputs)
    half = ncores // 2

    r1 = run_l1(_l1_maps(inputs, sh, bl, ncores))
    last_full = np.concatenate(
        [np.asarray(r1[c]["lastT"]).astype(np.float32)
         for c in range(ncores)], axis=2)

    r2 = run_l2(_l2_maps_v4(last_full, sh2, ncores))
    hs_f, hs_b = [], []
    for g in range(half):
        hs_f.append(np.asarray(r2[g]["hsTh"]))
        hb = np.asarray(r2[half + g]["hsTh"]).reshape(4, 128, S, 32)
        hs_b.append(np.ascontiguousarray(
            hb[:, :, ::-1, :]).reshape(4, 128, 32 * S))

    r3 = run_l3(_l3_maps_v3(hs_f, hs_b, sh, bl, ncores))
    out = np.empty((B, S, OUT), np.float32)
    for c in range(ncores):
        y = np.asarray(r3[c]["y"]).reshape(S // 2, 32, OUT)
        bs = 32 * (c // 2)
        ss = (S // 2) * (c % 2)
        out[bs:bs + 32, ss:ss + S // 2] = y.transpose(1, 0, 2)
    return out


def kernel(**inputs):
    x = np.asarray(inputs["x"])
    lmin = int((x.reshape(-1, Lc) != 0).sum(axis=1).min())
    l1, l2, l3 = _modules(BL, lmin)

    def runner(nc):
        def run(in_maps):
            res = bass_utils.run_bass_kernel_spmd(
                nc, in_maps, core_ids=list(range(NCORE)))
            return res.results
        return run

    return _pipeline(inputs, BL, NCORE, runner(l1), runner(l2), runner(l3))



# revision 13
# speedup vs baseline: 1.0376x; 1.0376x over previous
"""Trainium2 Bass kernel for nn_CharTaggerBiLSTM, 8-core SPMD, 3 launches.

L1 char LSTM: data-parallel over batch (16 sentences/core). Transposed
   layout (features-on-partitions), f32r matmuls; emits the masked last
   hidden state per word -> DRAM.
L2 word LSTM: one direction per core (cores 0-3 forward, 4-7 backward),
   32 sentences/core so each weight stream serves twice the rows.
   Direction is data: backward cores receive the char outputs with the
   sentence axis reversed on host and their outputs are un-reversed.
   x-part/bias matmuls for step s+1 are issued during step s's
   elementwise work to keep PE fed.
L3 MLP + log_softmax: data-parallel (16 sentences/core), bf16 GEMMs.

Host does embedding gather, weight reshapes, the two reshard steps, and
reassembly.
"""

import sys
import functools
from contextlib import ExitStack

sys.path.insert(0, "/opt/trn_rl_repo")

import numpy as np
import ml_dtypes
from concourse import bacc, bass, mybir, tile, bass_utils

BF_NP = ml_dtypes.bfloat16
U8 = mybir.dt.uint8
F8T = mybir.dt.float8e4
DR = mybir.MatmulPerfMode.DoubleRow
E = 64


B, S, Lc = 128, 128, 20
AB, E = 100, 64
Hc, H, OUT = 256, 512, 50
NCORE = 8
BL = B // NCORE            # sentences per core in L1/L3
FP = mybir.dt.float32
FR = mybir.dt.float32r
BF = mybir.dt.bfloat16
G4 = 4 * Hc
WG = 4 * H

Sig = mybir.ActivationFunctionType.Sigmoid
TanhF = mybir.ActivationFunctionType.Tanh
ReluF = mybir.ActivationFunctionType.Relu
ExpF = mybir.ActivationFunctionType.Exp
LnF = mybir.ActivationFunctionType.Ln
IdentF = mybir.ActivationFunctionType.Identity


def build_l1(bl=BL, lmin=18):
    """Char LSTM, data-parallel; writes lastT [2,128,nl] bf16 to DRAM."""
    nl = bl * S
    nc = bacc.Bacc("TRN2", target_bir_lowering=False, debug=False,
                   num_devices=NCORE)
    d_eT = nc.dram_tensor("eT", [Lc, E, nl], BF, kind="ExternalInput")
    d_lenrep = nc.dram_tensor("lenrep", [128, nl], BF, kind="ExternalInput")
    d_cWx2 = nc.dram_tensor("cWx2", [128, 4, 128], BF, kind="ExternalInput")
    d_cWhT = nc.dram_tensor("cWhT", [2, 128, G4], BF, kind="ExternalInput")
    d_cbias = nc.dram_tensor("cbias", [128, G4 // 128], FP,
                             kind="ExternalInput")
    d_last = nc.dram_tensor("lastT", [2, 128, nl], BF, kind="ExternalOutput")

    CH = 1024
    NCH = nl // CH
    GF = [Sig, Sig, TanhF, Sig]          # gate funcs for gi = i, f, g, o

    with tile.TileContext(nc) as tc:
        with ExitStack() as c1:
            cw = c1.enter_context(tc.tile_pool(name="cweights", bufs=1))
            cst = c1.enter_context(tc.tile_pool(name="cstate", bufs=1))
            ein = c1.enter_context(tc.tile_pool(name="ein", bufs=2))
            ctmp = c1.enter_context(tc.tile_pool(name="ctmp", bufs=3))
            cps = c1.enter_context(tc.tile_pool(name="cpsum", bufs=4,
                                                space="PSUM"))
            cWx2 = cw.tile([128, 4, 128], BF, tag="cWx2", name="cWx2")
            cWh = cw.tile([128, 2, G4], BF, tag="cWh", name="cWh")
            cb = cw.tile([128, G4 // 128], FP, tag="cb", name="cb")
            lenr = cw.tile([128, nl], BF, tag="lenr", name="lenr")
            nc.sync.dma_start(cWx2[:], d_cWx2.ap()[:])
            nc.sync.dma_start(cWh[:], d_cWhT.ap().rearrange("k p g -> p k g"))
            nc.sync.dma_start(cb[:], d_cbias.ap()[:])
            nc.sync.dma_start(lenr[:], d_lenrep.ap()[:])

            last = cst.tile([128, 2, nl], BF, tag="last", name="last")
            hh = [cst.tile([128, 2, nl], BF, tag=f"h{p}", name=f"h{p}")
                  for p in range(2)]
            cc = cst.tile([128, 2, nl], BF, tag="cc", name="cc")
            nc.vector.memset(cc[:], 0.0)
            nc.vector.memset(last[:], 0.0)

            for t in range(Lc):
                et2 = ein.tile([128, nl], BF, tag="et2", name="et2")
                nc.sync.dma_start(et2[0:E, :], d_eT.ap()[t])
                nc.sync.dma_start(et2[E:128, :], d_eT.ap()[t])
                masked = t >= lmin - 1
                hprev = hh[t % 2]
                hcur = hh[(t + 1) % 2]
                for ci in range(NCH):
                    cs = slice(ci * CH, (ci + 1) * CH)
                    if masked:
                        mk = ctmp.tile([128, CH], U8, tag="mk", name="mk")
                        nc.gpsimd.tensor_scalar(mk[:], lenr[:, cs], float(t),
                                                None,
                                                op0=mybir.AluOpType.is_gt)
                    for j in range(2):
                        ps = [cps.tile([128, CH], FP, tag="ps", name="ps")
                              for _ in range(4)]
                        # x-part: two K=64 row strips per PE pass, plus
                        # bias-free accumulation of the two h chunks;
                        # matmul outputs are split into 512-col halves
                        # (one PSUM bank each)
                        for hw_ in range(2):
                            o5 = slice(hw_ * 512, (hw_ + 1) * 512)
                            c5 = slice(ci * CH + hw_ * 512,
                                       ci * CH + (hw_ + 1) * 512)
                            for pi in range(2):
                                sl = 2 * j + pi
                                nc.tensor.matmul(ps[2 * pi][:, o5],
                                                 cWx2[0:E, sl, :],
                                                 et2[0:E, c5],
                                                 start=True, stop=(t == 0))
                                nc.tensor.matmul(ps[2 * pi + 1][:, o5],
                                                 cWx2[E:128, sl, :],
                                                 et2[E:128, c5],
                                                 start=True, stop=(t == 0))
                            if t > 0:
                                for gi in range(4):
                                    m = 2 * gi + j
                                    for k in range(2):
                                        nc.tensor.matmul(
                                            ps[gi][:, o5],
                                            cWh[:, k, m * 128:(m + 1) * 128],
                                            hprev[:, k, c5],
                                            start=False, stop=(k == 1))
                        gsb = [ctmp.tile([128, CH], BF, tag=f"g{gi}",
                                         name=f"g{gi}") for gi in range(4)]
                        for gi in range(4):
                            m = 2 * gi + j
                            nc.scalar.activation(gsb[gi][:], ps[gi][:],
                                                 GF[gi], bias=cb[:, m:m + 1])
                        ig = ctmp.tile([128, CH], BF, tag="ig", name="ig")
                        nc.vector.tensor_mul(ig[:], gsb[0][:], gsb[2][:])
                        fc = ctmp.tile([128, CH], BF, tag="fc", name="fc")
                        nc.vector.tensor_mul(fc[:], gsb[1][:], cc[:, j, cs])
                        nc.vector.tensor_add(cc[:, j, cs], fc[:], ig[:])
                        tct = ctmp.tile([128, CH], BF, tag="tct", name="tct")
                        nc.scalar.activation(tct[:], cc[:, j, cs], TanhF)
                        nc.vector.tensor_mul(hcur[:, j, cs], gsb[3][:],
                                             tct[:])
                        if masked:
                            nc.vector.copy_predicated(last[:, j, cs], mk[:],
                                                      hcur[:, j, cs])
            for j in range(2):
                nc.sync.dma_start(d_last.ap()[j], last[:, j, :])
    nc.compile()
    return nc


def build_l1_v2(bl=BL, lmin=18):
    """Char LSTM v2: act-engine-optimized.

    Psum per (word-chunk-512, j-half): [128, 4, 512] f32 (4 banks, one
    per gate type i,f,g,o; start=True only on each bank's first mm).
    x-part: K=65 bf16 (emb row 64 = ones -> bias rides in the weights,
    g rows pre-scaled 2x so one sigmoid serves tanh(g) as 2*sig(2x)-1).
    h-part: fp8 DoubleRow (h ring and W_hh both fp8, W scaled 16x, act
    scale 1/16 undoes it... n.b. x/bias must then also be 16x).
    One sigmoid act per (chunk, j) over all 4 gates; tails run j-wide
    on [128, 2048] slices; h*o on gpsimd.
    """
    nl = bl * S
    nc = bacc.Bacc("TRN2", target_bir_lowering=False, debug=False,
                   num_devices=NCORE)
    d_eT = nc.dram_tensor("eT65", [Lc, 65, nl], BF, kind="ExternalInput")
    d_lenrep = nc.dram_tensor("lenrep", [128, nl], BF, kind="ExternalInput")
    d_cWx = nc.dram_tensor("cWx65", [65, 2, 4, 128], BF,
                           kind="ExternalInput")
    d_cWh8 = nc.dram_tensor("cWh8", [128, 2, 2, 4, 128], F8T,
                            kind="ExternalInput")
    d_last = nc.dram_tensor("lastT", [2, 128, nl], BF, kind="ExternalOutput")

    CH = 512
    NCH = nl // CH
    with tile.TileContext(nc) as tc:
        with ExitStack() as c1:
            cw = c1.enter_context(tc.tile_pool(name="cweights", bufs=1))
            cst = c1.enter_context(tc.tile_pool(name="cstate", bufs=1))
            ein = c1.enter_context(tc.tile_pool(name="ein", bufs=2))
            ctmp = c1.enter_context(tc.tile_pool(name="ctmp", bufs=2))
            cps = c1.enter_context(tc.tile_pool(name="cpsum", bufs=2,
                                                space="PSUM"))
            cWx = cw.tile([65, 2, 4, 128], BF, tag="cWx", name="cWx")
            cWh = cw.tile([128, 2, 2, 4, 128], F8T, tag="cWh", name="cWh")
            lenr = cw.tile([128, nl], BF, tag="lenr", name="lenr")
            nc.sync.dma_start(cWx[:], d_cWx.ap()[:])
            nc.sync.dma_start(cWh[:], d_cWh8.ap()[:])
            nc.sync.dma_start(lenr[:], d_lenrep.ap()[:])

            last = cst.tile([128, 2, nl], BF, tag="last", name="last")
            # h ring: fp8 for the DR matmul + bf16 h for nothing else
            hh = [cst.tile([128, 2, nl], F8T, tag=f"h{p}", name=f"h{p}")
                  for p in range(2)]
            cc = cst.tile([128, 2, nl], BF, tag="cc", name="cc")
            gsb = [cst.tile([128, 4, nl], BF, tag=f"gs{j}", name=f"gs{j}")
                   for j in range(2)]
            g0t = [cst.tile([128, nl], BF, tag=f"g0{j}", name=f"g0{j}")
                   for j in range(2)]
            igt = [cst.tile([128, nl], BF, tag=f"ig{j}", name=f"ig{j}")
                   for j in range(2)]
            fct = [cst.tile([128, nl], BF, tag=f"fc{j}", name=f"fc{j}")
                   for j in range(2)]
            tctt = [cst.tile([128, nl], BF, tag=f"tc{j}", name=f"tc{j}")
                    for j in range(2)]
            nc.vector.memset(cc[:], 0.0)
            nc.vector.memset(last[:], 0.0)

            CK = 1024
            NCK = nl // CK

            def tail(j, ck, t, hcur, masked):
                ts_ = slice(ck * CK, (ck + 1) * CK)
                gj = gsb[j]
                nc.vector.tensor_scalar(g0t[j][:, ts_], gj[:, 2, ts_], 2.0,
                                        -1.0, op0=mybir.AluOpType.mult,
                                        op1=mybir.AluOpType.add)
                nc.vector.tensor_mul(igt[j][:, ts_], gj[:, 0, ts_],
                                     g0t[j][:, ts_])
                nc.vector.tensor_mul(fct[j][:, ts_], gj[:, 1, ts_],
                                     cc[:, j, ts_])
                nc.vector.tensor_add(cc[:, j, ts_], fct[j][:, ts_],
                                     igt[j][:, ts_])
                nc.scalar.activation(tctt[j][:, ts_], cc[:, j, ts_], TanhF)
                nc.gpsimd.tensor_mul(hcur[:, j, ts_], gj[:, 3, ts_],
                                     tctt[j][:, ts_])
                if masked:
                    mk = ctmp.tile([128, CK], U8, tag="mk", name="mk")
                    nc.gpsimd.tensor_scalar(mk[:], lenr[:, ts_], float(t),
                                            None,
                                            op0=mybir.AluOpType.is_gt)
                    hb = ctmp.tile([128, CK], BF, tag="hb", name="hb")
                    nc.vector.tensor_mul(hb[:], gj[:, 3, ts_],
                                         tctt[j][:, ts_])
                    nc.vector.copy_predicated(last[:, j, ts_], mk[:], hb[:])

            for t in range(Lc):
                et = ein.tile([65, nl], BF, tag="et", name="et")
                nc.sync.dma_start(et[:], d_eT.ap()[t])
                masked = t >= lmin - 1
                hprev = hh[t % 2]
                hcur = hh[(t + 1) % 2]
                for ci in range(NCH):
                    cs = slice(ci * CH, (ci + 1) * CH)
                    for j in range(2):
                        ps = cps.tile([128, 4, CH], FP, tag="ps", name="ps")
                        for gi in range(4):
                            nc.tensor.matmul(ps[:, gi, :],
                                             cWx[:, j, gi, :], et[:, cs],
                                             start=True,
                                             stop=(t == 0),
                                             skip_group_check=True)
                        if t > 0:
                            for gi in range(4):
                                nc.tensor.matmul(
                                    ps[:, gi, :], cWh[:, j, :, gi, :],
                                    hprev[:, :, cs],
                                    start=False, stop=True,
                                    perf_mode=DR, skip_group_check=True)
                        nc.scalar.activation(gsb[j][:, :, cs], ps[:], Sig,
                                             scale=1.0 / 16.0)
                    if ci % 2 == 1:
                        for j in range(2):
                            tail(j, ci // 2, t, hcur, masked)
            for j in range(2):
                nc.sync.dma_start(d_last.ap()[j], last[:, j, :])
    nc.compile()
    return nc


def build_l2_v4(bl2=32):
    """Word LSTM v4: gates-on-partitions, sentences-moving.

    Per step: psum tile [128, 16, 32] f32 (1 bank) holds all 2048 gates
    (16 blocks of 128 gate-dims on partitions) x 32 sentences on free.
    Seeded by bias (K=1 bf16) + x-part (fp8 DR vs last char state), then
    h-part (fp8 DR vs h^T ring) accumulates. One sigmoid act covers all
    gates (tanh(g) folded as 2*sig(2x)-1 with the 2x pre-scaled into the
    g rows host-side); cell tail on DVE; h^T ring written directly by
    the o*tanh(c) mul - no transposes, no inject.
    Gate type order: f, i, o, g (blocks 0:4, 4:8, 8:12, 12:16).
    """
    nl = bl2 * S
    nc = bacc.Bacc("TRN2", target_bir_lowering=False, debug=False,
                   num_devices=NCORE)
    d_lastT = nc.dram_tensor("lastT8", [128, 2, S, bl2], F8T,
                             kind="ExternalInput")
    d_wh8 = nc.dram_tensor("wh8v4", [128, 2, 16, 2, 128], F8T,
                           kind="ExternalInput")
    d_wx8 = nc.dram_tensor("wx8v4", [128, 16, 2, 128], F8T,
                           kind="ExternalInput")
    d_wb = nc.dram_tensor("wb16", [1, 16, 128], BF, kind="ExternalInput")
    d_ones = nc.dram_tensor("ones32", [1, bl2], BF, kind="ExternalInput")
    d_hs = nc.dram_tensor("hsTh", [4, 128, nl], BF, kind="ExternalOutput")

    with tile.TileContext(nc) as tc:
        with ExitStack() as c2:
            ww = c2.enter_context(tc.tile_pool(name="wweights", bufs=1))
            wst = c2.enter_context(tc.tile_pool(name="wstate", bufs=1))
            rgp = c2.enter_context(tc.tile_pool(name="wring", bufs=2))
            cpl = c2.enter_context(tc.tile_pool(name="wcell", bufs=2))
            sgp = c2.enter_context(tc.tile_pool(name="wsg", bufs=2))
            wtmp = c2.enter_context(tc.tile_pool(name="wtmp", bufs=2))
            wps = c2.enter_context(tc.tile_pool(name="wpsum", bufs=3,
                                                space="PSUM"))
            wh = ww.tile([128, 2, 16, 2, 128], F8T, tag="wh", name="wh")
            wx = ww.tile([128, 16, 2, 128], F8T, tag="wx", name="wx")
            wb = ww.tile([1, 16, 128], BF, tag="wb", name="wb")
            ones = ww.tile([1, bl2], BF, tag="ones", name="ones")
            lastT = ww.tile([128, 2, S, bl2], BF, tag="lastT", name="lastT")
            nc.sync.dma_start(wh[:], d_wh8.ap()[:])
            nc.sync.dma_start(wx[:], d_wx8.ap()[:])
            nc.sync.dma_start(wb[:], d_wb.ap()[:])
            nc.sync.dma_start(ones[:], d_ones.ap()[:])
            for sc in range(4):
                ss = slice(sc * (S // 4), (sc + 1) * (S // 4))
                nc.sync.dma_start(lastT[:, :, ss, :], d_lastT.ap()[:, :, ss, :])
            hsT = wst.tile([128, 4, S, bl2], BF, tag="hsT", name="hsT")

            rinit = [rgp.tile([128, 4, bl2], F8T, tag="ring", name=f"ri{i}")
                     for i in range(2)]
            for t_ in rinit:
                nc.vector.memset(t_[:], 0.0)
            ring_prev = rinit[1]
            cinit = [cpl.tile([128, 4, bl2], BF, tag="cc", name=f"ci{i}")
                     for i in range(2)]
            nc.vector.memset(cinit[1][:], 0.0)
            c_prev = cinit[1]

            for s in range(S):
                ps = wps.tile([128, 16, bl2], FP, tag="ps", name="ps")
                for gb in range(16):
                    nc.tensor.matmul(ps[:, gb, :], wb[:, gb, :], ones[:],
                                     start=True, stop=False,
                                     skip_group_check=True)
                for gb in range(16):
                    nc.tensor.matmul(ps[:, gb, :], wx[:, gb, :, :],
                                     lastT[:, :, s, :],
                                     start=False, stop=(s == 0),
                                     perf_mode=DR, skip_group_check=True)
                if s > 0:
                    for gb in range(16):
                        for q in range(2):
                            nc.tensor.matmul(
                                ps[:, gb, :], wh[:, q, gb, :, :],
                                ring_prev[:, 2 * q:2 * q + 2, :],
                                start=False, stop=(q == 1),
                                perf_mode=DR, skip_group_check=True)
                sg = sgp.tile([128, 16, bl2], BF, tag="sg", name="sg")
                nc.scalar.activation(sg[:], ps[:], Sig, scale=1.0 / 16.0)
                fco = wtmp.tile([128, 4, bl2], BF, tag="fco", name="fco")
                nc.vector.tensor_mul(fco[:], sg[:, 0:4, :], c_prev[:])
                tg = wtmp.tile([128, 4, bl2], BF, tag="tg", name="tg")
                nc.vector.scalar_tensor_tensor(
                    tg[:], sg[:, 12:16, :], 0.5, sg[:, 4:8, :],
                    op0=mybir.AluOpType.subtract, op1=mybir.AluOpType.mult)
                c_new = cpl.tile([128, 4, bl2], BF, tag="cc", name="cc")
                nc.vector.scalar_tensor_tensor(
                    c_new[:], tg[:], 2.0, fco[:],
                    op0=mybir.AluOpType.mult, op1=mybir.AluOpType.add)
                tct = wtmp.tile([128, 4, bl2], BF, tag="tct", name="tct")
                nc.scalar.activation(tct[:], c_new[:], TanhF)
                ring_new = rgp.tile([128, 4, bl2], F8T, tag="ring",
                                    name="ring")
                nc.vector.tensor_mul(ring_new[:], sg[:, 8:12, :], tct[:])
                nc.gpsimd.tensor_mul(hsT[:, :, s, :], sg[:, 8:12, :], tct[:])
                ring_prev = ring_new
                c_prev = c_new
            nc.sync.dma_start(
                d_hs.ap().rearrange("k p (s b) -> p k s b", b=bl2), hsT[:])
    nc.compile()
    return nc


def build_l2_v5(bl2=32, ngrp=2):
    """Word LSTM v5: like v4 but sentences split into ngrp interleaved
    groups with independent recurrence chains, so each group's
    (smaller) elementwise ops overlap the other group's matmuls."""
    nl = bl2 * S
    gw = bl2 // ngrp                 # sentences per group
    nc = bacc.Bacc("TRN2", target_bir_lowering=False, debug=False,
                   num_devices=NCORE)
    d_lastT = nc.dram_tensor("lastT16", [128, 2, S, bl2], BF,
                             kind="ExternalInput")
    d_wh8 = nc.dram_tensor("wh8v4", [128, 2, 16, 2, 128], F8T,
                           kind="ExternalInput")
    d_wx8 = nc.dram_tensor("wx16v4", [128, 16, 2, 128], BF,
                           kind="ExternalInput")
    d_wb = nc.dram_tensor("wb16", [1, 16, 128], BF, kind="ExternalInput")
    d_ones = nc.dram_tensor("ones32", [1, bl2], BF, kind="ExternalInput")
    d_hs = nc.dram_tensor("hsTh", [4, 128, nl], BF, kind="ExternalOutput")

    with tile.TileContext(nc) as tc:
        with ExitStack() as c2:
            ww = c2.enter_context(tc.tile_pool(name="wweights", bufs=1))
            wst = c2.enter_context(tc.tile_pool(name="wstate", bufs=1))
            rgp = c2.enter_context(tc.tile_pool(name="wring", bufs=2 * ngrp))
            cpl = c2.enter_context(tc.tile_pool(name="wcell", bufs=2 * ngrp))
            sgp = c2.enter_context(tc.tile_pool(name="wsg", bufs=2 * ngrp))
            wtmp = c2.enter_context(tc.tile_pool(name="wtmp", bufs=2 * ngrp))
            wps = c2.enter_context(tc.tile_pool(name="wpsum", bufs=3,
                                                space="PSUM"))
            wh = ww.tile([128, 2, 16, 2, 128], F8T, tag="wh", name="wh")
            wx = ww.tile([128, 16, 2, 128], BF, tag="wx", name="wx")
            wb = ww.tile([1, 16, 128], BF, tag="wb", name="wb")
            ones = ww.tile([1, bl2], BF, tag="ones", name="ones")
            lastT = ww.tile([128, 2, S, bl2], BF, tag="lastT", name="lastT")
            nc.sync.dma_start(wh[:], d_wh8.ap()[:])
            nc.sync.dma_start(wx[:], d_wx8.ap()[:])
            nc.sync.dma_start(wb[:], d_wb.ap()[:])
            nc.sync.dma_start(ones[:], d_ones.ap()[:])
            for sc in range(4):
                ss = slice(sc * (S // 4), (sc + 1) * (S // 4))
                nc.sync.dma_start(lastT[:, :, ss, :], d_lastT.ap()[:, :, ss, :])
            hsT = wst.tile([128, 4, S, bl2], BF, tag="hsT", name="hsT")

            c_prev, ring_prev = [], []
            for g in range(ngrp):
                ct = cpl.tile([128, 4, gw], BF, tag=f"cc{g}", name=f"ci{g}")
                nc.vector.memset(ct[:], 0.0)
                c_prev.append(ct)
                ring_prev.append(None)

            def step_mms(g, s):
                # psum tile is a full 2KB bank: start=True zeroes the whole
                # bank, so exactly one matmul (first bias) carries start.
                gs = slice(g * gw, (g + 1) * gw)
                ps = wps.tile([128, 16, 32], FP, tag=f"ps{g}", name=f"ps{g}")
                for gb in range(16):
                    nc.tensor.matmul(ps[:, gb, 0:gw], wb[:, gb, :],
                                     ones[:, gs],
                                     start=(gb == 0), stop=False,
                                     skip_group_check=True)
                for gb in range(16):
                    for r in range(2):
                        nc.tensor.matmul(ps[:, gb, 0:gw], wx[:, gb, r, :],
                                         lastT[:, r, s, gs],
                                         start=False,
                                         stop=(s == 0 and gb == 15
                                               and r == 1),
                                         skip_group_check=True)
                if s > 0:
                    for gb in range(16):
                        for q in range(2):
                            nc.tensor.matmul(
                                ps[:, gb, 0:gw], wh[:, q, gb, :, :],
                                ring_prev[g][:, 2 * q:2 * q + 2, :],
                                start=False,
                                stop=(gb == 15 and q == 1),
                                perf_mode=DR, skip_group_check=True)
                return ps

            def step_tail(g, s, ps):
                gs = slice(g * gw, (g + 1) * gw)
                sg = sgp.tile([128, 16, gw], BF, tag=f"sg{g}", name=f"sg{g}")
                nc.scalar.activation(sg[:], ps[:, :, 0:gw], Sig,
                                     scale=1.0 / 16.0)
                fco = wtmp.tile([128, 4, gw], BF, tag=f"fco{g}",
                                name=f"fco{g}")
                nc.vector.tensor_mul(fco[:], sg[:, 0:4, :], c_prev[g][:])
                tg = wtmp.tile([128, 4, gw], BF, tag=f"tg{g}", name=f"tg{g}")
                nc.vector.scalar_tensor_tensor(
                    tg[:], sg[:, 12:16, :], 0.5, sg[:, 4:8, :],
                    op0=mybir.AluOpType.subtract, op1=mybir.AluOpType.mult)
                c_new = cpl.tile([128, 4, gw], BF, tag=f"cc{g}",
                                 name=f"cc{g}")
                nc.vector.scalar_tensor_tensor(
                    c_new[:], tg[:], 2.0, fco[:],
                    op0=mybir.AluOpType.mult, op1=mybir.AluOpType.add)
                tct = wtmp.tile([128, 4, gw], BF, tag=f"tct{g}",
                                name=f"tct{g}")
                nc.scalar.activation(tct[:], c_new[:], TanhF)
                ring_new = rgp.tile([128, 4, gw], F8T, tag=f"ring{g}",
                                    name=f"ring{g}")
                nc.vector.tensor_mul(ring_new[:], sg[:, 8:12, :], tct[:])
                nc.gpsimd.tensor_mul(hsT[:, :, s, gs], sg[:, 8:12, :],
                                     tct[:])
                ring_prev[g] = ring_new
                c_prev[g] = c_new

            pend = {}
            for s in range(S):
                for g in range(ngrp):
                    pend[g] = step_mms(g, s)
                    og = (g + 1) % ngrp
                    if (og, 'tail') in pend:
                        gg, ss, pp = pend.pop((og, 'tail'))
                        step_tail(gg, ss, pp)
                    pend[(g, 'tail')] = (g, s, pend[g])
            for g in range(ngrp):
                if (g, 'tail') in pend:
                    gg, ss, pp = pend.pop((g, 'tail'))
                    step_tail(gg, ss, pp)
            nc.sync.dma_start(
                d_hs.ap().rearrange("k p (s b) -> p k s b", b=bl2), hsT[:])
    nc.compile()
    return nc


def build_l2(bl2=32, fp8=True):
    """Word LSTM v3: fp8 DoubleRow via half-pad windows, step-major."""
    nl = bl2 * S
    nc = bacc.Bacc("TRN2", target_bir_lowering=False, debug=False,
                   num_devices=NCORE)
    d_last = nc.dram_tensor("lastT2", [2, 128, nl], BF, kind="ExternalInput")
    d_wIT = nc.dram_tensor("wIT", [2, 128, WG], BF, kind="ExternalInput")
    d_wb = nc.dram_tensor("wb", [1, WG], BF, kind="ExternalInput")
    d_ones = nc.dram_tensor("onesr", [1, 128], BF, kind="ExternalInput")
    d_eyeb = nc.dram_tensor("eyeb", [128, 32], BF, kind="ExternalInput")
    d_scl = nc.dram_tensor("scl64", [64, 1], FP, kind="ExternalInput")
    d_wh = nc.dram_tensor("wh8", [2, 128, 2, WG], F8T, kind="ExternalInput")
    d_hs = nc.dram_tensor("hsTh", [4, 128, nl], BF, kind="ExternalOutput")
    NT = nl // 128
    IdF = mybir.ActivationFunctionType.Identity

    with tile.TileContext(nc) as tc:
        with ExitStack() as c2:
            ww = c2.enter_context(tc.tile_pool(name="wweights", bufs=1))
            wst = c2.enter_context(tc.tile_pool(name="wstate", bufs=1))
            wtmp = c2.enter_context(tc.tile_pool(name="wtmp", bufs=3))
            eyeb = ww.tile([128, 32], BF, tag="eyeb", name="eyeb")
            nc.sync.dma_start(eyeb[:], d_eyeb.ap()[:])
            ones = ww.tile([1, 128], BF, tag="ones", name="ones")
            nc.sync.dma_start(ones[:], d_ones.ap()[:])
            wbt = ww.tile([1, WG], BF, tag="wbt", name="wbt")
            nc.sync.dma_start(wbt[:], d_wb.ap()[:])
            scl = ww.tile([64, 1], FP, tag="scl", name="scl")
            nc.sync.dma_start(scl[:], d_scl.ap()[:])
            wh = ww.tile([128, 2, 2, WG], F8T, tag="wh", name="wh")
            nc.sync.dma_start(wh[:],
                              d_wh.ap().rearrange("q p i g -> p q i g"))
            xt = wst.tile([128, NT, WG], BF, tag="xt", name="xt")
            hsT = wst.tile([128, 4, S, bl2], BF, tag="hsT", name="hsT")

            lw = c2.enter_context(tc.tile_pool(name="lw", bufs=1))
            psA = c2.enter_context(tc.tile_pool(name="psA", bufs=2,
                                                space="PSUM"))
            lpool = c2.enter_context(tc.tile_pool(name="lpool", bufs=3))
            wIT = lw.tile([128, 2, WG], BF, tag="wIT", name="wIT")
            nc.sync.dma_start(wIT[:],
                              d_wIT.ap().rearrange("k p g -> p k g"))

            def emit_a(tt):
                ts = slice(tt * 128, (tt + 1) * 128)
                lt = lpool.tile([128, 2, 128], BF, tag="lt", name="lt")
                for j2 in range(2):
                    nc.sync.dma_start(lt[:, j2, :], d_last.ap()[j2][:, ts])
                for nch in range(4):
                    sl = slice(nch * 512, (nch + 1) * 512)
                    px = psA.tile([128, 512], FP, tag="px", name="px")
                    nc.tensor.matmul(px[:], ones[:, 0:128], wbt[:, sl],
                                     start=True, stop=False)
                    for j2 in range(2):
                        nc.tensor.matmul(px[:], lt[:, j2, :], wIT[:, j2, sl],
                                         start=False, stop=(j2 == 1))
                    if nch < 2:
                        nc.vector.tensor_scalar(xt[:, tt, sl], px[:], 16.0,
                                                None,
                                                op0=mybir.AluOpType.mult)
                    else:
                        nc.scalar.activation(xt[:, tt, sl], px[:], IdF,
                                             scale=16.0)

            APRE = 3
            for tt in range(APRE):
                emit_a(tt)

            wps = c2.enter_context(tc.tile_pool(name="wpsum", bufs=2,
                                                space="PSUM"))
            wpt = c2.enter_context(tc.tile_pool(name="wpt", bufs=2,
                                                space="PSUM"))
            rgp = c2.enter_context(tc.tile_pool(name="wring", bufs=1))
            # fp8 ring: h^T lives at cols 32-63 of a zero-padded window
            # tile; shifted 64-wide windows stack two gates per DR output
            rlist = []
            for ri in range(3):
                rt = rgp.tile([128, 4, 96], F8T, tag=f"r{ri}",
                              name=f"r{ri}")
                nc.vector.memset(rt[:], 0.0)
                rlist.append(rt)
            c32 = wst.tile([32, 512], BF, tag="c32", name="c32")
            nc.vector.memset(c32[:], 0.0)

            banks = {}

            def emit_inject(s):
                tt, so = divmod(s, 4)
                rs = slice(32 * so, 32 * so + 32)
                pA = wps.tile([64, 512], FP, tag="pA", name="pA")
                pB = wps.tile([64, 512], FP, tag="pB", name="pB")
                banks[s] = (pA, pB)
                for ti, pt_ in ((0, pA), (1, pB)):
                    for half in range(2):
                        g4 = (2 * ti + half) * 512
                        nc.tensor.matmul(pt_[32 * half:32 * half + 32, :],
                                         eyeb[rs, :],
                                         xt[rs, tt, g4:g4 + 512],
                                         start=True, stop=(s == 0),
                                         tile_position=(32 * so, 32 * half),
                                         skip_group_check=True)

            emit_inject(0)
            for s in range(S):
                tt, so = divmod(s, 4)
                if so == 0 and tt + APRE < NT:
                    emit_a(tt + APRE)
                pA, pB = banks.pop(s)
                ring = rlist[s % 3]
                nring = rlist[(s + 1) % 3]
                if s > 0:
                    for ti, pt_ in ((0, pA), (1, pB)):
                        for q in range(2):
                            for half in range(2):
                                g4 = (2 * ti + half) * 512
                                win = slice(32, 96) if half == 0 else \
                                    slice(0, 64)
                                nc.tensor.matmul(
                                    pt_[:], ring[:, 2 * q:2 * q + 2, win],
                                    wh[:, q, :, g4:g4 + 512],
                                    perf_mode=DR, start=False,
                                    stop=(q == 1 and half == 1),
                                    skip_group_check=True)
                if s + 1 < S:
                    emit_inject(s + 1)
                # acts: pA = (f|i) sigmoid; pB = (o|g') sigmoid with the
                # g strip at 2x scale (tanh(x) = 2*sigmoid(2x)-1)
                fi = wtmp.tile([64, 512], BF, tag="fi", name="fi")
                og = wtmp.tile([64, 512], BF, tag="og", name="og")
                nc.scalar.activation(fi[:], pA[:], Sig, scale=1.0 / 16.0)
                nc.scalar.activation(og[:], pB[:], Sig, scale=scl[:])
                g0 = wtmp.tile([64, 512], BF, tag="g0", name="g0")
                nc.vector.tensor_scalar(g0[32:64, :], og[32:64, :], 2.0,
                                        -1.0, op0=mybir.AluOpType.mult,
                                        op1=mybir.AluOpType.add)
                fc = wtmp.tile([32, 512], BF, tag="fc", name="fc")
                nc.vector.tensor_mul(fc[:], fi[0:32, :], c32[:])
                ig = wtmp.tile([32, 512], BF, tag="ig", name="ig")
                nc.vector.tensor_mul(ig[:], fi[32:64, :], g0[32:64, :])
                nc.vector.tensor_add(c32[:], fc[:], ig[:])
                # transposed tail, all inputs at base partition 0
                co = wpt.tile([128, 2, 4, bl2], BF, tag="co", name="co")
                # o-transposes first: they depend only on the act and run
                # during the DVE cell chain instead of queueing behind the
                # c-transposes (which wait on the add) in the PE FIFO
                for kk in range(4):
                    nc.tensor.transpose(co[:, 1, kk, :],
                                        og[0:32, kk * 128:(kk + 1) * 128],
                                        eyeb[0:32, 0:bl2])
                for kk in range(4):
                    nc.tensor.transpose(co[:, 0, kk, :],
                                        c32[:, kk * 128:(kk + 1) * 128],
                                        eyeb[0:32, 0:bl2])
                tct = wtmp.tile([128, 4, bl2], BF, tag="tct", name="tct")
                nc.scalar.activation(tct[:], co[:, 0, :, :], TanhF)
                nc.vector.tensor_mul(nring[:, :, 32:64], tct[:],
                                     co[:, 1, :, :])
                nc.vector.tensor_mul(hsT[:, :, s, :], tct[:],
                                     co[:, 1, :, :])
            nc.sync.dma_start(
                d_hs.ap().rearrange("k p (s b) -> p k s b", b=bl2), hsT[:])
    nc.compile()
    return nc


def build_l3(bl=BL):
    """MLP + log_softmax, data-parallel."""
    nl = bl * S
    nc = bacc.Bacc("TRN2", target_bir_lowering=False, debug=False,
                   num_devices=NCORE)
    d_hs = nc.dram_tensor("hsT8", [8, 128, nl], BF, kind="ExternalInput")
    d_W1T = nc.dram_tensor("W1T", [8, 128, 256], BF, kind="ExternalInput")
    d_b1 = nc.dram_tensor("b1m", [128, 2], FP, kind="ExternalInput")
    d_W2T = nc.dram_tensor("W2T", [2, 128, 256], BF, kind="ExternalInput")
    d_b2 = nc.dram_tensor("b2m", [128, 2], FP, kind="ExternalInput")
    d_W3T = nc.dram_tensor("W3T", [2, 128, OUT], BF, kind="ExternalInput")
    d_b3 = nc.dram_tensor("b3m", [OUT, 1], FP, kind="ExternalInput")
    d_eye = nc.dram_tensor("eye", [128, 128], FP, kind="ExternalInput")
    d_y = nc.dram_tensor("y", [nl, OUT], FP, kind="ExternalOutput")

    CH = min(512, nl)
    NCH = (nl + CH - 1) // CH

    with tile.TileContext(nc) as tc:
        with ExitStack() as c3:
            mw = c3.enter_context(tc.tile_pool(name="mweights", bufs=1))
            mact = c3.enter_context(tc.tile_pool(name="mact", bufs=1))
            mtmp = c3.enter_context(tc.tile_pool(name="mtmp", bufs=4))
            mps = c3.enter_context(tc.tile_pool(name="mpsum", bufs=2,
                                                space="PSUM"))
            sps = c3.enter_context(tc.tile_pool(name="spsum", bufs=2,
                                                space="PSUM"))
            eye_sb = mw.tile([128, 128], FP, tag="eye", name="eye")
            nc.sync.dma_start(eye_sb[:], d_eye.ap()[:])
            W1 = mw.tile([128, 8, 256], BF, tag="W1", name="W1")
            W2 = mw.tile([128, 2, 256], BF, tag="W2", name="W2")
            W3 = mw.tile([128, 2, OUT], BF, tag="W3", name="W3")
            b1 = mw.tile([128, 2], FP, tag="b1", name="b1")
            b2 = mw.tile([128, 2], FP, tag="b2", name="b2")
            b3 = mw.tile([OUT, 1], FP, tag="b3", name="b3")
            nc.sync.dma_start(W1[:], d_W1T.ap().rearrange("k p g -> p k g"))
            nc.sync.dma_start(W2[:], d_W2T.ap().rearrange("k p g -> p k g"))
            nc.sync.dma_start(W3[:], d_W3T.ap().rearrange("k p g -> p k g"))
            nc.sync.dma_start(b1[:], d_b1.ap()[:])
            nc.sync.dma_start(b2[:], d_b2.ap()[:])
            nc.sync.dma_start(b3[:], d_b3.ap()[:])
            hsT = [mw.tile([128, nl], BF, tag=f"hsT{k}", name=f"hsT{k}")
                   for k in range(8)]
            for ci in range((nl + 511) // 512):
                cs = slice(ci * 512, min(nl, (ci + 1) * 512))
                for k in range(8):
                    nc.sync.dma_start(hsT[k][:, cs], d_hs.ap()[k][:, cs])
            h1 = [mact.tile([128, nl], BF, tag=f"h1{m}", name=f"h1{m}")
                  for m in range(2)]
            h2 = [mact.tile([128, nl], BF, tag=f"h2{m}", name=f"h2{m}")
                  for m in range(2)]
            for ci in range(NCH):
                cs = slice(ci * CH, (ci + 1) * CH)
                for m in range(2):
                    p = mps.tile([128, CH], FP, tag="mp1", name="mp1")
                    for k in range(8):
                        nc.tensor.matmul(
                            p[:], W1[:, k, m * 128:(m + 1) * 128],
                            hsT[k][:, cs], start=(k == 0), stop=(k == 7))
                    nc.scalar.activation(h1[m][:, cs], p[:], ReluF,
                                         bias=b1[:, m:m + 1])
            for ci in range(NCH):
                cs = slice(ci * CH, (ci + 1) * CH)
                for m in range(2):
                    p = mps.tile([128, CH], FP, tag="mp2", name="mp2")
                    for k in range(2):
                        nc.tensor.matmul(
                            p[:], W2[:, k, m * 128:(m + 1) * 128],
                            h1[k][:, cs], start=(k == 0), stop=(k == 1))
                    nc.scalar.activation(h2[m][:, cs], p[:], ReluF,
                                         bias=b2[:, m:m + 1])
            # two passes batched by ACT function: all Exp, then all Ln,
            # so the Exp/Ln activation tables load once each instead of
            # per position-tile
            npt = max(1, nl // 128)
            lgs = [mact.tile([128, OUT], FP, tag=f"lgs{pi}", name=f"lgs{pi}")
                   for pi in range(npt)]
            nmxs = [mact.tile([128, 1], FP, tag=f"nmx{pi}", name=f"nmx{pi}")
                    for pi in range(npt)]
            sms = [mact.tile([128, 1], FP, tag=f"sm{pi}", name=f"sm{pi}")
                   for pi in range(npt)]
            for pi in range(npt):
                pcount = min(128, nl - pi * 128)
                psl = slice(pi * 128, pi * 128 + pcount)
                lg = mps.tile([OUT, pcount], FP, tag="mp3", name="mp3")
                for k in range(2):
                    nc.tensor.matmul(lg[:], W3[:, k, :], h2[k][:, psl],
                                     start=(k == 0), stop=(k == 1))
                lgb = mtmp.tile([OUT, pcount], FP, tag="lgb", name="lgb")
                nc.scalar.activation(lgb[:], lg[:], IdentF, bias=b3[:, 0:1])
                lgr = sps.tile([pcount, OUT], FP, tag="lgr", name="lgr")
                nc.tensor.transpose(lgr[:], lgb[:], eye_sb[0:OUT, 0:OUT])
                nc.vector.tensor_reduce(nmxs[pi][0:pcount, :], lgr[:],
                                        axis=mybir.AxisListType.X,
                                        op=mybir.AluOpType.max, negate=True)
                ex = mtmp.tile([pcount, OUT], FP, tag="ex", name="ex")
                nc.scalar.activation(ex[:], lgr[:], ExpF,
                                     bias=nmxs[pi][0:pcount, :],
                                     accum_out=sms[pi][0:pcount, :])
                nc.vector.tensor_copy(lgs[pi][0:pcount, :], lgr[:])
            for pi in range(npt):
                pcount = min(128, nl - pi * 128)
                psl = slice(pi * 128, pi * 128 + pcount)
                lsm = mtmp.tile([pcount, 1], FP, tag="lsm", name="lsm")
                nc.scalar.activation(lsm[:], sms[pi][0:pcount, :], LnF)
                shift = mtmp.tile([pcount, 1], FP, tag="shift", name="shift")
                nc.vector.tensor_sub(shift[:], nmxs[pi][0:pcount, :], lsm[:])
                yt = mtmp.tile([pcount, OUT], FP, tag="yt", name="yt")
                nc.vector.tensor_scalar(yt[:], lgs[pi][0:pcount, :],
                                        shift[:], None,
                                        op0=mybir.AluOpType.add)
                nc.sync.dma_start(d_y.ap()[psl, :], yt[:])
    nc.compile()
    return nc


def _prep_shared(inputs):
    f32 = np.float32
    cWxT = np.asarray(inputs["cW_ih"], f32).T
    cWx2 = np.zeros((128, 4, 128), f32)
    for j in range(2):
        for pi in range(2):
            cWx2[0:E, 2 * j + pi] = cWxT[:, (j + 4 * pi) * 128:
                                         (j + 4 * pi) * 128 + 128]
            cWx2[E:128, 2 * j + pi] = cWxT[:, (2 + j + 4 * pi) * 128:
                                           (2 + j + 4 * pi) * 128 + 128]
    cWhT = np.ascontiguousarray(
        np.asarray(inputs["cW_hh"], f32).T).reshape(2, 128, G4)
    cbias = (np.asarray(inputs["cb_ih"], f32)
             + np.asarray(inputs["cb_hh"], f32))
    cbias_m = np.ascontiguousarray(cbias.reshape(G4 // 128, 128).T)
    wW, wb = [], []
    for pre in ("f", "b"):
        wih = np.asarray(inputs[pre + "W_ih"], f32)
        whh = np.asarray(inputs[pre + "W_hh"], f32)
        wW.append(np.ascontiguousarray(
            np.concatenate([wih.T, whh.T], 0)).reshape(6, 128, WG))
        wb.append((np.asarray(inputs[pre + "b_ih"], f32)
                   + np.asarray(inputs[pre + "b_hh"], f32)).reshape(1, WG))
    W1T = np.ascontiguousarray(
        np.asarray(inputs["W1"], f32).T.astype(BF_NP)).reshape(8, 128, 256)
    b1m = np.ascontiguousarray(np.asarray(inputs["b1"], f32).reshape(2, 128).T)
    W2T = np.ascontiguousarray(
        np.asarray(inputs["W2"], f32).T.astype(BF_NP)).reshape(2, 128, 256)
    b2m = np.ascontiguousarray(np.asarray(inputs["b2"], f32).reshape(2, 128).T)
    W3T = np.ascontiguousarray(
        np.asarray(inputs["W3"], f32).T.astype(BF_NP)).reshape(2, 128, OUT)
    b3m = np.ascontiguousarray(np.asarray(inputs["b3"], f32).reshape(OUT, 1))
    eye = np.eye(128, dtype=f32)
    onesr = np.ones((1, 128), f32)
    return dict(cWx2=cWx2.astype(BF_NP), cWhT=cWhT.astype(BF_NP),
                cbias=cbias_m, wW=wW, wb=wb, W1T=W1T,
                b1m=b1m, W2T=W2T, b2m=b2m, W3T=W3T, b3m=b3m, eye=eye,
                onesr=onesr)


def _prep_l1_v2(inputs):
    f32 = np.float32
    wih = np.asarray(inputs["cW_ih"], f32)      # [1024, 64], order i,f,g,o
    whh = np.asarray(inputs["cW_hh"], f32)      # [1024, 256]
    b = (np.asarray(inputs["cb_ih"], f32) + np.asarray(inputs["cb_hh"], f32))
    sg = np.ones((4 * Hc, 1), f32) * 16.0
    sg[2 * Hc:3 * Hc] *= 2.0                    # g rows: tanh = 2*sig(2x)-1
    wihs, whhs, bs = wih * sg, whh * sg, b * sg[:, 0]
    # cWx65[k, j, gi, m]: k<64 -> W_ih[gate, k]; k=64 -> bias; gate = 256gi+128j+m
    cWx = np.zeros((65, 2, 4, 128), f32)
    g4 = wihs.reshape(4, 2, 128, 64)            # [gi, j, m, k]
    cWx[0:64] = g4.transpose(3, 1, 0, 2)
    cWx[64] = bs.reshape(4, 2, 128).transpose(1, 0, 2)
    # cWh8[p, j, r, gi, m] = whhs[256gi+128j+m, 128r+p]
    a = whhs.reshape(4, 2, 128, 2, 128)         # [gi, j, m, r, p]
    cWh8 = np.ascontiguousarray(a.transpose(4, 1, 3, 0, 2)).astype(F8_NP)
    return dict(cWx65=cWx.astype(BF_NP), cWh8=cWh8)


def _l1_maps_v2(inputs, sh, bl, ncores):
    x = np.asarray(inputs["x"])
    emb = np.asarray(inputs["emb"], np.float32)
    nl = bl * S
    maps = []
    for c in range(ncores):
        xc = x[c * bl:(c + 1) * bl].reshape(nl, Lc)
        lengths = (xc != 0).sum(axis=1).astype(np.float32)
        lenrep = np.ascontiguousarray(
            np.broadcast_to(lengths[None, :].astype(BF_NP), (128, nl)))
        eT = np.ones((Lc, 65, nl), np.float32)
        eT[:, 0:64, :] = emb[xc].transpose(1, 2, 0)
        maps.append(dict(eT65=np.ascontiguousarray(eT.astype(BF_NP)),
                         lenrep=lenrep, cWx65=sh["cWx65"],
                         cWh8=sh["cWh8"]))
    return maps


def _l1_maps(inputs, sh, bl, ncores):
    x = np.asarray(inputs["x"])
    emb = np.asarray(inputs["emb"], np.float32).astype(BF_NP)
    nl = bl * S
    maps = []
    for c in range(ncores):
        xc = x[c * bl:(c + 1) * bl].reshape(nl, Lc)
        lengths = (xc != 0).sum(axis=1).astype(np.float32)
        lenrep = np.ascontiguousarray(
            np.broadcast_to(lengths[None, :].astype(BF_NP), (128, nl)))
        eT = np.ascontiguousarray(emb[xc].transpose(1, 2, 0))
        maps.append(dict(eT=eT, lenrep=lenrep, cWx2=sh["cWx2"],
                         cWhT=sh["cWhT"], cbias=sh["cbias"]))
    return maps


@functools.lru_cache(maxsize=4)
def _modules(bl, lmin=18):
    return build_l1_v2(bl, lmin), build_l2_v5(32, 2), build_l3(bl)


def _gate_blocks(w, order):
    h4 = w.shape[0] // 4
    return np.concatenate([w[g * h4:(g + 1) * h4] for g in order], axis=0)


ORD = (1, 0, 3, 2)   # PyTorch (i,f,g,o) -> (f,i,o,g)
F8_NP = ml_dtypes.float8_e4m3


def _prep_l2_v4(inputs):
    f32 = np.float32
    wh8, wx8, wb16 = [], [], []
    sc = np.full((4 * H, 1), 16.0, f32)
    sc[3 * H:] *= 2.0          # tanh(g) = 2*sig(2x)-1: fold the 2x here
    for pre in ("f", "b"):
        wih = _gate_blocks(np.asarray(inputs[pre + "W_ih"], f32), ORD)
        whh = _gate_blocks(np.asarray(inputs[pre + "W_hh"], f32), ORD)
        b = (_gate_blocks(np.asarray(inputs[pre + "b_ih"], f32), ORD)
             + _gate_blocks(np.asarray(inputs[pre + "b_hh"], f32), ORD))
        whhT = (whh * sc).T                      # [512, 2048]
        wihT = (wih * sc).T                      # [256, 2048]
        a = whhT.reshape(2, 2, 128, 16, 128)     # [q, r, p, gb, m]
        wh8.append(np.ascontiguousarray(
            a.transpose(2, 0, 3, 1, 4)).astype(F8_NP))
        a = wihT.reshape(2, 128, 16, 128)        # [r, p, gb, m]
        wx8.append(np.ascontiguousarray(
            a.transpose(1, 2, 0, 3)).astype(BF_NP))
        wb16.append(np.ascontiguousarray(
            (b * sc[:, 0]).reshape(1, 16, 128)).astype(BF_NP))
    return dict(wh8=wh8, wx8=wx8, wb16=wb16,
                ones32=np.ones((1, 32), BF_NP))


def _l2_maps_v4(last_full, sh2, ncores):
    maps = []
    half = ncores // 2
    for c in range(ncores):
        d = 0 if c < half else 1
        g = c % half
        lt = last_full[:, :, g * 32 * S:(g + 1) * 32 * S].reshape(
            2, 128, 32, S)
        if d == 1:
            lt = lt[:, :, :, ::-1]
        lt = lt.transpose(1, 0, 3, 2)            # [128, 2, S, 32]
        maps.append(dict(lastT16=np.ascontiguousarray(lt).astype(BF_NP),
                         wh8v4=sh2["wh8"][d], wx16v4=sh2["wx8"][d],
                         wb16=sh2["wb16"][d], ones32=sh2["ones32"]))
    return maps


def _prep_l2(inputs):
    f32 = np.float32
    wIT, wb, wh8 = [], [], []
    for pre in ("f", "b"):
        wih = _gate_blocks(np.asarray(inputs[pre + "W_ih"], f32), ORD)
        whh = _gate_blocks(np.asarray(inputs[pre + "W_hh"], f32), ORD)
        wIT.append(np.ascontiguousarray(wih.T.astype(BF_NP)).reshape(
            2, 128, 4 * H))
        wb.append((_gate_blocks(np.asarray(inputs[pre + "b_ih"], f32), ORD)
                   + _gate_blocks(np.asarray(inputs[pre + "b_hh"], f32),
                                  ORD)).reshape(1, 4 * H).astype(BF_NP))
        whhT = whh.T * 16.0
        wh8.append(np.ascontiguousarray(
            whhT.reshape(2, 2, 128, 4 * H).transpose(0, 2, 1, 3)
            .astype(F8_NP)))
    eyeb = np.zeros((128, 32), f32)
    for p in range(128):
        eyeb[p, p % 32] = 1.0
    scl64 = np.full((64, 1), 1.0 / 16.0, f32)
    scl64[32:64] = 2.0 / 16.0
    return dict(wIT=wIT, wb=wb, wh8=wh8, eyeb=eyeb.astype(BF_NP),
                scl64=scl64, onesr=np.ones((1, 128), f32).astype(BF_NP))


def _l2_maps_v3(last_full, sh2, ncores):
    maps = []
    half = ncores // 2
    for c in range(ncores):
        d = 0 if c < half else 1
        g = c % half
        lt = last_full[:, :, g * 32 * S:(g + 1) * 32 * S].reshape(
            2, 128, 32, S)
        if d == 1:
            lt = lt[:, :, :, ::-1]
        lt = np.ascontiguousarray(
            lt.transpose(0, 1, 3, 2).reshape(2, 128, 32 * S)).astype(BF_NP)
        maps.append(dict(lastT2=lt, wIT=sh2["wIT"][d], wb=sh2["wb"][d],
                         onesr=sh2["onesr"], eyeb=sh2["eyeb"],
                         scl64=sh2["scl64"], wh8=sh2["wh8"][d]))
    return maps


def _l3_maps_v3(hs_f, hs_b, sh, bl, ncores):
    nl = bl * S
    maps = []
    for c in range(ncores):
        g, hf = c // 2, c % 2
        sl = slice(hf * nl, (hf + 1) * nl)
        hs8 = np.concatenate([hs_f[g][:, :, sl], hs_b[g][:, :, sl]], axis=0)
        maps.append(dict(hsT8=np.ascontiguousarray(hs8), W1T=sh["W1T"],
                         b1m=sh["b1m"], W2T=sh["W2T"], b2m=sh["b2m"],
                         W3T=sh["W3T"], b3m=sh["b3m"], eye=sh["eye"]))
    return maps


def _pipeline(inputs, bl, ncores, run_l1, run_l2, run_l3):
    sh = _prep_shared(inputs)
    sh2 = _prep_l2_v4(inputs)
    half = ncores // 2

    r1 = run_l1(_l1_maps_v2(inputs, _prep_l1_v2(inputs), bl, ncores))
    last_full = np.concatenate(
        [np.asarray(r1[c]["lastT"]).astype(np.float32)
         for c in range(ncores)], axis=2)

    r2 = run_l2(_l2_maps_v4(last_full, sh2, ncores))
    hs_f, hs_b = [], []
    for g in range(half):
        hs_f.append(np.asarray(r2[g]["hsTh"]))
        hb = np.asarray(r2[half + g]["hsTh"]).reshape(4, 128, S, 32)
        hs_b.append(np.ascontiguousarray(
            hb[:, :, ::-1, :]).reshape(4, 128, 32 * S))

    r3 = run_l3(_l3_maps_v3(hs_f, hs_b, sh, bl, ncores))
    out = np.empty((B, S, OUT), np.float32)
    for c in range(ncores):
        y = np.asarray(r3[c]["y"]).reshape(S // 2, 32, OUT)
        bs = 32 * (c // 2)
        ss = (S // 2) * (c % 2)
        out[bs:bs + 32, ss:ss + S // 2] = y.transpose(1, 0, 2)
    return out


def kernel(**inputs):
    x = np.asarray(inputs["x"])
    lmin = int((x.reshape(-1, Lc) != 0).sum(axis=1).min())
    l1, l2, l3 = _modules(BL, lmin)

    def runner(nc):
        def run(in_maps):
            res = bass_utils.run_bass_kernel_spmd(
                nc, in_maps, core_ids=list(range(NCORE)))
            return res.results
        return run

    return _pipeline(inputs, BL, NCORE, runner(l1), runner(l2), runner(l3))



# revision 17
# speedup vs baseline: 1.0690x; 1.0302x over previous
"""Trainium2 Bass kernel for nn_CharTaggerBiLSTM, 8-core SPMD, 3 launches.

L1 char LSTM: data-parallel over batch (16 sentences/core). Transposed
   layout (features-on-partitions), f32r matmuls; emits the masked last
   hidden state per word -> DRAM.
L2 word LSTM: one direction per core (cores 0-3 forward, 4-7 backward),
   32 sentences/core so each weight stream serves twice the rows.
   Direction is data: backward cores receive the char outputs with the
   sentence axis reversed on host and their outputs are un-reversed.
   x-part/bias matmuls for step s+1 are issued during step s's
   elementwise work to keep PE fed.
L3 MLP + log_softmax: data-parallel (16 sentences/core), bf16 GEMMs.

Host does embedding gather, weight reshapes, the two reshard steps, and
reassembly.
"""

import sys
import functools
from contextlib import ExitStack

sys.path.insert(0, "/opt/trn_rl_repo")

import numpy as np
import ml_dtypes
from concourse import bacc, bass, mybir, tile, bass_utils

BF_NP = ml_dtypes.bfloat16
U8 = mybir.dt.uint8
F8T = mybir.dt.float8e4
DR = mybir.MatmulPerfMode.DoubleRow
E = 64


B, S, Lc = 128, 128, 20
AB, E = 100, 64
Hc, H, OUT = 256, 512, 50
NCORE = 8
BL = B // NCORE            # sentences per core in L1/L3
FP = mybir.dt.float32
FR = mybir.dt.float32r
BF = mybir.dt.bfloat16
G4 = 4 * Hc
WG = 4 * H

Sig = mybir.ActivationFunctionType.Sigmoid
TanhF = mybir.ActivationFunctionType.Tanh
ReluF = mybir.ActivationFunctionType.Relu
ExpF = mybir.ActivationFunctionType.Exp
LnF = mybir.ActivationFunctionType.Ln
IdentF = mybir.ActivationFunctionType.Identity


def build_l1(bl=BL, lmin=18):
    """Char LSTM, data-parallel; writes lastT [2,128,nl] bf16 to DRAM."""
    nl = bl * S
    nc = bacc.Bacc("TRN2", target_bir_lowering=False, debug=False,
                   num_devices=NCORE)
    d_eT = nc.dram_tensor("eT", [Lc, E, nl], BF, kind="ExternalInput")
    d_lenrep = nc.dram_tensor("lenrep", [128, nl], BF, kind="ExternalInput")
    d_cWx2 = nc.dram_tensor("cWx2", [128, 4, 128], BF, kind="ExternalInput")
    d_cWhT = nc.dram_tensor("cWhT", [2, 128, G4], BF, kind="ExternalInput")
    d_cbias = nc.dram_tensor("cbias", [128, G4 // 128], FP,
                             kind="ExternalInput")
    d_last = nc.dram_tensor("lastT", [2, 128, nl], BF, kind="ExternalOutput")

    CH = 1024
    NCH = nl // CH
    GF = [Sig, Sig, TanhF, Sig]          # gate funcs for gi = i, f, g, o

    with tile.TileContext(nc) as tc:
        with ExitStack() as c1:
            cw = c1.enter_context(tc.tile_pool(name="cweights", bufs=1))
            cst = c1.enter_context(tc.tile_pool(name="cstate", bufs=1))
            ein = c1.enter_context(tc.tile_pool(name="ein", bufs=2))
            ctmp = c1.enter_context(tc.tile_pool(name="ctmp", bufs=3))
            cps = c1.enter_context(tc.tile_pool(name="cpsum", bufs=4,
                                                space="PSUM"))
            cWx2 = cw.tile([128, 4, 128], BF, tag="cWx2", name="cWx2")
            cWh = cw.tile([128, 2, G4], BF, tag="cWh", name="cWh")
            cb = cw.tile([128, G4 // 128], FP, tag="cb", name="cb")
            lenr = cw.tile([128, nl], BF, tag="lenr", name="lenr")
            nc.sync.dma_start(cWx2[:], d_cWx2.ap()[:])
            nc.sync.dma_start(cWh[:], d_cWhT.ap().rearrange("k p g -> p k g"))
            nc.sync.dma_start(cb[:], d_cbias.ap()[:])
            nc.sync.dma_start(lenr[:], d_lenrep.ap()[:])

            last = cst.tile([128, 2, nl], BF, tag="last", name="last")
            hh = [cst.tile([128, 2, nl], BF, tag=f"h{p}", name=f"h{p}")
                  for p in range(2)]
            cc = cst.tile([128, 2, nl], BF, tag="cc", name="cc")
            nc.vector.memset(cc[:], 0.0)
            nc.vector.memset(last[:], 0.0)

            for t in range(Lc):
                et2 = ein.tile([128, nl], BF, tag="et2", name="et2")
                nc.sync.dma_start(et2[0:E, :], d_eT.ap()[t])
                nc.sync.dma_start(et2[E:128, :], d_eT.ap()[t])
                masked = t >= lmin - 1
                hprev = hh[t % 2]
                hcur = hh[(t + 1) % 2]
                for ci in range(NCH):
                    cs = slice(ci * CH, (ci + 1) * CH)
                    if masked:
                        mk = ctmp.tile([128, CH], U8, tag="mk", name="mk")
                        nc.gpsimd.tensor_scalar(mk[:], lenr[:, cs], float(t),
                                                None,
                                                op0=mybir.AluOpType.is_gt)
                    for j in range(2):
                        ps = [cps.tile([128, CH], FP, tag="ps", name="ps")
                              for _ in range(4)]
                        # x-part: two K=64 row strips per PE pass, plus
                        # bias-free accumulation of the two h chunks;
                        # matmul outputs are split into 512-col halves
                        # (one PSUM bank each)
                        for hw_ in range(2):
                            o5 = slice(hw_ * 512, (hw_ + 1) * 512)
                            c5 = slice(ci * CH + hw_ * 512,
                                       ci * CH + (hw_ + 1) * 512)
                            for pi in range(2):
                                sl = 2 * j + pi
                                nc.tensor.matmul(ps[2 * pi][:, o5],
                                                 cWx2[0:E, sl, :],
                                                 et2[0:E, c5],
                                                 start=True, stop=(t == 0))
                                nc.tensor.matmul(ps[2 * pi + 1][:, o5],
                                                 cWx2[E:128, sl, :],
                                                 et2[E:128, c5],
                                                 start=True, stop=(t == 0))
                            if t > 0:
                                for gi in range(4):
                                    m = 2 * gi + j
                                    for k in range(2):
                                        nc.tensor.matmul(
                                            ps[gi][:, o5],
                                            cWh[:, k, m * 128:(m + 1) * 128],
                                            hprev[:, k, c5],
                                            start=False, stop=(k == 1))
                        gsb = [ctmp.tile([128, CH], BF, tag=f"g{gi}",
                                         name=f"g{gi}") for gi in range(4)]
                        for gi in range(4):
                            m = 2 * gi + j
                            nc.scalar.activation(gsb[gi][:], ps[gi][:],
                                                 GF[gi], bias=cb[:, m:m + 1])
                        ig = ctmp.tile([128, CH], BF, tag="ig", name="ig")
                        nc.vector.tensor_mul(ig[:], gsb[0][:], gsb[2][:])
                        fc = ctmp.tile([128, CH], BF, tag="fc", name="fc")
                        nc.vector.tensor_mul(fc[:], gsb[1][:], cc[:, j, cs])
                        nc.vector.tensor_add(cc[:, j, cs], fc[:], ig[:])
                        tct = ctmp.tile([128, CH], BF, tag="tct", name="tct")
                        nc.scalar.activation(tct[:], cc[:, j, cs], TanhF)
                        nc.vector.tensor_mul(hcur[:, j, cs], gsb[3][:],
                                             tct[:])
                        if masked:
                            nc.vector.copy_predicated(last[:, j, cs], mk[:],
                                                      hcur[:, j, cs])
            for j in range(2):
                nc.sync.dma_start(d_last.ap()[j], last[:, j, :])
    nc.compile()
    return nc


def build_l1_v2(bl=BL, lmin=18):
    """Char LSTM v2: act-engine-optimized.

    Psum per (word-chunk-512, j-half): [128, 4, 512] f32 (4 banks, one
    per gate type i,f,g,o; start=True only on each bank's first mm).
    x-part: K=65 bf16 (emb row 64 = ones -> bias rides in the weights,
    g rows pre-scaled 2x so one sigmoid serves tanh(g) as 2*sig(2x)-1).
    h-part: fp8 DoubleRow (h ring and W_hh both fp8, W scaled 16x, act
    scale 1/16 undoes it... n.b. x/bias must then also be 16x).
    One sigmoid act per (chunk, j) over all 4 gates; tails run j-wide
    on [128, 2048] slices; h*o on gpsimd.
    """
    nl = bl * S
    nc = bacc.Bacc("TRN2", target_bir_lowering=False, debug=False,
                   num_devices=NCORE)
    d_eT = nc.dram_tensor("eT65", [Lc, 65, nl], BF, kind="ExternalInput")
    d_lenrep = nc.dram_tensor("lenrep", [128, nl], BF, kind="ExternalInput")
    d_cWx = nc.dram_tensor("cWx65", [65, 2, 4, 128], BF,
                           kind="ExternalInput")
    d_cWh8 = nc.dram_tensor("cWh8", [128, 2, 2, 4, 128], F8T,
                            kind="ExternalInput")
    d_last = nc.dram_tensor("lastT", [2, 128, nl], BF, kind="ExternalOutput")

    CH = 512
    NCH = nl // CH
    with tile.TileContext(nc) as tc:
        with ExitStack() as c1:
            cw = c1.enter_context(tc.tile_pool(name="cweights", bufs=1))
            cst = c1.enter_context(tc.tile_pool(name="cstate", bufs=1))
            ein = c1.enter_context(tc.tile_pool(name="ein", bufs=2))
            ctmp = c1.enter_context(tc.tile_pool(name="ctmp", bufs=2))
            cps = c1.enter_context(tc.tile_pool(name="cpsum", bufs=2,
                                                space="PSUM"))
            cWx = cw.tile([65, 2, 4, 128], BF, tag="cWx", name="cWx")
            cWh = cw.tile([128, 2, 2, 4, 128], F8T, tag="cWh", name="cWh")
            lenr = cw.tile([128, nl], BF, tag="lenr", name="lenr")
            nc.sync.dma_start(cWx[:], d_cWx.ap()[:])
            nc.sync.dma_start(cWh[:], d_cWh8.ap()[:])
            nc.sync.dma_start(lenr[:], d_lenrep.ap()[:])

            last = cst.tile([128, 2, nl], BF, tag="last", name="last")
            # h ring: fp8 for the DR matmul + bf16 h for nothing else
            hh = [cst.tile([128, 2, nl], F8T, tag=f"h{p}", name=f"h{p}")
                  for p in range(2)]
            cc = cst.tile([128, 2, nl], BF, tag="cc", name="cc")
            gsb = [cst.tile([128, 4, nl], BF, tag=f"gs{j}", name=f"gs{j}")
                   for j in range(2)]
            g0t = [cst.tile([128, nl], BF, tag=f"g0{j}", name=f"g0{j}")
                   for j in range(2)]
            igt = [cst.tile([128, nl], BF, tag=f"ig{j}", name=f"ig{j}")
                   for j in range(2)]
            fct = [cst.tile([128, nl], BF, tag=f"fc{j}", name=f"fc{j}")
                   for j in range(2)]
            tctt = [cst.tile([128, nl], BF, tag=f"tc{j}", name=f"tc{j}")
                    for j in range(2)]
            nc.vector.memset(cc[:], 0.0)
            nc.vector.memset(last[:], 0.0)

            CK = 1024
            NCK = nl // CK

            def tail(j, ck, t, hcur, masked):
                ts_ = slice(ck * CK, (ck + 1) * CK)
                gj = gsb[j]
                nc.vector.tensor_scalar(g0t[j][:, ts_], gj[:, 2, ts_], 2.0,
                                        -1.0, op0=mybir.AluOpType.mult,
                                        op1=mybir.AluOpType.add)
                nc.vector.tensor_mul(igt[j][:, ts_], gj[:, 0, ts_],
                                     g0t[j][:, ts_])
                nc.vector.tensor_mul(fct[j][:, ts_], gj[:, 1, ts_],
                                     cc[:, j, ts_])
                nc.vector.tensor_add(cc[:, j, ts_], fct[j][:, ts_],
                                     igt[j][:, ts_])
                nc.scalar.activation(tctt[j][:, ts_], cc[:, j, ts_], TanhF)
                nc.gpsimd.tensor_mul(hcur[:, j, ts_], gj[:, 3, ts_],
                                     tctt[j][:, ts_])
                if masked:
                    mk = ctmp.tile([128, CK], U8, tag="mk", name="mk")
                    nc.gpsimd.tensor_scalar(mk[:], lenr[:, ts_], float(t),
                                            None,
                                            op0=mybir.AluOpType.is_gt)
                    hb = ctmp.tile([128, CK], BF, tag="hb", name="hb")
                    nc.vector.tensor_mul(hb[:], gj[:, 3, ts_],
                                         tctt[j][:, ts_])
                    nc.vector.copy_predicated(last[:, j, ts_], mk[:], hb[:])

            for t in range(Lc):
                et = ein.tile([65, nl], BF, tag="et", name="et")
                nc.sync.dma_start(et[:], d_eT.ap()[t])
                masked = t >= lmin - 1
                hprev = hh[t % 2]
                hcur = hh[(t + 1) % 2]
                for ci in range(NCH):
                    cs = slice(ci * CH, (ci + 1) * CH)
                    for j in range(2):
                        ps = cps.tile([128, 4, CH], FP, tag="ps", name="ps")
                        for gi in range(4):
                            nc.tensor.matmul(ps[:, gi, :],
                                             cWx[:, j, gi, :], et[:, cs],
                                             start=True,
                                             stop=(t == 0),
                                             skip_group_check=True)
                        if t > 0:
                            for gi in range(4):
                                nc.tensor.matmul(
                                    ps[:, gi, :], cWh[:, j, :, gi, :],
                                    hprev[:, :, cs],
                                    start=False, stop=True,
                                    perf_mode=DR, skip_group_check=True)
                        nc.scalar.activation(gsb[j][:, :, cs], ps[:], Sig,
                                             scale=1.0 / 16.0)
                    if ci % 2 == 1:
                        for j in range(2):
                            tail(j, ci // 2, t, hcur, masked)
            for j in range(2):
                nc.sync.dma_start(d_last.ap()[j], last[:, j, :])
    nc.compile()
    return nc


def build_l2_v4(bl2=32):
    """Word LSTM v4: gates-on-partitions, sentences-moving.

    Per step: psum tile [128, 16, 32] f32 (1 bank) holds all 2048 gates
    (16 blocks of 128 gate-dims on partitions) x 32 sentences on free.
    Seeded by bias (K=1 bf16) + x-part (fp8 DR vs last char state), then
    h-part (fp8 DR vs h^T ring) accumulates. One sigmoid act covers all
    gates (tanh(g) folded as 2*sig(2x)-1 with the 2x pre-scaled into the
    g rows host-side); cell tail on DVE; h^T ring written directly by
    the o*tanh(c) mul - no transposes, no inject.
    Gate type order: f, i, o, g (blocks 0:4, 4:8, 8:12, 12:16).
    """
    nl = bl2 * S
    nc = bacc.Bacc("TRN2", target_bir_lowering=False, debug=False,
                   num_devices=NCORE)
    d_lastT = nc.dram_tensor("lastT8", [128, 2, S, bl2], F8T,
                             kind="ExternalInput")
    d_wh8 = nc.dram_tensor("wh8v4", [128, 2, 16, 2, 128], F8T,
                           kind="ExternalInput")
    d_wx8 = nc.dram_tensor("wx8v4", [128, 16, 2, 128], F8T,
                           kind="ExternalInput")
    d_wb = nc.dram_tensor("wb16", [1, 16, 128], BF, kind="ExternalInput")
    d_ones = nc.dram_tensor("ones32", [1, bl2], BF, kind="ExternalInput")
    d_hs = nc.dram_tensor("hsTh", [4, 128, nl], BF, kind="ExternalOutput")

    with tile.TileContext(nc) as tc:
        with ExitStack() as c2:
            ww = c2.enter_context(tc.tile_pool(name="wweights", bufs=1))
            wst = c2.enter_context(tc.tile_pool(name="wstate", bufs=1))
            rgp = c2.enter_context(tc.tile_pool(name="wring", bufs=2))
            cpl = c2.enter_context(tc.tile_pool(name="wcell", bufs=2))
            sgp = c2.enter_context(tc.tile_pool(name="wsg", bufs=2))
            wtmp = c2.enter_context(tc.tile_pool(name="wtmp", bufs=2))
            wps = c2.enter_context(tc.tile_pool(name="wpsum", bufs=3,
                                                space="PSUM"))
            wh = ww.tile([128, 2, 16, 2, 128], F8T, tag="wh", name="wh")
            wx = ww.tile([128, 16, 2, 128], F8T, tag="wx", name="wx")
            wb = ww.tile([1, 16, 128], BF, tag="wb", name="wb")
            ones = ww.tile([1, bl2], BF, tag="ones", name="ones")
            lastT = ww.tile([128, 2, S, bl2], BF, tag="lastT", name="lastT")
            nc.sync.dma_start(wh[:], d_wh8.ap()[:])
            nc.sync.dma_start(wx[:], d_wx8.ap()[:])
            nc.sync.dma_start(wb[:], d_wb.ap()[:])
            nc.sync.dma_start(ones[:], d_ones.ap()[:])
            for sc in range(4):
                ss = slice(sc * (S // 4), (sc + 1) * (S // 4))
                nc.sync.dma_start(lastT[:, :, ss, :], d_lastT.ap()[:, :, ss, :])
            hsT = wst.tile([128, 4, S, bl2], BF, tag="hsT", name="hsT")

            rinit = [rgp.tile([128, 4, bl2], F8T, tag="ring", name=f"ri{i}")
                     for i in range(2)]
            for t_ in rinit:
                nc.vector.memset(t_[:], 0.0)
            ring_prev = rinit[1]
            cinit = [cpl.tile([128, 4, bl2], BF, tag="cc", name=f"ci{i}")
                     for i in range(2)]
            nc.vector.memset(cinit[1][:], 0.0)
            c_prev = cinit[1]

            for s in range(S):
                ps = wps.tile([128, 16, bl2], FP, tag="ps", name="ps")
                for gb in range(16):
                    nc.tensor.matmul(ps[:, gb, :], wb[:, gb, :], ones[:],
                                     start=True, stop=False,
                                     skip_group_check=True)
                for gb in range(16):
                    nc.tensor.matmul(ps[:, gb, :], wx[:, gb, :, :],
                                     lastT[:, :, s, :],
                                     start=False, stop=(s == 0),
                                     perf_mode=DR, skip_group_check=True)
                if s > 0:
                    for gb in range(16):
                        for q in range(2):
                            nc.tensor.matmul(
                                ps[:, gb, :], wh[:, q, gb, :, :],
                                ring_prev[:, 2 * q:2 * q + 2, :],
                                start=False, stop=(q == 1),
                                perf_mode=DR, skip_group_check=True)
                sg = sgp.tile([128, 16, bl2], BF, tag="sg", name="sg")
                nc.scalar.activation(sg[:], ps[:], Sig, scale=1.0 / 16.0)
                fco = wtmp.tile([128, 4, bl2], BF, tag="fco", name="fco")
                nc.vector.tensor_mul(fco[:], sg[:, 0:4, :], c_prev[:])
                tg = wtmp.tile([128, 4, bl2], BF, tag="tg", name="tg")
                nc.vector.scalar_tensor_tensor(
                    tg[:], sg[:, 12:16, :], 0.5, sg[:, 4:8, :],
                    op0=mybir.AluOpType.subtract, op1=mybir.AluOpType.mult)
                c_new = cpl.tile([128, 4, bl2], BF, tag="cc", name="cc")
                nc.vector.scalar_tensor_tensor(
                    c_new[:], tg[:], 2.0, fco[:],
                    op0=mybir.AluOpType.mult, op1=mybir.AluOpType.add)
                tct = wtmp.tile([128, 4, bl2], BF, tag="tct", name="tct")
                nc.scalar.activation(tct[:], c_new[:], TanhF)
                ring_new = rgp.tile([128, 4, bl2], F8T, tag="ring",
                                    name="ring")
                nc.vector.tensor_mul(ring_new[:], sg[:, 8:12, :], tct[:])
                nc.gpsimd.tensor_mul(hsT[:, :, s, :], sg[:, 8:12, :], tct[:])
                ring_prev = ring_new
                c_prev = c_new
            nc.sync.dma_start(
                d_hs.ap().rearrange("k p (s b) -> p k s b", b=bl2), hsT[:])
    nc.compile()
    return nc


def build_l2_v5(bl2=32, ngrp=2):
    """Word LSTM v5: like v4 but sentences split into ngrp interleaved
    groups with independent recurrence chains, so each group's
    (smaller) elementwise ops overlap the other group's matmuls."""
    nl = bl2 * S
    gw = bl2 // ngrp                 # sentences per group
    nc = bacc.Bacc("TRN2", target_bir_lowering=False, debug=False,
                   num_devices=NCORE)
    d_lastT = nc.dram_tensor("lastT16", [128, 2, S, bl2], BF,
                             kind="ExternalInput")
    d_wh8 = nc.dram_tensor("wh8v4", [128, 2, 16, 2, 128], F8T,
                           kind="ExternalInput")
    d_wx8 = nc.dram_tensor("wx16v4", [128, 16, 2, 128], BF,
                           kind="ExternalInput")
    d_wb = nc.dram_tensor("wb16", [1, 16, 128], BF, kind="ExternalInput")
    d_ones = nc.dram_tensor("ones32", [1, bl2], BF, kind="ExternalInput")
    d_hs = nc.dram_tensor("hsTh", [4, 128, nl], BF, kind="ExternalOutput")

    with tile.TileContext(nc) as tc:
        with ExitStack() as c2:
            ww = c2.enter_context(tc.tile_pool(name="wweights", bufs=1))
            wst = c2.enter_context(tc.tile_pool(name="wstate", bufs=1))
            rgp = c2.enter_context(tc.tile_pool(name="wring", bufs=2 * ngrp))
            cpl = c2.enter_context(tc.tile_pool(name="wcell", bufs=2 * ngrp))
            sgp = c2.enter_context(tc.tile_pool(name="wsg", bufs=2 * ngrp))
            wtmp = c2.enter_context(tc.tile_pool(name="wtmp", bufs=2 * ngrp))
            wps = c2.enter_context(tc.tile_pool(name="wpsum", bufs=3,
                                                space="PSUM"))
            wh = ww.tile([128, 2, 16, 2, 128], F8T, tag="wh", name="wh")
            wx = ww.tile([128, 16, 2, 128], BF, tag="wx", name="wx")
            wb = ww.tile([1, 16, 128], BF, tag="wb", name="wb")
            ones = ww.tile([1, bl2], BF, tag="ones", name="ones")
            lastT = ww.tile([128, 2, S, bl2], BF, tag="lastT", name="lastT")
            nc.sync.dma_start(wh[:], d_wh8.ap()[:])
            nc.sync.dma_start(wx[:], d_wx8.ap()[:])
            nc.sync.dma_start(wb[:], d_wb.ap()[:])
            nc.sync.dma_start(ones[:], d_ones.ap()[:])
            for sc in range(4):
                ss = slice(sc * (S // 4), (sc + 1) * (S // 4))
                nc.sync.dma_start(lastT[:, :, ss, :], d_lastT.ap()[:, :, ss, :])
            hsT = wst.tile([128, 4, S, bl2], BF, tag="hsT", name="hsT")

            c_prev, ring_prev = [], []
            for g in range(ngrp):
                ct = cpl.tile([128, 4, gw], BF, tag=f"cc{g}", name=f"ci{g}")
                nc.vector.memset(ct[:], 0.0)
                c_prev.append(ct)
                ring_prev.append(None)

            def step_mms(g, s):
                # psum tile is a full 2KB bank: start=True zeroes the whole
                # bank, so exactly one matmul (first bias) carries start.
                gs = slice(g * gw, (g + 1) * gw)
                ps = wps.tile([128, 16, 32], FP, tag=f"ps{g}", name=f"ps{g}")
                for gb in range(16):
                    nc.tensor.matmul(ps[:, gb, 0:gw], wb[:, gb, :],
                                     ones[:, gs],
                                     start=(gb == 0), stop=False,
                                     skip_group_check=True)
                for gb in range(16):
                    for r in range(2):
                        nc.tensor.matmul(ps[:, gb, 0:gw], wx[:, gb, r, :],
                                         lastT[:, r, s, gs],
                                         start=False,
                                         stop=(s == 0 and gb == 15
                                               and r == 1),
                                         skip_group_check=True)
                if s > 0:
                    for gb in range(16):
                        for q in range(2):
                            nc.tensor.matmul(
                                ps[:, gb, 0:gw], wh[:, q, gb, :, :],
                                ring_prev[g][:, 2 * q:2 * q + 2, :],
                                start=False,
                                stop=(gb == 15 and q == 1),
                                perf_mode=DR, skip_group_check=True)
                return ps

            def step_tail(g, s, ps):
                gs = slice(g * gw, (g + 1) * gw)
                sg = sgp.tile([128, 16, gw], BF, tag=f"sg{g}", name=f"sg{g}")
                nc.scalar.activation(sg[:], ps[:, :, 0:gw], Sig,
                                     scale=1.0 / 16.0)
                fco = wtmp.tile([128, 4, gw], BF, tag=f"fco{g}",
                                name=f"fco{g}")
                nc.vector.tensor_mul(fco[:], sg[:, 0:4, :], c_prev[g][:])
                tg = wtmp.tile([128, 4, gw], BF, tag=f"tg{g}", name=f"tg{g}")
                nc.vector.scalar_tensor_tensor(
                    tg[:], sg[:, 12:16, :], 0.5, sg[:, 4:8, :],
                    op0=mybir.AluOpType.subtract, op1=mybir.AluOpType.mult)
                c_new = cpl.tile([128, 4, gw], BF, tag=f"cc{g}",
                                 name=f"cc{g}")
                nc.vector.scalar_tensor_tensor(
                    c_new[:], tg[:], 2.0, fco[:],
                    op0=mybir.AluOpType.mult, op1=mybir.AluOpType.add)
                tct = wtmp.tile([128, 4, gw], BF, tag=f"tct{g}",
                                name=f"tct{g}")
                nc.scalar.activation(tct[:], c_new[:], TanhF)
                ring_new = rgp.tile([128, 4, gw], F8T, tag=f"ring{g}",
                                    name=f"ring{g}")
                nc.vector.tensor_mul(ring_new[:], sg[:, 8:12, :], tct[:])
                nc.gpsimd.tensor_mul(hsT[:, :, s, gs], sg[:, 8:12, :],
                                     tct[:])
                ring_prev[g] = ring_new
                c_prev[g] = c_new

            pend = {}
            for s in range(S):
                for g in range(ngrp):
                    pend[g] = step_mms(g, s)
                    og = (g + 1) % ngrp
                    if (og, 'tail') in pend:
                        gg, ss, pp = pend.pop((og, 'tail'))
                        step_tail(gg, ss, pp)
                    pend[(g, 'tail')] = (g, s, pend[g])
            for g in range(ngrp):
                if (g, 'tail') in pend:
                    gg, ss, pp = pend.pop((g, 'tail'))
                    step_tail(gg, ss, pp)
            nc.sync.dma_start(
                d_hs.ap().rearrange("k p (s b) -> p k s b", b=bl2), hsT[:])
    nc.compile()
    return nc


def build_l2(bl2=32, fp8=True):
    """Word LSTM v3: fp8 DoubleRow via half-pad windows, step-major."""
    nl = bl2 * S
    nc = bacc.Bacc("TRN2", target_bir_lowering=False, debug=False,
                   num_devices=NCORE)
    d_last = nc.dram_tensor("lastT2", [2, 128, nl], BF, kind="ExternalInput")
    d_wIT = nc.dram_tensor("wIT", [2, 128, WG], BF, kind="ExternalInput")
    d_wb = nc.dram_tensor("wb", [1, WG], BF, kind="ExternalInput")
    d_ones = nc.dram_tensor("onesr", [1, 128], BF, kind="ExternalInput")
    d_eyeb = nc.dram_tensor("eyeb", [128, 32], BF, kind="ExternalInput")
    d_scl = nc.dram_tensor("scl64", [64, 1], FP, kind="ExternalInput")
    d_wh = nc.dram_tensor("wh8", [2, 128, 2, WG], F8T, kind="ExternalInput")
    d_hs = nc.dram_tensor("hsTh", [4, 128, nl], BF, kind="ExternalOutput")
    NT = nl // 128
    IdF = mybir.ActivationFunctionType.Identity

    with tile.TileContext(nc) as tc:
        with ExitStack() as c2:
            ww = c2.enter_context(tc.tile_pool(name="wweights", bufs=1))
            wst = c2.enter_context(tc.tile_pool(name="wstate", bufs=1))
            wtmp = c2.enter_context(tc.tile_pool(name="wtmp", bufs=3))
            eyeb = ww.tile([128, 32], BF, tag="eyeb", name="eyeb")
            nc.sync.dma_start(eyeb[:], d_eyeb.ap()[:])
            ones = ww.tile([1, 128], BF, tag="ones", name="ones")
            nc.sync.dma_start(ones[:], d_ones.ap()[:])
            wbt = ww.tile([1, WG], BF, tag="wbt", name="wbt")
            nc.sync.dma_start(wbt[:], d_wb.ap()[:])
            scl = ww.tile([64, 1], FP, tag="scl", name="scl")
            nc.sync.dma_start(scl[:], d_scl.ap()[:])
            wh = ww.tile([128, 2, 2, WG], F8T, tag="wh", name="wh")
            nc.sync.dma_start(wh[:],
                              d_wh.ap().rearrange("q p i g -> p q i g"))
            xt = wst.tile([128, NT, WG], BF, tag="xt", name="xt")
            hsT = wst.tile([128, 4, S, bl2], BF, tag="hsT", name="hsT")

            lw = c2.enter_context(tc.tile_pool(name="lw", bufs=1))
            psA = c2.enter_context(tc.tile_pool(name="psA", bufs=2,
                                                space="PSUM"))
            lpool = c2.enter_context(tc.tile_pool(name="lpool", bufs=3))
            wIT = lw.tile([128, 2, WG], BF, tag="wIT", name="wIT")
            nc.sync.dma_start(wIT[:],
                              d_wIT.ap().rearrange("k p g -> p k g"))

            def emit_a(tt):
                ts = slice(tt * 128, (tt + 1) * 128)
                lt = lpool.tile([128, 2, 128], BF, tag="lt", name="lt")
                for j2 in range(2):
                    nc.sync.dma_start(lt[:, j2, :], d_last.ap()[j2][:, ts])
                for nch in range(4):
                    sl = slice(nch * 512, (nch + 1) * 512)
                    px = psA.tile([128, 512], FP, tag="px", name="px")
                    nc.tensor.matmul(px[:], ones[:, 0:128], wbt[:, sl],
                                     start=True, stop=False)
                    for j2 in range(2):
                        nc.tensor.matmul(px[:], lt[:, j2, :], wIT[:, j2, sl],
                                         start=False, stop=(j2 == 1))
                    if nch < 2:
                        nc.vector.tensor_scalar(xt[:, tt, sl], px[:], 16.0,
                                                None,
                                                op0=mybir.AluOpType.mult)
                    else:
                        nc.scalar.activation(xt[:, tt, sl], px[:], IdF,
                                             scale=16.0)

            APRE = 3
            for tt in range(APRE):
                emit_a(tt)

            wps = c2.enter_context(tc.tile_pool(name="wpsum", bufs=2,
                                                space="PSUM"))
            wpt = c2.enter_context(tc.tile_pool(name="wpt", bufs=2,
                                                space="PSUM"))
            rgp = c2.enter_context(tc.tile_pool(name="wring", bufs=1))
            # fp8 ring: h^T lives at cols 32-63 of a zero-padded window
            # tile; shifted 64-wide windows stack two gates per DR output
            rlist = []
            for ri in range(3):
                rt = rgp.tile([128, 4, 96], F8T, tag=f"r{ri}",
                              name=f"r{ri}")
                nc.vector.memset(rt[:], 0.0)
                rlist.append(rt)
            c32 = wst.tile([32, 512], BF, tag="c32", name="c32")
            nc.vector.memset(c32[:], 0.0)

            banks = {}

            def emit_inject(s):
                tt, so = divmod(s, 4)
                rs = slice(32 * so, 32 * so + 32)
                pA = wps.tile([64, 512], FP, tag="pA", name="pA")
                pB = wps.tile([64, 512], FP, tag="pB", name="pB")
                banks[s] = (pA, pB)
                for ti, pt_ in ((0, pA), (1, pB)):
                    for half in range(2):
                        g4 = (2 * ti + half) * 512
                        nc.tensor.matmul(pt_[32 * half:32 * half + 32, :],
                                         eyeb[rs, :],
                                         xt[rs, tt, g4:g4 + 512],
                                         start=True, stop=(s == 0),
                                         tile_position=(32 * so, 32 * half),
                                         skip_group_check=True)

            emit_inject(0)
            for s in range(S):
                tt, so = divmod(s, 4)
                if so == 0 and tt + APRE < NT:
                    emit_a(tt + APRE)
                pA, pB = banks.pop(s)
                ring = rlist[s % 3]
                nring = rlist[(s + 1) % 3]
                if s > 0:
                    for ti, pt_ in ((0, pA), (1, pB)):
                        for q in range(2):
                            for half in range(2):
                                g4 = (2 * ti + half) * 512
                                win = slice(32, 96) if half == 0 else \
                                    slice(0, 64)
                                nc.tensor.matmul(
                                    pt_[:], ring[:, 2 * q:2 * q + 2, win],
                                    wh[:, q, :, g4:g4 + 512],
                                    perf_mode=DR, start=False,
                                    stop=(q == 1 and half == 1),
                                    skip_group_check=True)
                if s + 1 < S:
                    emit_inject(s + 1)
                # acts: pA = (f|i) sigmoid; pB = (o|g') sigmoid with the
                # g strip at 2x scale (tanh(x) = 2*sigmoid(2x)-1)
                fi = wtmp.tile([64, 512], BF, tag="fi", name="fi")
                og = wtmp.tile([64, 512], BF, tag="og", name="og")
                nc.scalar.activation(fi[:], pA[:], Sig, scale=1.0 / 16.0)
                nc.scalar.activation(og[:], pB[:], Sig, scale=scl[:])
                g0 = wtmp.tile([64, 512], BF, tag="g0", name="g0")
                nc.vector.tensor_scalar(g0[32:64, :], og[32:64, :], 2.0,
                                        -1.0, op0=mybir.AluOpType.mult,
                                        op1=mybir.AluOpType.add)
                fc = wtmp.tile([32, 512], BF, tag="fc", name="fc")
                nc.vector.tensor_mul(fc[:], fi[0:32, :], c32[:])
                ig = wtmp.tile([32, 512], BF, tag="ig", name="ig")
                nc.vector.tensor_mul(ig[:], fi[32:64, :], g0[32:64, :])
                nc.vector.tensor_add(c32[:], fc[:], ig[:])
                # transposed tail, all inputs at base partition 0
                co = wpt.tile([128, 2, 4, bl2], BF, tag="co", name="co")
                # o-transposes first: they depend only on the act and run
                # during the DVE cell chain instead of queueing behind the
                # c-transposes (which wait on the add) in the PE FIFO
                for kk in range(4):
                    nc.tensor.transpose(co[:, 1, kk, :],
                                        og[0:32, kk * 128:(kk + 1) * 128],
                                        eyeb[0:32, 0:bl2])
                for kk in range(4):
                    nc.tensor.transpose(co[:, 0, kk, :],
                                        c32[:, kk * 128:(kk + 1) * 128],
                                        eyeb[0:32, 0:bl2])
                tct = wtmp.tile([128, 4, bl2], BF, tag="tct", name="tct")
                nc.scalar.activation(tct[:], co[:, 0, :, :], TanhF)
                nc.vector.tensor_mul(nring[:, :, 32:64], tct[:],
                                     co[:, 1, :, :])
                nc.vector.tensor_mul(hsT[:, :, s, :], tct[:],
                                     co[:, 1, :, :])
            nc.sync.dma_start(
                d_hs.ap().rearrange("k p (s b) -> p k s b", b=bl2), hsT[:])
    nc.compile()
    return nc


def build_l3_v2(bl=BL):
    """MLP + log_softmax v2, data-parallel.

    W1/W2 fp8 DoubleRow (weights 16x, relu act rescales 1/16); h1 kept
    fp8 for W2's moving operand. log_softmax without max-subtraction
    (|logits| < 6 so exp is safe in f32): exp(logits+b3) on the act,
    partition-sum via a ones-stationary matmul, ln, broadcast back via
    ones matmul, and y = (logits + b3) - ln(sum) as one DVE STT reading
    the logits psum. No transposes; output is [chunk, OUT, 512].
    """
    nl = bl * S
    CH = 512
    NCH = nl // CH
    nc = bacc.Bacc("TRN2", target_bir_lowering=False, debug=False,
                   num_devices=NCORE)
    d_hs = nc.dram_tensor("hs8p", [128, 8, nl], F8T, kind="ExternalInput")
    d_W18 = nc.dram_tensor("W18", [128, 4, 2, 2, 128], F8T,
                           kind="ExternalInput")
    d_b1 = nc.dram_tensor("b1m", [128, 2], FP, kind="ExternalInput")
    d_W28 = nc.dram_tensor("W28", [128, 2, 2, 128], F8T,
                           kind="ExternalInput")
    d_b2 = nc.dram_tensor("b2m", [128, 2], FP, kind="ExternalInput")
    d_W3T = nc.dram_tensor("W3T", [2, 128, OUT], BF, kind="ExternalInput")
    d_b3 = nc.dram_tensor("b3m", [OUT, 1], FP, kind="ExternalInput")
    d_ones = nc.dram_tensor("onesL3", [OUT, OUT + 1], BF,
                            kind="ExternalInput")
    d_y = nc.dram_tensor("y", [NCH, OUT, CH], FP, kind="ExternalOutput")

    with tile.TileContext(nc) as tc:
        with ExitStack() as c3:
            mw = c3.enter_context(tc.tile_pool(name="mweights", bufs=1))
            mact = c3.enter_context(tc.tile_pool(name="mact", bufs=1))
            mtmp = c3.enter_context(tc.tile_pool(name="mtmp", bufs=3))
            mps = c3.enter_context(tc.tile_pool(name="mpsum", bufs=2,
                                                space="PSUM"))
            sps = c3.enter_context(tc.tile_pool(name="spsum", bufs=1,
                                                space="PSUM"))
            W18 = mw.tile([128, 4, 2, 2, 128], F8T, tag="W18", name="W18")
            W28 = mw.tile([128, 2, 2, 128], F8T, tag="W28", name="W28")
            W3 = mw.tile([128, 2, OUT], BF, tag="W3", name="W3")
            b1 = mw.tile([128, 2], FP, tag="b1", name="b1")
            b2 = mw.tile([128, 2], FP, tag="b2", name="b2")
            b3 = mw.tile([OUT, 1], FP, tag="b3", name="b3")
            onesb = mw.tile([OUT, OUT + 1], BF, tag="ones", name="ones")
            nc.sync.dma_start(W18[:], d_W18.ap()[:])
            nc.sync.dma_start(W28[:], d_W28.ap()[:])
            nc.sync.dma_start(W3[:], d_W3T.ap().rearrange("k p g -> p k g"))
            nc.sync.dma_start(b1[:], d_b1.ap()[:])
            nc.sync.dma_start(b2[:], d_b2.ap()[:])
            nc.sync.dma_start(b3[:], d_b3.ap()[:])
            nc.sync.dma_start(onesb[:], d_ones.ap()[:])
            hs = mw.tile([128, 8, nl], F8T, tag="hs", name="hs")
            for ci in range(NCH):
                cs = slice(ci * CH, (ci + 1) * CH)
                nc.sync.dma_start(hs[:, :, cs], d_hs.ap()[:, :, cs])
            h1 = mact.tile([128, 2, nl], F8T, tag="h1", name="h1")
            h2 = mact.tile([128, 2, nl], BF, tag="h2", name="h2")
            for ci in range(NCH):
                cs = slice(ci * CH, (ci + 1) * CH)
                for m in range(2):
                    p = mps.tile([128, CH], FP, tag="mp1", name="mp1")
                    for q in range(4):
                        nc.tensor.matmul(
                            p[:], W18[:, q, :, m, :], hs[:, 2 * q:2 * q + 2, cs],
                            start=(q == 0), stop=(q == 3),
                            perf_mode=DR, skip_group_check=True)
                    nc.scalar.activation(h1[:, m, cs], p[:], ReluF,
                                         bias=b1[:, m:m + 1],
                                         scale=1.0 / 16.0)
            for ci in range(NCH):
                cs = slice(ci * CH, (ci + 1) * CH)
                for m in range(2):
                    p = mps.tile([128, CH], FP, tag="mp2", name="mp2")
                    nc.tensor.matmul(p[:], W28[:, :, m, :], h1[:, :, cs],
                                     start=True, stop=True,
                                     perf_mode=DR, skip_group_check=True)
                    nc.scalar.activation(h2[:, m, cs], p[:], ReluF,
                                         bias=b2[:, m:m + 1],
                                         scale=1.0 / 16.0)
            for ci in range(NCH):
                cs = slice(ci * CH, (ci + 1) * CH)
                lgp = mps.tile([OUT, CH], FP, tag="mp3", name="mp3")
                for k in range(2):
                    nc.tensor.matmul(lgp[:], W3[:, k, :], h2[:, k, cs],
                                     start=(k == 0), stop=(k == 1),
                                     skip_group_check=True)
                ex = mtmp.tile([OUT, CH], BF, tag="ex", name="ex")
                nc.scalar.activation(ex[:], lgp[:], ExpF, bias=b3[:, 0:1])
                lg = mtmp.tile([OUT, CH], FP, tag="lg", name="lg")
                nc.scalar.activation(lg[:], lgp[:], IdentF, bias=b3[:, 0:1])
                smp = sps.tile([1, CH], FP, tag="smp", name="smp")
                nc.tensor.matmul(smp[:], onesb[:, 0:1], ex[:],
                                 start=True, stop=True,
                                 skip_group_check=True)
                lsm = mtmp.tile([1, CH], BF, tag="lsm", name="lsm")
                nc.scalar.activation(lsm[:], smp[:], LnF)
                lsb = sps.tile([OUT, CH], FP, tag="lsb", name="lsb")
                nc.tensor.matmul(lsb[:], onesb[0:1, 0:OUT], lsm[:],
                                 start=True, stop=True,
                                 skip_group_check=True)
                yt = mtmp.tile([OUT, CH], FP, tag="yt", name="yt")
                nc.vector.tensor_sub(yt[:], lg[:], lsb[:])
                nc.sync.dma_start(d_y.ap()[ci], yt[:])
    nc.compile()
    return nc


def build_l3(bl=BL):
    """MLP + log_softmax, data-parallel."""
    nl = bl * S
    nc = bacc.Bacc("TRN2", target_bir_lowering=False, debug=False,
                   num_devices=NCORE)
    d_hs = nc.dram_tensor("hsT8", [8, 128, nl], BF, kind="ExternalInput")
    d_W1T = nc.dram_tensor("W1T", [8, 128, 256], BF, kind="ExternalInput")
    d_b1 = nc.dram_tensor("b1m", [128, 2], FP, kind="ExternalInput")
    d_W2T = nc.dram_tensor("W2T", [2, 128, 256], BF, kind="ExternalInput")
    d_b2 = nc.dram_tensor("b2m", [128, 2], FP, kind="ExternalInput")
    d_W3T = nc.dram_tensor("W3T", [2, 128, OUT], BF, kind="ExternalInput")
    d_b3 = nc.dram_tensor("b3m", [OUT, 1], FP, kind="ExternalInput")
    d_eye = nc.dram_tensor("eye", [128, 128], FP, kind="ExternalInput")
    d_y = nc.dram_tensor("y", [nl, OUT], FP, kind="ExternalOutput")

    CH = min(512, nl)
    NCH = (nl + CH - 1) // CH

    with tile.TileContext(nc) as tc:
        with ExitStack() as c3:
            mw = c3.enter_context(tc.tile_pool(name="mweights", bufs=1))
            mact = c3.enter_context(tc.tile_pool(name="mact", bufs=1))
            mtmp = c3.enter_context(tc.tile_pool(name="mtmp", bufs=4))
            mps = c3.enter_context(tc.tile_pool(name="mpsum", bufs=2,
                                                space="PSUM"))
            sps = c3.enter_context(tc.tile_pool(name="spsum", bufs=2,
                                                space="PSUM"))
            eye_sb = mw.tile([128, 128], FP, tag="eye", name="eye")
            nc.sync.dma_start(eye_sb[:], d_eye.ap()[:])
            W1 = mw.tile([128, 8, 256], BF, tag="W1", name="W1")
            W2 = mw.tile([128, 2, 256], BF, tag="W2", name="W2")
            W3 = mw.tile([128, 2, OUT], BF, tag="W3", name="W3")
            b1 = mw.tile([128, 2], FP, tag="b1", name="b1")
            b2 = mw.tile([128, 2], FP, tag="b2", name="b2")
            b3 = mw.tile([OUT, 1], FP, tag="b3", name="b3")
            nc.sync.dma_start(W1[:], d_W1T.ap().rearrange("k p g -> p k g"))
            nc.sync.dma_start(W2[:], d_W2T.ap().rearrange("k p g -> p k g"))
            nc.sync.dma_start(W3[:], d_W3T.ap().rearrange("k p g -> p k g"))
            nc.sync.dma_start(b1[:], d_b1.ap()[:])
            nc.sync.dma_start(b2[:], d_b2.ap()[:])
            nc.sync.dma_start(b3[:], d_b3.ap()[:])
            hsT = [mw.tile([128, nl], BF, tag=f"hsT{k}", name=f"hsT{k}")
                   for k in range(8)]
            for ci in range((nl + 511) // 512):
                cs = slice(ci * 512, min(nl, (ci + 1) * 512))
                for k in range(8):
                    nc.sync.dma_start(hsT[k][:, cs], d_hs.ap()[k][:, cs])
            h1 = [mact.tile([128, nl], BF, tag=f"h1{m}", name=f"h1{m}")
                  for m in range(2)]
            h2 = [mact.tile([128, nl], BF, tag=f"h2{m}", name=f"h2{m}")
                  for m in range(2)]
            for ci in range(NCH):
                cs = slice(ci * CH, (ci + 1) * CH)
                for m in range(2):
                    p = mps.tile([128, CH], FP, tag="mp1", name="mp1")
                    for k in range(8):
                        nc.tensor.matmul(
                            p[:], W1[:, k, m * 128:(m + 1) * 128],
                            hsT[k][:, cs], start=(k == 0), stop=(k == 7))
                    nc.scalar.activation(h1[m][:, cs], p[:], ReluF,
                                         bias=b1[:, m:m + 1])
            for ci in range(NCH):
                cs = slice(ci * CH, (ci + 1) * CH)
                for m in range(2):
                    p = mps.tile([128, CH], FP, tag="mp2", name="mp2")
                    for k in range(2):
                        nc.tensor.matmul(
                            p[:], W2[:, k, m * 128:(m + 1) * 128],
                            h1[k][:, cs], start=(k == 0), stop=(k == 1))
                    nc.scalar.activation(h2[m][:, cs], p[:], ReluF,
                                         bias=b2[:, m:m + 1])
            # two passes batched by ACT function: all Exp, then all Ln,
            # so the Exp/Ln activation tables load once each instead of
            # per position-tile
            npt = max(1, nl // 128)
            lgs = [mact.tile([128, OUT], FP, tag=f"lgs{pi}", name=f"lgs{pi}")
                   for pi in range(npt)]
            nmxs = [mact.tile([128, 1], FP, tag=f"nmx{pi}", name=f"nmx{pi}")
                    for pi in range(npt)]
            sms = [mact.tile([128, 1], FP, tag=f"sm{pi}", name=f"sm{pi}")
                   for pi in range(npt)]
            for pi in range(npt):
                pcount = min(128, nl - pi * 128)
                psl = slice(pi * 128, pi * 128 + pcount)
                lg = mps.tile([OUT, pcount], FP, tag="mp3", name="mp3")
                for k in range(2):
                    nc.tensor.matmul(lg[:], W3[:, k, :], h2[k][:, psl],
                                     start=(k == 0), stop=(k == 1))
                lgb = mtmp.tile([OUT, pcount], FP, tag="lgb", name="lgb")
                nc.scalar.activation(lgb[:], lg[:], IdentF, bias=b3[:, 0:1])
                lgr = sps.tile([pcount, OUT], FP, tag="lgr", name="lgr")
                nc.tensor.transpose(lgr[:], lgb[:], eye_sb[0:OUT, 0:OUT])
                nc.vector.tensor_reduce(nmxs[pi][0:pcount, :], lgr[:],
                                        axis=mybir.AxisListType.X,
                                        op=mybir.AluOpType.max, negate=True)
                ex = mtmp.tile([pcount, OUT], FP, tag="ex", name="ex")
                nc.scalar.activation(ex[:], lgr[:], ExpF,
                                     bias=nmxs[pi][0:pcount, :],
                                     accum_out=sms[pi][0:pcount, :])
                nc.vector.tensor_copy(lgs[pi][0:pcount, :], lgr[:])
            for pi in range(npt):
                pcount = min(128, nl - pi * 128)
                psl = slice(pi * 128, pi * 128 + pcount)
                lsm = mtmp.tile([pcount, 1], FP, tag="lsm", name="lsm")
                nc.scalar.activation(lsm[:], sms[pi][0:pcount, :], LnF)
                shift = mtmp.tile([pcount, 1], FP, tag="shift", name="shift")
                nc.vector.tensor_sub(shift[:], nmxs[pi][0:pcount, :], lsm[:])
                yt = mtmp.tile([pcount, OUT], FP, tag="yt", name="yt")
                nc.vector.tensor_scalar(yt[:], lgs[pi][0:pcount, :],
                                        shift[:], None,
                                        op0=mybir.AluOpType.add)
                nc.sync.dma_start(d_y.ap()[psl, :], yt[:])
    nc.compile()
    return nc


def _prep_shared(inputs):
    f32 = np.float32
    cWxT = np.asarray(inputs["cW_ih"], f32).T
    cWx2 = np.zeros((128, 4, 128), f32)
    for j in range(2):
        for pi in range(2):
            cWx2[0:E, 2 * j + pi] = cWxT[:, (j + 4 * pi) * 128:
                                         (j + 4 * pi) * 128 + 128]
            cWx2[E:128, 2 * j + pi] = cWxT[:, (2 + j + 4 * pi) * 128:
                                           (2 + j + 4 * pi) * 128 + 128]
    cWhT = np.ascontiguousarray(
        np.asarray(inputs["cW_hh"], f32).T).reshape(2, 128, G4)
    cbias = (np.asarray(inputs["cb_ih"], f32)
             + np.asarray(inputs["cb_hh"], f32))
    cbias_m = np.ascontiguousarray(cbias.reshape(G4 // 128, 128).T)
    wW, wb = [], []
    for pre in ("f", "b"):
        wih = np.asarray(inputs[pre + "W_ih"], f32)
        whh = np.asarray(inputs[pre + "W_hh"], f32)
        wW.append(np.ascontiguousarray(
            np.concatenate([wih.T, whh.T], 0)).reshape(6, 128, WG))
        wb.append((np.asarray(inputs[pre + "b_ih"], f32)
                   + np.asarray(inputs[pre + "b_hh"], f32)).reshape(1, WG))
    W1T = np.ascontiguousarray(
        np.asarray(inputs["W1"], f32).T.astype(BF_NP)).reshape(8, 128, 256)
    b1m = np.ascontiguousarray(np.asarray(inputs["b1"], f32).reshape(2, 128).T)
    W2T = np.ascontiguousarray(
        np.asarray(inputs["W2"], f32).T.astype(BF_NP)).reshape(2, 128, 256)
    b2m = np.ascontiguousarray(np.asarray(inputs["b2"], f32).reshape(2, 128).T)
    W3T = np.ascontiguousarray(
        np.asarray(inputs["W3"], f32).T.astype(BF_NP)).reshape(2, 128, OUT)
    b3m = np.ascontiguousarray(np.asarray(inputs["b3"], f32).reshape(OUT, 1))
    eye = np.eye(128, dtype=f32)
    onesr = np.ones((1, 128), f32)
    return dict(cWx2=cWx2.astype(BF_NP), cWhT=cWhT.astype(BF_NP),
                cbias=cbias_m, wW=wW, wb=wb, W1T=W1T,
                b1m=b1m, W2T=W2T, b2m=b2m, W3T=W3T, b3m=b3m, eye=eye,
                onesr=onesr)


def _prep_l1_v2(inputs):
    f32 = np.float32
    wih = np.asarray(inputs["cW_ih"], f32)      # [1024, 64], order i,f,g,o
    whh = np.asarray(inputs["cW_hh"], f32)      # [1024, 256]
    b = (np.asarray(inputs["cb_ih"], f32) + np.asarray(inputs["cb_hh"], f32))
    sg = np.ones((4 * Hc, 1), f32) * 16.0
    sg[2 * Hc:3 * Hc] *= 2.0                    # g rows: tanh = 2*sig(2x)-1
    wihs, whhs, bs = wih * sg, whh * sg, b * sg[:, 0]
    # cWx65[k, j, gi, m]: k<64 -> W_ih[gate, k]; k=64 -> bias; gate = 256gi+128j+m
    cWx = np.zeros((65, 2, 4, 128), f32)
    g4 = wihs.reshape(4, 2, 128, 64)            # [gi, j, m, k]
    cWx[0:64] = g4.transpose(3, 1, 0, 2)
    cWx[64] = bs.reshape(4, 2, 128).transpose(1, 0, 2)
    # cWh8[p, j, r, gi, m] = whhs[256gi+128j+m, 128r+p]
    a = whhs.reshape(4, 2, 128, 2, 128)         # [gi, j, m, r, p]
    cWh8 = np.ascontiguousarray(a.transpose(4, 1, 3, 0, 2)).astype(F8_NP)
    return dict(cWx65=cWx.astype(BF_NP), cWh8=cWh8)


def _l1_maps_v2(inputs, sh, bl, ncores):
    x = np.asarray(inputs["x"])
    emb = np.asarray(inputs["emb"], np.float32)
    nl = bl * S
    maps = []
    for c in range(ncores):
        xc = x[c * bl:(c + 1) * bl].reshape(nl, Lc)
        lengths = (xc != 0).sum(axis=1).astype(np.float32)
        lenrep = np.ascontiguousarray(
            np.broadcast_to(lengths[None, :].astype(BF_NP), (128, nl)))
        eT = np.ones((Lc, 65, nl), np.float32)
        eT[:, 0:64, :] = emb[xc].transpose(1, 2, 0)
        maps.append(dict(eT65=np.ascontiguousarray(eT.astype(BF_NP)),
                         lenrep=lenrep, cWx65=sh["cWx65"],
                         cWh8=sh["cWh8"]))
    return maps


def _l1_maps(inputs, sh, bl, ncores):
    x = np.asarray(inputs["x"])
    emb = np.asarray(inputs["emb"], np.float32).astype(BF_NP)
    nl = bl * S
    maps = []
    for c in range(ncores):
        xc = x[c * bl:(c + 1) * bl].reshape(nl, Lc)
        lengths = (xc != 0).sum(axis=1).astype(np.float32)
        lenrep = np.ascontiguousarray(
            np.broadcast_to(lengths[None, :].astype(BF_NP), (128, nl)))
        eT = np.ascontiguousarray(emb[xc].transpose(1, 2, 0))
        maps.append(dict(eT=eT, lenrep=lenrep, cWx2=sh["cWx2"],
                         cWhT=sh["cWhT"], cbias=sh["cbias"]))
    return maps


@functools.lru_cache(maxsize=4)
def _modules(bl, lmin=18):
    return build_l1_v2(bl, lmin), build_l2_v5(32, 2), build_l3_v2(bl)


def _gate_blocks(w, order):
    h4 = w.shape[0] // 4
    return np.concatenate([w[g * h4:(g + 1) * h4] for g in order], axis=0)


ORD = (1, 0, 3, 2)   # PyTorch (i,f,g,o) -> (f,i,o,g)
F8_NP = ml_dtypes.float8_e4m3


def _prep_l2_v4(inputs):
    f32 = np.float32
    wh8, wx8, wb16 = [], [], []
    sc = np.full((4 * H, 1), 16.0, f32)
    sc[3 * H:] *= 2.0          # tanh(g) = 2*sig(2x)-1: fold the 2x here
    for pre in ("f", "b"):
        wih = _gate_blocks(np.asarray(inputs[pre + "W_ih"], f32), ORD)
        whh = _gate_blocks(np.asarray(inputs[pre + "W_hh"], f32), ORD)
        b = (_gate_blocks(np.asarray(inputs[pre + "b_ih"], f32), ORD)
             + _gate_blocks(np.asarray(inputs[pre + "b_hh"], f32), ORD))
        whhT = (whh * sc).T                      # [512, 2048]
        wihT = (wih * sc).T                      # [256, 2048]
        a = whhT.reshape(2, 2, 128, 16, 128)     # [q, r, p, gb, m]
        wh8.append(np.ascontiguousarray(
            a.transpose(2, 0, 3, 1, 4)).astype(F8_NP))
        a = wihT.reshape(2, 128, 16, 128)        # [r, p, gb, m]
        wx8.append(np.ascontiguousarray(
            a.transpose(1, 2, 0, 3)).astype(BF_NP))
        wb16.append(np.ascontiguousarray(
            (b * sc[:, 0]).reshape(1, 16, 128)).astype(BF_NP))
    return dict(wh8=wh8, wx8=wx8, wb16=wb16,
                ones32=np.ones((1, 32), BF_NP))


def _l2_maps_v4(last_full, sh2, ncores):
    maps = []
    half = ncores // 2
    for c in range(ncores):
        d = 0 if c < half else 1
        g = c % half
        lt = last_full[:, :, g * 32 * S:(g + 1) * 32 * S].reshape(
            2, 128, 32, S)
        if d == 1:
            lt = lt[:, :, :, ::-1]
        lt = lt.transpose(1, 0, 3, 2)            # [128, 2, S, 32]
        maps.append(dict(lastT16=np.ascontiguousarray(lt).astype(BF_NP),
                         wh8v4=sh2["wh8"][d], wx16v4=sh2["wx8"][d],
                         wb16=sh2["wb16"][d], ones32=sh2["ones32"]))
    return maps


def _prep_l2(inputs):
    f32 = np.float32
    wIT, wb, wh8 = [], [], []
    for pre in ("f", "b"):
        wih = _gate_blocks(np.asarray(inputs[pre + "W_ih"], f32), ORD)
        whh = _gate_blocks(np.asarray(inputs[pre + "W_hh"], f32), ORD)
        wIT.append(np.ascontiguousarray(wih.T.astype(BF_NP)).reshape(
            2, 128, 4 * H))
        wb.append((_gate_blocks(np.asarray(inputs[pre + "b_ih"], f32), ORD)
                   + _gate_blocks(np.asarray(inputs[pre + "b_hh"], f32),
                                  ORD)).reshape(1, 4 * H).astype(BF_NP))
        whhT = whh.T * 16.0
        wh8.append(np.ascontiguousarray(
            whhT.reshape(2, 2, 128, 4 * H).transpose(0, 2, 1, 3)
            .astype(F8_NP)))
    eyeb = np.zeros((128, 32), f32)
    for p in range(128):
        eyeb[p, p % 32] = 1.0
    scl64 = np.full((64, 1), 1.0 / 16.0, f32)
    scl64[32:64] = 2.0 / 16.0
    return dict(wIT=wIT, wb=wb, wh8=wh8, eyeb=eyeb.astype(BF_NP),
                scl64=scl64, onesr=np.ones((1, 128), f32).astype(BF_NP))


def _l2_maps_v3(last_full, sh2, ncores):
    maps = []
    half = ncores // 2
    for c in range(ncores):
        d = 0 if c < half else 1
        g = c % half
        lt = last_full[:, :, g * 32 * S:(g + 1) * 32 * S].reshape(
            2, 128, 32, S)
        if d == 1:
            lt = lt[:, :, :, ::-1]
        lt = np.ascontiguousarray(
            lt.transpose(0, 1, 3, 2).reshape(2, 128, 32 * S)).astype(BF_NP)
        maps.append(dict(lastT2=lt, wIT=sh2["wIT"][d], wb=sh2["wb"][d],
                         onesr=sh2["onesr"], eyeb=sh2["eyeb"],
                         scl64=sh2["scl64"], wh8=sh2["wh8"][d]))
    return maps


def _prep_l3_v2(inputs, sh):
    f32 = np.float32
    W1T = np.asarray(inputs["W1"], f32).T * 16.0       # [1024, 256]
    a = W1T.reshape(4, 2, 128, 2, 128)                 # [q, r, p, m, o]
    W18 = np.ascontiguousarray(a.transpose(2, 0, 1, 3, 4)).astype(F8_NP)
    W2T = np.asarray(inputs["W2"], f32).T * 16.0       # [256, 256]
    a = W2T.reshape(2, 128, 2, 128)                    # [r, p, m, o]
    W28 = np.ascontiguousarray(a.transpose(1, 0, 2, 3)).astype(F8_NP)
    ones = np.ones((OUT, OUT + 1), f32).astype(BF_NP)
    return dict(W18=W18, W28=W28, onesL3=ones, W3T=sh["W3T"],
                b1m=sh["b1m"], b2m=sh["b2m"], b3m=sh["b3m"])


def _l3_maps_v4(hs_f, hs_b, sh3, bl, ncores):
    nl = bl * S
    maps = []
    for c in range(ncores):
        g, hf = c // 2, c % 2
        sl = slice(hf * nl, (hf + 1) * nl)
        hs8 = np.concatenate([hs_f[g][:, :, sl], hs_b[g][:, :, sl]], axis=0)
        hs8p = np.ascontiguousarray(
            hs8.transpose(1, 0, 2)).astype(F8_NP)      # [128, 8, nl]
        maps.append(dict(hs8p=hs8p, W18=sh3["W18"], b1m=sh3["b1m"],
                         W28=sh3["W28"], b2m=sh3["b2m"], W3T=sh3["W3T"],
                         b3m=sh3["b3m"], onesL3=sh3["onesL3"]))
    return maps


def _l3_maps_v3(hs_f, hs_b, sh, bl, ncores):
    nl = bl * S
    maps = []
    for c in range(ncores):
        g, hf = c // 2, c % 2
        sl = slice(hf * nl, (hf + 1) * nl)
        hs8 = np.concatenate([hs_f[g][:, :, sl], hs_b[g][:, :, sl]], axis=0)
        maps.append(dict(hsT8=np.ascontiguousarray(hs8), W1T=sh["W1T"],
                         b1m=sh["b1m"], W2T=sh["W2T"], b2m=sh["b2m"],
                         W3T=sh["W3T"], b3m=sh["b3m"], eye=sh["eye"]))
    return maps


def _pipeline(inputs, bl, ncores, run_l1, run_l2, run_l3):
    sh = _prep_shared(inputs)
    sh2 = _prep_l2_v4(inputs)
    half = ncores // 2

    r1 = run_l1(_l1_maps_v2(inputs, _prep_l1_v2(inputs), bl, ncores))
    last_full = np.concatenate(
        [np.asarray(r1[c]["lastT"]).astype(np.float32)
         for c in range(ncores)], axis=2)

    r2 = run_l2(_l2_maps_v4(last_full, sh2, ncores))
    hs_f, hs_b = [], []
    for g in range(half):
        hs_f.append(np.asarray(r2[g]["hsTh"]))
        hb = np.asarray(r2[half + g]["hsTh"]).reshape(4, 128, S, 32)
        hs_b.append(np.ascontiguousarray(
            hb[:, :, ::-1, :]).reshape(4, 128, 32 * S))

    r3 = run_l3(_l3_maps_v4(hs_f, hs_b, _prep_l3_v2(inputs, sh),
                            bl, ncores))
    out = np.empty((B, S, OUT), np.float32)
    for c in range(ncores):
        yr = np.asarray(r3[c]["y"])                    # [NCH, OUT, CH]
        y = yr.transpose(0, 2, 1).reshape(-1, OUT).reshape(S // 2, 32, OUT)
        bs = 32 * (c // 2)
        ss = (S // 2) * (c % 2)
        out[bs:bs + 32, ss:ss + S // 2] = y.transpose(1, 0, 2)
    return out


def kernel(**inputs):
    x = np.asarray(inputs["x"])
    lmin = int((x.reshape(-1, Lc) != 0).sum(axis=1).min())
    l1, l2, l3 = _modules(BL, lmin)

    def runner(nc):
        def run(in_maps):
            res = bass_utils.run_bass_kernel_spmd(
                nc, in_maps, core_ids=list(range(NCORE)))
            return res.results
        return run

    return _pipeline(inputs, BL, NCORE, runner(l1), runner(l2), runner(l3))



# revision 18
# speedup vs baseline: 1.0788x; 1.0091x over previous
"""Trainium2 Bass kernel for nn_CharTaggerBiLSTM, 8-core SPMD, 3 launches.

L1 char LSTM: data-parallel over batch (16 sentences/core). Transposed
   layout (features-on-partitions), f32r matmuls; emits the masked last
   hidden state per word -> DRAM.
L2 word LSTM: one direction per core (cores 0-3 forward, 4-7 backward),
   32 sentences/core so each weight stream serves twice the rows.
   Direction is data: backward cores receive the char outputs with the
   sentence axis reversed on host and their outputs are un-reversed.
   x-part/bias matmuls for step s+1 are issued during step s's
   elementwise work to keep PE fed.
L3 MLP + log_softmax: data-parallel (16 sentences/core), bf16 GEMMs.

Host does embedding gather, weight reshapes, the two reshard steps, and
reassembly.
"""

import sys
import functools
from contextlib import ExitStack

sys.path.insert(0, "/opt/trn_rl_repo")

import numpy as np
import ml_dtypes
from concourse import bacc, bass, mybir, tile, bass_utils

BF_NP = ml_dtypes.bfloat16
U8 = mybir.dt.uint8
F8T = mybir.dt.float8e4
DR = mybir.MatmulPerfMode.DoubleRow
E = 64


B, S, Lc = 128, 128, 20
AB, E = 100, 64
Hc, H, OUT = 256, 512, 50
NCORE = 8
BL = B // NCORE            # sentences per core in L1/L3
FP = mybir.dt.float32
FR = mybir.dt.float32r
BF = mybir.dt.bfloat16
G4 = 4 * Hc
WG = 4 * H

Sig = mybir.ActivationFunctionType.Sigmoid
TanhF = mybir.ActivationFunctionType.Tanh
ReluF = mybir.ActivationFunctionType.Relu
ExpF = mybir.ActivationFunctionType.Exp
LnF = mybir.ActivationFunctionType.Ln
IdentF = mybir.ActivationFunctionType.Identity


def build_l1(bl=BL, lmin=18):
    """Char LSTM, data-parallel; writes lastT [2,128,nl] bf16 to DRAM."""
    nl = bl * S
    nc = bacc.Bacc("TRN2", target_bir_lowering=False, debug=False,
                   num_devices=NCORE)
    d_eT = nc.dram_tensor("eT", [Lc, E, nl], BF, kind="ExternalInput")
    d_lenrep = nc.dram_tensor("lenrep", [128, nl], BF, kind="ExternalInput")
    d_cWx2 = nc.dram_tensor("cWx2", [128, 4, 128], BF, kind="ExternalInput")
    d_cWhT = nc.dram_tensor("cWhT", [2, 128, G4], BF, kind="ExternalInput")
    d_cbias = nc.dram_tensor("cbias", [128, G4 // 128], FP,
                             kind="ExternalInput")
    d_last = nc.dram_tensor("lastT", [2, 128, nl], BF, kind="ExternalOutput")

    CH = 1024
    NCH = nl // CH
    GF = [Sig, Sig, TanhF, Sig]          # gate funcs for gi = i, f, g, o

    with tile.TileContext(nc) as tc:
        with ExitStack() as c1:
            cw = c1.enter_context(tc.tile_pool(name="cweights", bufs=1))
            cst = c1.enter_context(tc.tile_pool(name="cstate", bufs=1))
            ein = c1.enter_context(tc.tile_pool(name="ein", bufs=2))
            ctmp = c1.enter_context(tc.tile_pool(name="ctmp", bufs=3))
            cps = c1.enter_context(tc.tile_pool(name="cpsum", bufs=4,
                                                space="PSUM"))
            cWx2 = cw.tile([128, 4, 128], BF, tag="cWx2", name="cWx2")
            cWh = cw.tile([128, 2, G4], BF, tag="cWh", name="cWh")
            cb = cw.tile([128, G4 // 128], FP, tag="cb", name="cb")
            lenr = cw.tile([128, nl], BF, tag="lenr", name="lenr")
            nc.sync.dma_start(cWx2[:], d_cWx2.ap()[:])
            nc.sync.dma_start(cWh[:], d_cWhT.ap().rearrange("k p g -> p k g"))
            nc.sync.dma_start(cb[:], d_cbias.ap()[:])
            nc.sync.dma_start(lenr[:], d_lenrep.ap()[:])

            last = cst.tile([128, 2, nl], BF, tag="last", name="last")
            hh = [cst.tile([128, 2, nl], BF, tag=f"h{p}", name=f"h{p}")
                  for p in range(2)]
            cc = cst.tile([128, 2, nl], BF, tag="cc", name="cc")
            nc.vector.memset(cc[:], 0.0)
            nc.vector.memset(last[:], 0.0)

            for t in range(Lc):
                et2 = ein.tile([128, nl], BF, tag="et2", name="et2")
                nc.sync.dma_start(et2[0:E, :], d_eT.ap()[t])
                nc.sync.dma_start(et2[E:128, :], d_eT.ap()[t])
                masked = t >= lmin - 1
                hprev = hh[t % 2]
                hcur = hh[(t + 1) % 2]
                for ci in range(NCH):
                    cs = slice(ci * CH, (ci + 1) * CH)
                    if masked:
                        mk = ctmp.tile([128, CH], U8, tag="mk", name="mk")
                        nc.gpsimd.tensor_scalar(mk[:], lenr[:, cs], float(t),
                                                None,
                                                op0=mybir.AluOpType.is_gt)
                    for j in range(2):
                        ps = [cps.tile([128, CH], FP, tag="ps", name="ps")
                              for _ in range(4)]
                        # x-part: two K=64 row strips per PE pass, plus
                        # bias-free accumulation of the two h chunks;
                        # matmul outputs are split into 512-col halves
                        # (one PSUM bank each)
                        for hw_ in range(2):
                            o5 = slice(hw_ * 512, (hw_ + 1) * 512)
                            c5 = slice(ci * CH + hw_ * 512,
                                       ci * CH + (hw_ + 1) * 512)
                            for pi in range(2):
                                sl = 2 * j + pi
                                nc.tensor.matmul(ps[2 * pi][:, o5],
                                                 cWx2[0:E, sl, :],
                                                 et2[0:E, c5],
                                                 start=True, stop=(t == 0))
                                nc.tensor.matmul(ps[2 * pi + 1][:, o5],
                                                 cWx2[E:128, sl, :],
                                                 et2[E:128, c5],
                                                 start=True, stop=(t == 0))
                            if t > 0:
                                for gi in range(4):
                                    m = 2 * gi + j
                                    for k in range(2):
                                        nc.tensor.matmul(
                                            ps[gi][:, o5],
                                            cWh[:, k, m * 128:(m + 1) * 128],
                                            hprev[:, k, c5],
                                            start=False, stop=(k == 1))
                        gsb = [ctmp.tile([128, CH], BF, tag=f"g{gi}",
                                         name=f"g{gi}") for gi in range(4)]
                        for gi in range(4):
                            m = 2 * gi + j
                            nc.scalar.activation(gsb[gi][:], ps[gi][:],
                                                 GF[gi], bias=cb[:, m:m + 1])
                        ig = ctmp.tile([128, CH], BF, tag="ig", name="ig")
                        nc.vector.tensor_mul(ig[:], gsb[0][:], gsb[2][:])
                        fc = ctmp.tile([128, CH], BF, tag="fc", name="fc")
                        nc.vector.tensor_mul(fc[:], gsb[1][:], cc[:, j, cs])
                        nc.vector.tensor_add(cc[:, j, cs], fc[:], ig[:])
                        tct = ctmp.tile([128, CH], BF, tag="tct", name="tct")
                        nc.scalar.activation(tct[:], cc[:, j, cs], TanhF)
                        nc.vector.tensor_mul(hcur[:, j, cs], gsb[3][:],
                                             tct[:])
                        if masked:
                            nc.vector.copy_predicated(last[:, j, cs], mk[:],
                                                      hcur[:, j, cs])
            for j in range(2):
                nc.sync.dma_start(d_last.ap()[j], last[:, j, :])
    nc.compile()
    return nc


def build_l1_v2(bl=BL, lmin=18):
    """Char LSTM v2: act-engine-optimized.

    Psum per (word-chunk-512, j-half): [128, 4, 512] f32 (4 banks, one
    per gate type i,f,g,o; start=True only on each bank's first mm).
    x-part: K=65 bf16 (emb row 64 = ones -> bias rides in the weights,
    g rows pre-scaled 2x so one sigmoid serves tanh(g) as 2*sig(2x)-1).
    h-part: fp8 DoubleRow (h ring and W_hh both fp8, W scaled 16x, act
    scale 1/16 undoes it... n.b. x/bias must then also be 16x).
    One sigmoid act per (chunk, j) over all 4 gates; tails run j-wide
    on [128, 2048] slices; h*o on gpsimd.
    """
    nl = bl * S
    nc = bacc.Bacc("TRN2", target_bir_lowering=False, debug=False,
                   num_devices=NCORE)
    d_eT = nc.dram_tensor("eT65", [Lc, 65, nl], BF, kind="ExternalInput")
    d_lenrep = nc.dram_tensor("lenrep", [128, nl], BF, kind="ExternalInput")
    d_cWx = nc.dram_tensor("cWx65", [65, 2, 4, 128], BF,
                           kind="ExternalInput")
    d_cWh8 = nc.dram_tensor("cWh8", [128, 2, 2, 4, 128], F8T,
                            kind="ExternalInput")
    d_last = nc.dram_tensor("lastT", [2, 128, nl], BF, kind="ExternalOutput")

    CH = 512
    NCH = nl // CH
    with tile.TileContext(nc) as tc:
        with ExitStack() as c1:
            cw = c1.enter_context(tc.tile_pool(name="cweights", bufs=1))
            cst = c1.enter_context(tc.tile_pool(name="cstate", bufs=1))
            ein = c1.enter_context(tc.tile_pool(name="ein", bufs=2))
            ctmp = c1.enter_context(tc.tile_pool(name="ctmp", bufs=2))
            cps = c1.enter_context(tc.tile_pool(name="cpsum", bufs=2,
                                                space="PSUM"))
            cWx = cw.tile([65, 2, 4, 128], BF, tag="cWx", name="cWx")
            cWh = cw.tile([128, 2, 2, 4, 128], F8T, tag="cWh", name="cWh")
            lenr = cw.tile([128, nl], BF, tag="lenr", name="lenr")
            nc.sync.dma_start(cWx[:], d_cWx.ap()[:])
            nc.sync.dma_start(cWh[:], d_cWh8.ap()[:])
            nc.sync.dma_start(lenr[:], d_lenrep.ap()[:])

            last = cst.tile([128, 2, nl], BF, tag="last", name="last")
            # h ring: fp8 for the DR matmul + bf16 h for nothing else
            hh = [cst.tile([128, 2, nl], F8T, tag=f"h{p}", name=f"h{p}")
                  for p in range(2)]
            cc = cst.tile([128, 2, nl], BF, tag="cc", name="cc")
            gsb = [cst.tile([128, 4, nl], BF, tag=f"gs{j}", name=f"gs{j}")
                   for j in range(2)]
            g0t = [cst.tile([128, nl], BF, tag=f"g0{j}", name=f"g0{j}")
                   for j in range(2)]
            igt = [cst.tile([128, nl], BF, tag=f"ig{j}", name=f"ig{j}")
                   for j in range(2)]
            fct = [cst.tile([128, nl], BF, tag=f"fc{j}", name=f"fc{j}")
                   for j in range(2)]
            tctt = [cst.tile([128, nl], BF, tag=f"tc{j}", name=f"tc{j}")
                    for j in range(2)]
            nc.vector.memset(cc[:], 0.0)
            nc.vector.memset(last[:], 0.0)

            CK = 1024
            NCK = nl // CK

            def tail(j, ck, t, hcur, masked):
                ts_ = slice(ck * CK, (ck + 1) * CK)
                gj = gsb[j]
                nc.vector.tensor_scalar(g0t[j][:, ts_], gj[:, 2, ts_], 2.0,
                                        -1.0, op0=mybir.AluOpType.mult,
                                        op1=mybir.AluOpType.add)
                nc.vector.tensor_mul(igt[j][:, ts_], gj[:, 0, ts_],
                                     g0t[j][:, ts_])
                nc.vector.tensor_mul(fct[j][:, ts_], gj[:, 1, ts_],
                                     cc[:, j, ts_])
                nc.vector.tensor_add(cc[:, j, ts_], fct[j][:, ts_],
                                     igt[j][:, ts_])
                nc.scalar.activation(tctt[j][:, ts_], cc[:, j, ts_], TanhF)
                nc.gpsimd.tensor_mul(hcur[:, j, ts_], gj[:, 3, ts_],
                                     tctt[j][:, ts_])
                if masked:
                    mk = ctmp.tile([128, CK], U8, tag="mk", name="mk")
                    nc.gpsimd.tensor_scalar(mk[:], lenr[:, ts_], float(t),
                                            None,
                                            op0=mybir.AluOpType.is_gt)
                    hb = ctmp.tile([128, CK], BF, tag="hb", name="hb")
                    nc.vector.tensor_mul(hb[:], gj[:, 3, ts_],
                                         tctt[j][:, ts_])
                    nc.vector.copy_predicated(last[:, j, ts_], mk[:], hb[:])

            for t in range(Lc):
                et = ein.tile([65, nl], BF, tag="et", name="et")
                nc.sync.dma_start(et[:], d_eT.ap()[t])
                masked = t >= lmin - 1
                hprev = hh[t % 2]
                hcur = hh[(t + 1) % 2]
                for ci in range(NCH):
                    cs = slice(ci * CH, (ci + 1) * CH)
                    for j in range(2):
                        ps = cps.tile([128, 4, CH], FP, tag="ps", name="ps")
                        for gi in range(4):
                            nc.tensor.matmul(ps[:, gi, :],
                                             cWx[:, j, gi, :], et[:, cs],
                                             start=True,
                                             stop=(t == 0),
                                             skip_group_check=True)
                        if t > 0:
                            for gi in range(4):
                                nc.tensor.matmul(
                                    ps[:, gi, :], cWh[:, j, :, gi, :],
                                    hprev[:, :, cs],
                                    start=False, stop=True,
                                    perf_mode=DR, skip_group_check=True)
                        nc.scalar.activation(gsb[j][:, :, cs], ps[:], Sig,
                                             scale=1.0 / 16.0)
                    if ci % 2 == 1:
                        for j in range(2):
                            tail(j, ci // 2, t, hcur, masked)
            for j in range(2):
                nc.sync.dma_start(d_last.ap()[j], last[:, j, :])
    nc.compile()
    return nc


def build_l2_v4(bl2=32):
    """Word LSTM v4: gates-on-partitions, sentences-moving.

    Per step: psum tile [128, 16, 32] f32 (1 bank) holds all 2048 gates
    (16 blocks of 128 gate-dims on partitions) x 32 sentences on free.
    Seeded by bias (K=1 bf16) + x-part (fp8 DR vs last char state), then
    h-part (fp8 DR vs h^T ring) accumulates. One sigmoid act covers all
    gates (tanh(g) folded as 2*sig(2x)-1 with the 2x pre-scaled into the
    g rows host-side); cell tail on DVE; h^T ring written directly by
    the o*tanh(c) mul - no transposes, no inject.
    Gate type order: f, i, o, g (blocks 0:4, 4:8, 8:12, 12:16).
    """
    nl = bl2 * S
    nc = bacc.Bacc("TRN2", target_bir_lowering=False, debug=False,
                   num_devices=NCORE)
    d_lastT = nc.dram_tensor("lastT8", [128, 2, S, bl2], F8T,
                             kind="ExternalInput")
    d_wh8 = nc.dram_tensor("wh8v4", [128, 2, 16, 2, 128], F8T,
                           kind="ExternalInput")
    d_wx8 = nc.dram_tensor("wx8v4", [128, 16, 2, 128], F8T,
                           kind="ExternalInput")
    d_wb = nc.dram_tensor("wb16", [1, 16, 128], BF, kind="ExternalInput")
    d_ones = nc.dram_tensor("ones32", [1, bl2], BF, kind="ExternalInput")
    d_hs = nc.dram_tensor("hsTh", [4, 128, nl], BF, kind="ExternalOutput")

    with tile.TileContext(nc) as tc:
        with ExitStack() as c2:
            ww = c2.enter_context(tc.tile_pool(name="wweights", bufs=1))
            wst = c2.enter_context(tc.tile_pool(name="wstate", bufs=1))
            rgp = c2.enter_context(tc.tile_pool(name="wring", bufs=2))
            cpl = c2.enter_context(tc.tile_pool(name="wcell", bufs=2))
            sgp = c2.enter_context(tc.tile_pool(name="wsg", bufs=2))
            wtmp = c2.enter_context(tc.tile_pool(name="wtmp", bufs=2))
            wps = c2.enter_context(tc.tile_pool(name="wpsum", bufs=3,
                                                space="PSUM"))
            wh = ww.tile([128, 2, 16, 2, 128], F8T, tag="wh", name="wh")
            wx = ww.tile([128, 16, 2, 128], F8T, tag="wx", name="wx")
            wb = ww.tile([1, 16, 128], BF, tag="wb", name="wb")
            ones = ww.tile([1, bl2], BF, tag="ones", name="ones")
            lastT = ww.tile([128, 2, S, bl2], BF, tag="lastT", name="lastT")
            nc.sync.dma_start(wh[:], d_wh8.ap()[:])
            nc.sync.dma_start(wx[:], d_wx8.ap()[:])
            nc.sync.dma_start(wb[:], d_wb.ap()[:])
            nc.sync.dma_start(ones[:], d_ones.ap()[:])
            for sc in range(4):
                ss = slice(sc * (S // 4), (sc + 1) * (S // 4))
                nc.sync.dma_start(lastT[:, :, ss, :], d_lastT.ap()[:, :, ss, :])
            hsT = wst.tile([128, 4, S, bl2], BF, tag="hsT", name="hsT")

            rinit = [rgp.tile([128, 4, bl2], F8T, tag="ring", name=f"ri{i}")
                     for i in range(2)]
            for t_ in rinit:
                nc.vector.memset(t_[:], 0.0)
            ring_prev = rinit[1]
            cinit = [cpl.tile([128, 4, bl2], BF, tag="cc", name=f"ci{i}")
                     for i in range(2)]
            nc.vector.memset(cinit[1][:], 0.0)
            c_prev = cinit[1]

            for s in range(S):
                ps = wps.tile([128, 16, bl2], FP, tag="ps", name="ps")
                for gb in range(16):
                    nc.tensor.matmul(ps[:, gb, :], wb[:, gb, :], ones[:],
                                     start=True, stop=False,
                                     skip_group_check=True)
                for gb in range(16):
                    nc.tensor.matmul(ps[:, gb, :], wx[:, gb, :, :],
                                     lastT[:, :, s, :],
                                     start=False, stop=(s == 0),
                                     perf_mode=DR, skip_group_check=True)
                if s > 0:
                    for gb in range(16):
                        for q in range(2):
                            nc.tensor.matmul(
                                ps[:, gb, :], wh[:, q, gb, :, :],
                                ring_prev[:, 2 * q:2 * q + 2, :],
                                start=False, stop=(q == 1),
                                perf_mode=DR, skip_group_check=True)
                sg = sgp.tile([128, 16, bl2], BF, tag="sg", name="sg")
                nc.scalar.activation(sg[:], ps[:], Sig, scale=1.0 / 16.0)
                fco = wtmp.tile([128, 4, bl2], BF, tag="fco", name="fco")
                nc.vector.tensor_mul(fco[:], sg[:, 0:4, :], c_prev[:])
                tg = wtmp.tile([128, 4, bl2], BF, tag="tg", name="tg")
                nc.vector.scalar_tensor_tensor(
                    tg[:], sg[:, 12:16, :], 0.5, sg[:, 4:8, :],
                    op0=mybir.AluOpType.subtract, op1=mybir.AluOpType.mult)
                c_new = cpl.tile([128, 4, bl2], BF, tag="cc", name="cc")
                nc.vector.scalar_tensor_tensor(
                    c_new[:], tg[:], 2.0, fco[:],
                    op0=mybir.AluOpType.mult, op1=mybir.AluOpType.add)
                tct = wtmp.tile([128, 4, bl2], BF, tag="tct", name="tct")
                nc.scalar.activation(tct[:], c_new[:], TanhF)
                ring_new = rgp.tile([128, 4, bl2], F8T, tag="ring",
                                    name="ring")
                nc.vector.tensor_mul(ring_new[:], sg[:, 8:12, :], tct[:])
                nc.gpsimd.tensor_mul(hsT[:, :, s, :], sg[:, 8:12, :], tct[:])
                ring_prev = ring_new
                c_prev = c_new
            nc.sync.dma_start(
                d_hs.ap().rearrange("k p (s b) -> p k s b", b=bl2), hsT[:])
    nc.compile()
    return nc


def build_l2_v5(bl2=32, ngrp=2):
    """Word LSTM v5: like v4 but sentences split into ngrp interleaved
    groups with independent recurrence chains, so each group's
    (smaller) elementwise ops overlap the other group's matmuls."""
    nl = bl2 * S
    gw = bl2 // ngrp                 # sentences per group
    nc = bacc.Bacc("TRN2", target_bir_lowering=False, debug=False,
                   num_devices=NCORE)
    d_lastT = nc.dram_tensor("lastT16", [128, 2, S, bl2], BF,
                             kind="ExternalInput")
    d_wh8 = nc.dram_tensor("wh8v4", [128, 2, 16, 2, 128], F8T,
                           kind="ExternalInput")
    d_wx8 = nc.dram_tensor("wx16v4", [128, 16, 2, 128], BF,
                           kind="ExternalInput")
    d_wb = nc.dram_tensor("wb16", [1, 16, 128], BF, kind="ExternalInput")
    d_ones = nc.dram_tensor("ones32", [1, bl2], BF, kind="ExternalInput")
    d_hs = nc.dram_tensor("hsTh", [4, 128, nl], BF, kind="ExternalOutput")

    with tile.TileContext(nc) as tc:
        with ExitStack() as c2:
            ww = c2.enter_context(tc.tile_pool(name="wweights", bufs=1))
            wst = c2.enter_context(tc.tile_pool(name="wstate", bufs=1))
            rgp = c2.enter_context(tc.tile_pool(name="wring", bufs=2 * ngrp))
            cpl = c2.enter_context(tc.tile_pool(name="wcell", bufs=2 * ngrp))
            sgp = c2.enter_context(tc.tile_pool(name="wsg", bufs=2 * ngrp))
            wtmp = c2.enter_context(tc.tile_pool(name="wtmp", bufs=2 * ngrp))
            wps = c2.enter_context(tc.tile_pool(name="wpsum", bufs=3,
                                                space="PSUM"))
            wh = ww.tile([128, 2, 16, 2, 128], F8T, tag="wh", name="wh")
            wx = ww.tile([128, 16, 2, 128], BF, tag="wx", name="wx")
            wb = ww.tile([1, 16, 128], BF, tag="wb", name="wb")
            ones = ww.tile([1, bl2], BF, tag="ones", name="ones")
            lastT = ww.tile([128, 2, S, bl2], BF, tag="lastT", name="lastT")
            nc.sync.dma_start(wh[:], d_wh8.ap()[:])
            nc.sync.dma_start(wx[:], d_wx8.ap()[:])
            nc.sync.dma_start(wb[:], d_wb.ap()[:])
            nc.sync.dma_start(ones[:], d_ones.ap()[:])
            for sc in range(4):
                ss = slice(sc * (S // 4), (sc + 1) * (S // 4))
                nc.sync.dma_start(lastT[:, :, ss, :], d_lastT.ap()[:, :, ss, :])
            hsT = wst.tile([128, 4, S, bl2], BF, tag="hsT", name="hsT")

            c_prev, ring_prev = [], []
            for g in range(ngrp):
                ct = cpl.tile([128, 4, gw], BF, tag=f"cc{g}", name=f"ci{g}")
                nc.vector.memset(ct[:], 0.0)
                c_prev.append(ct)
                ring_prev.append(None)

            def step_mms(g, s):
                # psum tile is a full 2KB bank: start=True zeroes the whole
                # bank, so exactly one matmul (first bias) carries start.
                gs = slice(g * gw, (g + 1) * gw)
                ps = wps.tile([128, 16, 32], FP, tag=f"ps{g}", name=f"ps{g}")
                for gb in range(16):
                    nc.tensor.matmul(ps[:, gb, 0:gw], wb[:, gb, :],
                                     ones[:, gs],
                                     start=(gb == 0), stop=False,
                                     skip_group_check=True)
                for gb in range(16):
                    for r in range(2):
                        nc.tensor.matmul(ps[:, gb, 0:gw], wx[:, gb, r, :],
                                         lastT[:, r, s, gs],
                                         start=False,
                                         stop=(s == 0 and gb == 15
                                               and r == 1),
                                         skip_group_check=True)
                if s > 0:
                    for gb in range(16):
                        for q in range(2):
                            nc.tensor.matmul(
                                ps[:, gb, 0:gw], wh[:, q, gb, :, :],
                                ring_prev[g][:, 2 * q:2 * q + 2, :],
                                start=False,
                                stop=(gb == 15 and q == 1),
                                perf_mode=DR, skip_group_check=True)
                return ps

            def step_tail(g, s, ps):
                gs = slice(g * gw, (g + 1) * gw)
                sg = sgp.tile([128, 16, gw], BF, tag=f"sg{g}", name=f"sg{g}")
                nc.scalar.activation(sg[:], ps[:, :, 0:gw], Sig,
                                     scale=1.0 / 16.0)
                fco = wtmp.tile([128, 4, gw], BF, tag=f"fco{g}",
                                name=f"fco{g}")
                nc.gpsimd.tensor_mul(fco[:], sg[:, 0:4, :], c_prev[g][:])
                tg = wtmp.tile([128, 4, gw], BF, tag=f"tg{g}", name=f"tg{g}")
                nc.vector.scalar_tensor_tensor(
                    tg[:], sg[:, 12:16, :], 0.5, sg[:, 4:8, :],
                    op0=mybir.AluOpType.subtract, op1=mybir.AluOpType.mult)
                c_new = cpl.tile([128, 4, gw], BF, tag=f"cc{g}",
                                 name=f"cc{g}")
                nc.vector.scalar_tensor_tensor(
                    c_new[:], tg[:], 2.0, fco[:],
                    op0=mybir.AluOpType.mult, op1=mybir.AluOpType.add)
                tct = wtmp.tile([128, 4, gw], BF, tag=f"tct{g}",
                                name=f"tct{g}")
                nc.scalar.activation(tct[:], c_new[:], TanhF)
                ring_new = rgp.tile([128, 4, gw], F8T, tag=f"ring{g}",
                                    name=f"ring{g}")
                nc.vector.tensor_mul(ring_new[:], sg[:, 8:12, :], tct[:])
                nc.vector.tensor_mul(hsT[:, :, s, gs], sg[:, 8:12, :],
                                     tct[:])
                ring_prev[g] = ring_new
                c_prev[g] = c_new

            pend = {}
            for s in range(S):
                for g in range(ngrp):
                    pend[g] = step_mms(g, s)
                    og = (g + 1) % ngrp
                    if (og, 'tail') in pend:
                        gg, ss, pp = pend.pop((og, 'tail'))
                        step_tail(gg, ss, pp)
                    pend[(g, 'tail')] = (g, s, pend[g])
            for g in range(ngrp):
                if (g, 'tail') in pend:
                    gg, ss, pp = pend.pop((g, 'tail'))
                    step_tail(gg, ss, pp)
            hsr = d_hs.ap().rearrange("k p (s b) -> p k s b", b=bl2)
            for sc in range(8):
                ss = slice(sc * (S // 8), (sc + 1) * (S // 8))
                nc.sync.dma_start(hsr[:, :, ss, :], hsT[:, :, ss, :])
    nc.compile()
    return nc


def build_l2(bl2=32, fp8=True):
    """Word LSTM v3: fp8 DoubleRow via half-pad windows, step-major."""
    nl = bl2 * S
    nc = bacc.Bacc("TRN2", target_bir_lowering=False, debug=False,
                   num_devices=NCORE)
    d_last = nc.dram_tensor("lastT2", [2, 128, nl], BF, kind="ExternalInput")
    d_wIT = nc.dram_tensor("wIT", [2, 128, WG], BF, kind="ExternalInput")
    d_wb = nc.dram_tensor("wb", [1, WG], BF, kind="ExternalInput")
    d_ones = nc.dram_tensor("onesr", [1, 128], BF, kind="ExternalInput")
    d_eyeb = nc.dram_tensor("eyeb", [128, 32], BF, kind="ExternalInput")
    d_scl = nc.dram_tensor("scl64", [64, 1], FP, kind="ExternalInput")
    d_wh = nc.dram_tensor("wh8", [2, 128, 2, WG], F8T, kind="ExternalInput")
    d_hs = nc.dram_tensor("hsTh", [4, 128, nl], BF, kind="ExternalOutput")
    NT = nl // 128
    IdF = mybir.ActivationFunctionType.Identity

    with tile.TileContext(nc) as tc:
        with ExitStack() as c2:
            ww = c2.enter_context(tc.tile_pool(name="wweights", bufs=1))
            wst = c2.enter_context(tc.tile_pool(name="wstate", bufs=1))
            wtmp = c2.enter_context(tc.tile_pool(name="wtmp", bufs=3))
            eyeb = ww.tile([128, 32], BF, tag="eyeb", name="eyeb")
            nc.sync.dma_start(eyeb[:], d_eyeb.ap()[:])
            ones = ww.tile([1, 128], BF, tag="ones", name="ones")
            nc.sync.dma_start(ones[:], d_ones.ap()[:])
            wbt = ww.tile([1, WG], BF, tag="wbt", name="wbt")
            nc.sync.dma_start(wbt[:], d_wb.ap()[:])
            scl = ww.tile([64, 1], FP, tag="scl", name="scl")
            nc.sync.dma_start(scl[:], d_scl.ap()[:])
            wh = ww.tile([128, 2, 2, WG], F8T, tag="wh", name="wh")
            nc.sync.dma_start(wh[:],
                              d_wh.ap().rearrange("q p i g -> p q i g"))
            xt = wst.tile([128, NT, WG], BF, tag="xt", name="xt")
            hsT = wst.tile([128, 4, S, bl2], BF, tag="hsT", name="hsT")

            lw = c2.enter_context(tc.tile_pool(name="lw", bufs=1))
            psA = c2.enter_context(tc.tile_pool(name="psA", bufs=2,
                                                space="PSUM"))
            lpool = c2.enter_context(tc.tile_pool(name="lpool", bufs=3))
            wIT = lw.tile([128, 2, WG], BF, tag="wIT", name="wIT")
            nc.sync.dma_start(wIT[:],
                              d_wIT.ap().rearrange("k p g -> p k g"))

            def emit_a(tt):
                ts = slice(tt * 128, (tt + 1) * 128)
                lt = lpool.tile([128, 2, 128], BF, tag="lt", name="lt")
                for j2 in range(2):
                    nc.sync.dma_start(lt[:, j2, :], d_last.ap()[j2][:, ts])
                for nch in range(4):
                    sl = slice(nch * 512, (nch + 1) * 512)
                    px = psA.tile([128, 512], FP, tag="px", name="px")
                    nc.tensor.matmul(px[:], ones[:, 0:128], wbt[:, sl],
                                     start=True, stop=False)
                    for j2 in range(2):
                        nc.tensor.matmul(px[:], lt[:, j2, :], wIT[:, j2, sl],
                                         start=False, stop=(j2 == 1))
                    if nch < 2:
                        nc.vector.tensor_scalar(xt[:, tt, sl], px[:], 16.0,
                                                None,
                                                op0=mybir.AluOpType.mult)
                    else:
                        nc.scalar.activation(xt[:, tt, sl], px[:], IdF,
                                             scale=16.0)

            APRE = 3
            for tt in range(APRE):
                emit_a(tt)

            wps = c2.enter_context(tc.tile_pool(name="wpsum", bufs=2,
                                                space="PSUM"))
            wpt = c2.enter_context(tc.tile_pool(name="wpt", bufs=2,
                                                space="PSUM"))
            rgp = c2.enter_context(tc.tile_pool(name="wring", bufs=1))
            # fp8 ring: h^T lives at cols 32-63 of a zero-padded window
            # tile; shifted 64-wide windows stack two gates per DR output
            rlist = []
            for ri in range(3):
                rt = rgp.tile([128, 4, 96], F8T, tag=f"r{ri}",
                              name=f"r{ri}")
                nc.vector.memset(rt[:], 0.0)
                rlist.append(rt)
            c32 = wst.tile([32, 512], BF, tag="c32", name="c32")
            nc.vector.memset(c32[:], 0.0)

            banks = {}

            def emit_inject(s):
                tt, so = divmod(s, 4)
                rs = slice(32 * so, 32 * so + 32)
                pA = wps.tile([64, 512], FP, tag="pA", name="pA")
                pB = wps.tile([64, 512], FP, tag="pB", name="pB")
                banks[s] = (pA, pB)
                for ti, pt_ in ((0, pA), (1, pB)):
                    for half in range(2):
                        g4 = (2 * ti + half) * 512
                        nc.tensor.matmul(pt_[32 * half:32 * half + 32, :],
                                         eyeb[rs, :],
                                         xt[rs, tt, g4:g4 + 512],
                                         start=True, stop=(s == 0),
                                         tile_position=(32 * so, 32 * half),
                                         skip_group_check=True)

            emit_inject(0)
            for s in range(S):
                tt, so = divmod(s, 4)
                if so == 0 and tt + APRE < NT:
                    emit_a(tt + APRE)
                pA, pB = banks.pop(s)
                ring = rlist[s % 3]
                nring = rlist[(s + 1) % 3]
                if s > 0:
                    for ti, pt_ in ((0, pA), (1, pB)):
                        for q in range(2):
                            for half in range(2):
                                g4 = (2 * ti + half) * 512
                                win = slice(32, 96) if half == 0 else \
                                    slice(0, 64)
                                nc.tensor.matmul(
                                    pt_[:], ring[:, 2 * q:2 * q + 2, win],
                                    wh[:, q, :, g4:g4 + 512],
                                    perf_mode=DR, start=False,
                                    stop=(q == 1 and half == 1),
                                    skip_group_check=True)
                if s + 1 < S:
                    emit_inject(s + 1)
                # acts: pA = (f|i) sigmoid; pB = (o|g') sigmoid with the
                # g strip at 2x scale (tanh(x) = 2*sigmoid(2x)-1)
                fi = wtmp.tile([64, 512], BF, tag="fi", name="fi")
                og = wtmp.tile([64, 512], BF, tag="og", name="og")
                nc.scalar.activation(fi[:], pA[:], Sig, scale=1.0 / 16.0)
                nc.scalar.activation(og[:], pB[:], Sig, scale=scl[:])
                g0 = wtmp.tile([64, 512], BF, tag="g0", name="g0")
                nc.vector.tensor_scalar(g0[32:64, :], og[32:64, :], 2.0,
                                        -1.0, op0=mybir.AluOpType.mult,
                                        op1=mybir.AluOpType.add)
                fc = wtmp.tile([32, 512], BF, tag="fc", name="fc")
                nc.vector.tensor_mul(fc[:], fi[0:32, :], c32[:])
                ig = wtmp.tile([32, 512], BF, tag="ig", name="ig")
                nc.vector.tensor_mul(ig[:], fi[32:64, :], g0[32:64, :])
                nc.vector.tensor_add(c32[:], fc[:], ig[:])
                # transposed tail, all inputs at base partition 0
                co = wpt.tile([128, 2, 4, bl2], BF, tag="co", name="co")
                # o-transposes first: they depend only on the act and run
                # during the DVE cell chain instead of queueing behind the
                # c-transposes (which wait on the add) in the PE FIFO
                for kk in range(4):
                    nc.tensor.transpose(co[:, 1, kk, :],
                                        og[0:32, kk * 128:(kk + 1) * 128],
                                        eyeb[0:32, 0:bl2])
                for kk in range(4):
                    nc.tensor.transpose(co[:, 0, kk, :],
                                        c32[:, kk * 128:(kk + 1) * 128],
                                        eyeb[0:32, 0:bl2])
                tct = wtmp.tile([128, 4, bl2], BF, tag="tct", name="tct")
                nc.scalar.activation(tct[:], co[:, 0, :, :], TanhF)
                nc.vector.tensor_mul(nring[:, :, 32:64], tct[:],
                                     co[:, 1, :, :])
                nc.vector.tensor_mul(hsT[:, :, s, :], tct[:],
                                     co[:, 1, :, :])
            nc.sync.dma_start(
                d_hs.ap().rearrange("k p (s b) -> p k s b", b=bl2), hsT[:])
    nc.compile()
    return nc


def build_l3_v2(bl=BL):
    """MLP + log_softmax v2, data-parallel.

    W1/W2 fp8 DoubleRow (weights 16x, relu act rescales 1/16); h1 kept
    fp8 for W2's moving operand. log_softmax without max-subtraction
    (|logits| < 6 so exp is safe in f32): exp(logits+b3) on the act,
    partition-sum via a ones-stationary matmul, ln, broadcast back via
    ones matmul, and y = (logits + b3) - ln(sum) as one DVE STT reading
    the logits psum. No transposes; output is [chunk, OUT, 512].
    """
    nl = bl * S
    CH = 512
    NCH = nl // CH
    nc = bacc.Bacc("TRN2", target_bir_lowering=False, debug=False,
                   num_devices=NCORE)
    d_hs = nc.dram_tensor("hs8p", [128, 8, nl], F8T, kind="ExternalInput")
    d_W18 = nc.dram_tensor("W18", [128, 4, 2, 2, 128], F8T,
                           kind="ExternalInput")
    d_b1 = nc.dram_tensor("b1m", [128, 2], FP, kind="ExternalInput")
    d_W28 = nc.dram_tensor("W28", [128, 2, 2, 128], F8T,
                           kind="ExternalInput")
    d_b2 = nc.dram_tensor("b2m", [128, 2], FP, kind="ExternalInput")
    d_W3T = nc.dram_tensor("W3T", [2, 128, OUT], BF, kind="ExternalInput")
    d_b3 = nc.dram_tensor("b3m", [OUT, 1], FP, kind="ExternalInput")
    d_ones = nc.dram_tensor("onesL3", [OUT, OUT + 1], BF,
                            kind="ExternalInput")
    d_y = nc.dram_tensor("y", [NCH, OUT, CH], FP, kind="ExternalOutput")

    with tile.TileContext(nc) as tc:
        with ExitStack() as c3:
            mw = c3.enter_context(tc.tile_pool(name="mweights", bufs=1))
            mact = c3.enter_context(tc.tile_pool(name="mact", bufs=1))
            mtmp = c3.enter_context(tc.tile_pool(name="mtmp", bufs=3))
            mps = c3.enter_context(tc.tile_pool(name="mpsum", bufs=2,
                                                space="PSUM"))
            sps = c3.enter_context(tc.tile_pool(name="spsum", bufs=1,
                                                space="PSUM"))
            W18 = mw.tile([128, 4, 2, 2, 128], F8T, tag="W18", name="W18")
            W28 = mw.tile([128, 2, 2, 128], F8T, tag="W28", name="W28")
            W3 = mw.tile([128, 2, OUT], BF, tag="W3", name="W3")
            b1 = mw.tile([128, 2], FP, tag="b1", name="b1")
            b2 = mw.tile([128, 2], FP, tag="b2", name="b2")
            b3 = mw.tile([OUT, 1], FP, tag="b3", name="b3")
            onesb = mw.tile([OUT, OUT + 1], BF, tag="ones", name="ones")
            nc.sync.dma_start(W18[:], d_W18.ap()[:])
            nc.sync.dma_start(W28[:], d_W28.ap()[:])
            nc.sync.dma_start(W3[:], d_W3T.ap().rearrange("k p g -> p k g"))
            nc.sync.dma_start(b1[:], d_b1.ap()[:])
            nc.sync.dma_start(b2[:], d_b2.ap()[:])
            nc.sync.dma_start(b3[:], d_b3.ap()[:])
            nc.sync.dma_start(onesb[:], d_ones.ap()[:])
            hs = mw.tile([128, 8, nl], F8T, tag="hs", name="hs")
            for ci in range(NCH):
                cs = slice(ci * CH, (ci + 1) * CH)
                nc.sync.dma_start(hs[:, :, cs], d_hs.ap()[:, :, cs])
            h1 = mact.tile([128, 2, nl], F8T, tag="h1", name="h1")
            h2 = mact.tile([128, 2, nl], BF, tag="h2", name="h2")
            for ci in range(NCH):
                cs = slice(ci * CH, (ci + 1) * CH)
                for m in range(2):
                    p = mps.tile([128, CH], FP, tag="mp1", name="mp1")
                    for q in range(4):
                        nc.tensor.matmul(
                            p[:], W18[:, q, :, m, :], hs[:, 2 * q:2 * q + 2, cs],
                            start=(q == 0), stop=(q == 3),
                            perf_mode=DR, skip_group_check=True)
                    nc.scalar.activation(h1[:, m, cs], p[:], ReluF,
                                         bias=b1[:, m:m + 1],
                                         scale=1.0 / 16.0)
            for ci in range(NCH):
                cs = slice(ci * CH, (ci + 1) * CH)
                for m in range(2):
                    p = mps.tile([128, CH], FP, tag="mp2", name="mp2")
                    nc.tensor.matmul(p[:], W28[:, :, m, :], h1[:, :, cs],
                                     start=True, stop=True,
                                     perf_mode=DR, skip_group_check=True)
                    nc.scalar.activation(h2[:, m, cs], p[:], ReluF,
                                         bias=b2[:, m:m + 1],
                                         scale=1.0 / 16.0)
            for ci in range(NCH):
                cs = slice(ci * CH, (ci + 1) * CH)
                lgp = mps.tile([OUT, CH], FP, tag="mp3", name="mp3")
                for k in range(2):
                    nc.tensor.matmul(lgp[:], W3[:, k, :], h2[:, k, cs],
                                     start=(k == 0), stop=(k == 1),
                                     skip_group_check=True)
                ex = mtmp.tile([OUT, CH], BF, tag="ex", name="ex")
                nc.scalar.activation(ex[:], lgp[:], ExpF, bias=b3[:, 0:1])
                lg = mtmp.tile([OUT, CH], FP, tag="lg", name="lg")
                nc.scalar.activation(lg[:], lgp[:], IdentF, bias=b3[:, 0:1])
                smp = sps.tile([1, CH], FP, tag="smp", name="smp")
                nc.tensor.matmul(smp[:], onesb[:, 0:1], ex[:],
                                 start=True, stop=True,
                                 skip_group_check=True)
                lsm = mtmp.tile([1, CH], BF, tag="lsm", name="lsm")
                nc.scalar.activation(lsm[:], smp[:], LnF)
                lsb = sps.tile([OUT, CH], FP, tag="lsb", name="lsb")
                nc.tensor.matmul(lsb[:], onesb[0:1, 0:OUT], lsm[:],
                                 start=True, stop=True,
                                 skip_group_check=True)
                yt = mtmp.tile([OUT, CH], FP, tag="yt", name="yt")
                nc.vector.tensor_sub(yt[:], lg[:], lsb[:])
                nc.sync.dma_start(d_y.ap()[ci], yt[:])
    nc.compile()
    return nc


def build_l3(bl=BL):
    """MLP + log_softmax, data-parallel."""
    nl = bl * S
    nc = bacc.Bacc("TRN2", target_bir_lowering=False, debug=False,
                   num_devices=NCORE)
    d_hs = nc.dram_tensor("hsT8", [8, 128, nl], BF, kind="ExternalInput")
    d_W1T = nc.dram_tensor("W1T", [8, 128, 256], BF, kind="ExternalInput")
    d_b1 = nc.dram_tensor("b1m", [128, 2], FP, kind="ExternalInput")
    d_W2T = nc.dram_tensor("W2T", [2, 128, 256], BF, kind="ExternalInput")
    d_b2 = nc.dram_tensor("b2m", [128, 2], FP, kind="ExternalInput")
    d_W3T = nc.dram_tensor("W3T", [2, 128, OUT], BF, kind="ExternalInput")
    d_b3 = nc.dram_tensor("b3m", [OUT, 1], FP, kind="ExternalInput")
    d_eye = nc.dram_tensor("eye", [128, 128], FP, kind="ExternalInput")
    d_y = nc.dram_tensor("y", [nl, OUT], FP, kind="ExternalOutput")

    CH = min(512, nl)
    NCH = (nl + CH - 1) // CH

    with tile.TileContext(nc) as tc:
        with ExitStack() as c3:
            mw = c3.enter_context(tc.tile_pool(name="mweights", bufs=1))
            mact = c3.enter_context(tc.tile_pool(name="mact", bufs=1))
            mtmp = c3.enter_context(tc.tile_pool(name="mtmp", bufs=4))
            mps = c3.enter_context(tc.tile_pool(name="mpsum", bufs=2,
                                                space="PSUM"))
            sps = c3.enter_context(tc.tile_pool(name="spsum", bufs=2,
                                                space="PSUM"))
            eye_sb = mw.tile([128, 128], FP, tag="eye", name="eye")
            nc.sync.dma_start(eye_sb[:], d_eye.ap()[:])
            W1 = mw.tile([128, 8, 256], BF, tag="W1", name="W1")
            W2 = mw.tile([128, 2, 256], BF, tag="W2", name="W2")
            W3 = mw.tile([128, 2, OUT], BF, tag="W3", name="W3")
            b1 = mw.tile([128, 2], FP, tag="b1", name="b1")
            b2 = mw.tile([128, 2], FP, tag="b2", name="b2")
            b3 = mw.tile([OUT, 1], FP, tag="b3", name="b3")
            nc.sync.dma_start(W1[:], d_W1T.ap().rearrange("k p g -> p k g"))
            nc.sync.dma_start(W2[:], d_W2T.ap().rearrange("k p g -> p k g"))
            nc.sync.dma_start(W3[:], d_W3T.ap().rearrange("k p g -> p k g"))
            nc.sync.dma_start(b1[:], d_b1.ap()[:])
            nc.sync.dma_start(b2[:], d_b2.ap()[:])
            nc.sync.dma_start(b3[:], d_b3.ap()[:])
            hsT = [mw.tile([128, nl], BF, tag=f"hsT{k}", name=f"hsT{k}")
                   for k in range(8)]
            for ci in range((nl + 511) // 512):
                cs = slice(ci * 512, min(nl, (ci + 1) * 512))
                for k in range(8):
                    nc.sync.dma_start(hsT[k][:, cs], d_hs.ap()[k][:, cs])
            h1 = [mact.tile([128, nl], BF, tag=f"h1{m}", name=f"h1{m}")
                  for m in range(2)]
            h2 = [mact.tile([128, nl], BF, tag=f"h2{m}", name=f"h2{m}")
                  for m in range(2)]
            for ci in range(NCH):
                cs = slice(ci * CH, (ci + 1) * CH)
                for m in range(2):
                    p = mps.tile([128, CH], FP, tag="mp1", name="mp1")
                    for k in range(8):
                        nc.tensor.matmul(
                            p[:], W1[:, k, m * 128:(m + 1) * 128],
                            hsT[k][:, cs], start=(k == 0), stop=(k == 7))
                    nc.scalar.activation(h1[m][:, cs], p[:], ReluF,
                                         bias=b1[:, m:m + 1])
            for ci in range(NCH):
                cs = slice(ci * CH, (ci + 1) * CH)
                for m in range(2):
                    p = mps.tile([128, CH], FP, tag="mp2", name="mp2")
                    for k in range(2):
                        nc.tensor.matmul(
                            p[:], W2[:, k, m * 128:(m + 1) * 128],
                            h1[k][:, cs], start=(k == 0), stop=(k == 1))
                    nc.scalar.activation(h2[m][:, cs], p[:], ReluF,
                                         bias=b2[:, m:m + 1])
            # two passes batched by ACT function: all Exp, then all Ln,
            # so the Exp/Ln activation tables load once each instead of
            # per position-tile
            npt = max(1, nl // 128)
            lgs = [mact.tile([128, OUT], FP, tag=f"lgs{pi}", name=f"lgs{pi}")
                   for pi in range(npt)]
            nmxs = [mact.tile([128, 1], FP, tag=f"nmx{pi}", name=f"nmx{pi}")
                    for pi in range(npt)]
            sms = [mact.tile([128, 1], FP, tag=f"sm{pi}", name=f"sm{pi}")
                   for pi in range(npt)]
            for pi in range(npt):
                pcount = min(128, nl - pi * 128)
                psl = slice(pi * 128, pi * 128 + pcount)
                lg = mps.tile([OUT, pcount], FP, tag="mp3", name="mp3")
                for k in range(2):
                    nc.tensor.matmul(lg[:], W3[:, k, :], h2[k][:, psl],
                                     start=(k == 0), stop=(k == 1))
                lgb = mtmp.tile([OUT, pcount], FP, tag="lgb", name="lgb")
                nc.scalar.activation(lgb[:], lg[:], IdentF, bias=b3[:, 0:1])
                lgr = sps.tile([pcount, OUT], FP, tag="lgr", name="lgr")
                nc.tensor.transpose(lgr[:], lgb[:], eye_sb[0:OUT, 0:OUT])
                nc.vector.tensor_reduce(nmxs[pi][0:pcount, :], lgr[:],
                                        axis=mybir.AxisListType.X,
                                        op=mybir.AluOpType.max, negate=True)
                ex = mtmp.tile([pcount, OUT], FP, tag="ex", name="ex")
                nc.scalar.activation(ex[:], lgr[:], ExpF,
                                     bias=nmxs[pi][0:pcount, :],
                                     accum_out=sms[pi][0:pcount, :])
                nc.vector.tensor_copy(lgs[pi][0:pcount, :], lgr[:])
            for pi in range(npt):
                pcount = min(128, nl - pi * 128)
                psl = slice(pi * 128, pi * 128 + pcount)
                lsm = mtmp.tile([pcount, 1], FP, tag="lsm", name="lsm")
                nc.scalar.activation(lsm[:], sms[pi][0:pcount, :], LnF)
                shift = mtmp.tile([pcount, 1], FP, tag="shift", name="shift")
                nc.vector.tensor_sub(shift[:], nmxs[pi][0:pcount, :], lsm[:])
                yt = mtmp.tile([pcount, OUT], FP, tag="yt", name="yt")
                nc.vector.tensor_scalar(yt[:], lgs[pi][0:pcount, :],
                                        shift[:], None,
                                        op0=mybir.AluOpType.add)
                nc.sync.dma_start(d_y.ap()[psl, :], yt[:])
    nc.compile()
    return nc


def _prep_shared(inputs):
    f32 = np.float32
    cWxT = np.asarray(inputs["cW_ih"], f32).T
    cWx2 = np.zeros((128, 4, 128), f32)
    for j in range(2):
        for pi in range(2):
            cWx2[0:E, 2 * j + pi] = cWxT[:, (j + 4 * pi) * 128:
                                         (j + 4 * pi) * 128 + 128]
            cWx2[E:128, 2 * j + pi] = cWxT[:, (2 + j + 4 * pi) * 128:
                                           (2 + j + 4 * pi) * 128 + 128]
    cWhT = np.ascontiguousarray(
        np.asarray(inputs["cW_hh"], f32).T).reshape(2, 128, G4)
    cbias = (np.asarray(inputs["cb_ih"], f32)
             + np.asarray(inputs["cb_hh"], f32))
    cbias_m = np.ascontiguousarray(cbias.reshape(G4 // 128, 128).T)
    wW, wb = [], []
    for pre in ("f", "b"):
        wih = np.asarray(inputs[pre + "W_ih"], f32)
        whh = np.asarray(inputs[pre + "W_hh"], f32)
        wW.append(np.ascontiguousarray(
            np.concatenate([wih.T, whh.T], 0)).reshape(6, 128, WG))
        wb.append((np.asarray(inputs[pre + "b_ih"], f32)
                   + np.asarray(inputs[pre + "b_hh"], f32)).reshape(1, WG))
    W1T = np.ascontiguousarray(
        np.asarray(inputs["W1"], f32).T.astype(BF_NP)).reshape(8, 128, 256)
    b1m = np.ascontiguousarray(np.asarray(inputs["b1"], f32).reshape(2, 128).T)
    W2T = np.ascontiguousarray(
        np.asarray(inputs["W2"], f32).T.astype(BF_NP)).reshape(2, 128, 256)
    b2m = np.ascontiguousarray(np.asarray(inputs["b2"], f32).reshape(2, 128).T)
    W3T = np.ascontiguousarray(
        np.asarray(inputs["W3"], f32).T.astype(BF_NP)).reshape(2, 128, OUT)
    b3m = np.ascontiguousarray(np.asarray(inputs["b3"], f32).reshape(OUT, 1))
    eye = np.eye(128, dtype=f32)
    onesr = np.ones((1, 128), f32)
    return dict(cWx2=cWx2.astype(BF_NP), cWhT=cWhT.astype(BF_NP),
                cbias=cbias_m, wW=wW, wb=wb, W1T=W1T,
                b1m=b1m, W2T=W2T, b2m=b2m, W3T=W3T, b3m=b3m, eye=eye,
                onesr=onesr)


def _prep_l1_v2(inputs):
    f32 = np.float32
    wih = np.asarray(inputs["cW_ih"], f32)      # [1024, 64], order i,f,g,o
    whh = np.asarray(inputs["cW_hh"], f32)      # [1024, 256]
    b = (np.asarray(inputs["cb_ih"], f32) + np.asarray(inputs["cb_hh"], f32))
    sg = np.ones((4 * Hc, 1), f32) * 16.0
    sg[2 * Hc:3 * Hc] *= 2.0                    # g rows: tanh = 2*sig(2x)-1
    wihs, whhs, bs = wih * sg, whh * sg, b * sg[:, 0]
    # cWx65[k, j, gi, m]: k<64 -> W_ih[gate, k]; k=64 -> bias; gate = 256gi+128j+m
    cWx = np.zeros((65, 2, 4, 128), f32)
    g4 = wihs.reshape(4, 2, 128, 64)            # [gi, j, m, k]
    cWx[0:64] = g4.transpose(3, 1, 0, 2)
    cWx[64] = bs.reshape(4, 2, 128).transpose(1, 0, 2)
    # cWh8[p, j, r, gi, m] = whhs[256gi+128j+m, 128r+p]
    a = whhs.reshape(4, 2, 128, 2, 128)         # [gi, j, m, r, p]
    cWh8 = np.ascontiguousarray(a.transpose(4, 1, 3, 0, 2)).astype(F8_NP)
    return dict(cWx65=cWx.astype(BF_NP), cWh8=cWh8)


def _l1_maps_v2(inputs, sh, bl, ncores):
    x = np.asarray(inputs["x"])
    emb = np.asarray(inputs["emb"], np.float32)
    nl = bl * S
    maps = []
    for c in range(ncores):
        xc = x[c * bl:(c + 1) * bl].reshape(nl, Lc)
        lengths = (xc != 0).sum(axis=1).astype(np.float32)
        lenrep = np.ascontiguousarray(
            np.broadcast_to(lengths[None, :].astype(BF_NP), (128, nl)))
        eT = np.ones((Lc, 65, nl), np.float32)
        eT[:, 0:64, :] = emb[xc].transpose(1, 2, 0)
        maps.append(dict(eT65=np.ascontiguousarray(eT.astype(BF_NP)),
                         lenrep=lenrep, cWx65=sh["cWx65"],
                         cWh8=sh["cWh8"]))
    return maps


def _l1_maps(inputs, sh, bl, ncores):
    x = np.asarray(inputs["x"])
    emb = np.asarray(inputs["emb"], np.float32).astype(BF_NP)
    nl = bl * S
    maps = []
    for c in range(ncores):
        xc = x[c * bl:(c + 1) * bl].reshape(nl, Lc)
        lengths = (xc != 0).sum(axis=1).astype(np.float32)
        lenrep = np.ascontiguousarray(
            np.broadcast_to(lengths[None, :].astype(BF_NP), (128, nl)))
        eT = np.ascontiguousarray(emb[xc].transpose(1, 2, 0))
        maps.append(dict(eT=eT, lenrep=lenrep, cWx2=sh["cWx2"],
                         cWhT=sh["cWhT"], cbias=sh["cbias"]))
    return maps


@functools.lru_cache(maxsize=4)
def _modules(bl, lmin=18):
    return build_l1_v2(bl, lmin), build_l2_v5(32, 2), build_l3_v2(bl)


def _gate_blocks(w, order):
    h4 = w.shape[0] // 4
    return np.concatenate([w[g * h4:(g + 1) * h4] for g in order], axis=0)


ORD = (1, 0, 3, 2)   # PyTorch (i,f,g,o) -> (f,i,o,g)
F8_NP = ml_dtypes.float8_e4m3


def _prep_l2_v4(inputs):
    f32 = np.float32
    wh8, wx8, wb16 = [], [], []
    sc = np.full((4 * H, 1), 16.0, f32)
    sc[3 * H:] *= 2.0          # tanh(g) = 2*sig(2x)-1: fold the 2x here
    for pre in ("f", "b"):
        wih = _gate_blocks(np.asarray(inputs[pre + "W_ih"], f32), ORD)
        whh = _gate_blocks(np.asarray(inputs[pre + "W_hh"], f32), ORD)
        b = (_gate_blocks(np.asarray(inputs[pre + "b_ih"], f32), ORD)
             + _gate_blocks(np.asarray(inputs[pre + "b_hh"], f32), ORD))
        whhT = (whh * sc).T                      # [512, 2048]
        wihT = (wih * sc).T                      # [256, 2048]
        a = whhT.reshape(2, 2, 128, 16, 128)     # [q, r, p, gb, m]
        wh8.append(np.ascontiguousarray(
            a.transpose(2, 0, 3, 1, 4)).astype(F8_NP))
        a = wihT.reshape(2, 128, 16, 128)        # [r, p, gb, m]
        wx8.append(np.ascontiguousarray(
            a.transpose(1, 2, 0, 3)).astype(BF_NP))
        wb16.append(np.ascontiguousarray(
            (b * sc[:, 0]).reshape(1, 16, 128)).astype(BF_NP))
    return dict(wh8=wh8, wx8=wx8, wb16=wb16,
                ones32=np.ones((1, 32), BF_NP))


def _l2_maps_v4(last_full, sh2, ncores):
    maps = []
    half = ncores // 2
    for c in range(ncores):
        d = 0 if c < half else 1
        g = c % half
        lt = last_full[:, :, g * 32 * S:(g + 1) * 32 * S].reshape(
            2, 128, 32, S)
        if d == 1:
            lt = lt[:, :, :, ::-1]
        lt = lt.transpose(1, 0, 3, 2)            # [128, 2, S, 32]
        maps.append(dict(lastT16=np.ascontiguousarray(lt).astype(BF_NP),
                         wh8v4=sh2["wh8"][d], wx16v4=sh2["wx8"][d],
                         wb16=sh2["wb16"][d], ones32=sh2["ones32"]))
    return maps


def _prep_l2(inputs):
    f32 = np.float32
    wIT, wb, wh8 = [], [], []
    for pre in ("f", "b"):
        wih = _gate_blocks(np.asarray(inputs[pre + "W_ih"], f32), ORD)
        whh = _gate_blocks(np.asarray(inputs[pre + "W_hh"], f32), ORD)
        wIT.append(np.ascontiguousarray(wih.T.astype(BF_NP)).reshape(
            2, 128, 4 * H))
        wb.append((_gate_blocks(np.asarray(inputs[pre + "b_ih"], f32), ORD)
                   + _gate_blocks(np.asarray(inputs[pre + "b_hh"], f32),
                                  ORD)).reshape(1, 4 * H).astype(BF_NP))
        whhT = whh.T * 16.0
        wh8.append(np.ascontiguousarray(
            whhT.reshape(2, 2, 128, 4 * H).transpose(0, 2, 1, 3)
            .astype(F8_NP)))
    eyeb = np.zeros((128, 32), f32)
    for p in range(128):
        eyeb[p, p % 32] = 1.0
    scl64 = np.full((64, 1), 1.0 / 16.0, f32)
    scl64[32:64] = 2.0 / 16.0
    return dict(wIT=wIT, wb=wb, wh8=wh8, eyeb=eyeb.astype(BF_NP),
                scl64=scl64, onesr=np.ones((1, 128), f32).astype(BF_NP))


def _l2_maps_v3(last_full, sh2, ncores):
    maps = []
    half = ncores // 2
    for c in range(ncores):
        d = 0 if c < half else 1
        g = c % half
        lt = last_full[:, :, g * 32 * S:(g + 1) * 32 * S].reshape(
            2, 128, 32, S)
        if d == 1:
            lt = lt[:, :, :, ::-1]
        lt = np.ascontiguousarray(
            lt.transpose(0, 1, 3, 2).reshape(2, 128, 32 * S)).astype(BF_NP)
        maps.append(dict(lastT2=lt, wIT=sh2["wIT"][d], wb=sh2["wb"][d],
                         onesr=sh2["onesr"], eyeb=sh2["eyeb"],
                         scl64=sh2["scl64"], wh8=sh2["wh8"][d]))
    return maps


def _prep_l3_v2(inputs, sh):
    f32 = np.float32
    W1T = np.asarray(inputs["W1"], f32).T * 16.0       # [1024, 256]
    a = W1T.reshape(4, 2, 128, 2, 128)                 # [q, r, p, m, o]
    W18 = np.ascontiguousarray(a.transpose(2, 0, 1, 3, 4)).astype(F8_NP)
    W2T = np.asarray(inputs["W2"], f32).T * 16.0       # [256, 256]
    a = W2T.reshape(2, 128, 2, 128)                    # [r, p, m, o]
    W28 = np.ascontiguousarray(a.transpose(1, 0, 2, 3)).astype(F8_NP)
    ones = np.ones((OUT, OUT + 1), f32).astype(BF_NP)
    return dict(W18=W18, W28=W28, onesL3=ones, W3T=sh["W3T"],
                b1m=sh["b1m"], b2m=sh["b2m"], b3m=sh["b3m"])


def _l3_maps_v4(hs_f, hs_b, sh3, bl, ncores):
    nl = bl * S
    maps = []
    for c in range(ncores):
        g, hf = c // 2, c % 2
        sl = slice(hf * nl, (hf + 1) * nl)
        hs8 = np.concatenate([hs_f[g][:, :, sl], hs_b[g][:, :, sl]], axis=0)
        hs8p = np.ascontiguousarray(
            hs8.transpose(1, 0, 2)).astype(F8_NP)      # [128, 8, nl]
        maps.append(dict(hs8p=hs8p, W18=sh3["W18"], b1m=sh3["b1m"],
                         W28=sh3["W28"], b2m=sh3["b2m"], W3T=sh3["W3T"],
                         b3m=sh3["b3m"], onesL3=sh3["onesL3"]))
    return maps


def _l3_maps_v3(hs_f, hs_b, sh, bl, ncores):
    nl = bl * S
    maps = []
    for c in range(ncores):
        g, hf = c // 2, c % 2
        sl = slice(hf * nl, (hf + 1) * nl)
        hs8 = np.concatenate([hs_f[g][:, :, sl], hs_b[g][:, :, sl]], axis=0)
        maps.append(dict(hsT8=np.ascontiguousarray(hs8), W1T=sh["W1T"],
                         b1m=sh["b1m"], W2T=sh["W2T"], b2m=sh["b2m"],
                         W3T=sh["W3T"], b3m=sh["b3m"], eye=sh["eye"]))
    return maps


def _pipeline(inputs, bl, ncores, run_l1, run_l2, run_l3):
    sh = _prep_shared(inputs)
    sh2 = _prep_l2_v4(inputs)
    half = ncores // 2

    r1 = run_l1(_l1_maps_v2(inputs, _prep_l1_v2(inputs), bl, ncores))
    last_full = np.concatenate(
        [np.asarray(r1[c]["lastT"]).astype(np.float32)
         for c in range(ncores)], axis=2)

    r2 = run_l2(_l2_maps_v4(last_full, sh2, ncores))
    hs_f, hs_b = [], []
    for g in range(half):
        hs_f.append(np.asarray(r2[g]["hsTh"]))
        hb = np.asarray(r2[half + g]["hsTh"]).reshape(4, 128, S, 32)
        hs_b.append(np.ascontiguousarray(
            hb[:, :, ::-1, :]).reshape(4, 128, 32 * S))

    r3 = run_l3(_l3_maps_v4(hs_f, hs_b, _prep_l3_v2(inputs, sh),
                            bl, ncores))
    out = np.empty((B, S, OUT), np.float32)
    for c in range(ncores):
        yr = np.asarray(r3[c]["y"])                    # [NCH, OUT, CH]
        y = yr.transpose(0, 2, 1).reshape(-1, OUT).reshape(S // 2, 32, OUT)
        bs = 32 * (c // 2)
        ss = (S // 2) * (c % 2)
        out[bs:bs + 32, ss:ss + S // 2] = y.transpose(1, 0, 2)
    return out


def kernel(**inputs):
    x = np.asarray(inputs["x"])
    lmin = int((x.reshape(-1, Lc) != 0).sum(axis=1).min())
    l1, l2, l3 = _modules(BL, lmin)

    def runner(nc):
        def run(in_maps):
            res = bass_utils.run_bass_kernel_spmd(
                nc, in_maps, core_ids=list(range(NCORE)))
            return res.results
        return run

    return _pipeline(inputs, BL, NCORE, runner(l1), runner(l2), runner(l3))

